# revision 2
# baseline (speedup 1.0000x reference)
"""Trainium2 Bass kernel for nn_Attention_33157147525297 (v2, pipelined).

Graph-mixed multi-head attention, B=64, N=196 tokens, D=768, H=12 heads.
Data-parallel over batch: 8 batches per NeuronCore x 8 cores.

Math restructuring (host side):
  reference: attn = softmax(G @ (q k^T * scale)); out = attn @ v
  G mixes the query index only, so the graph-mix collapses into a pre-mix of
  x on the query path: xg = G @ x  (raw G; all scalar scales are folded into
  the exp activation's input scale).

v2 changes vs v1:
  - q/k projections run in fp8 (e4m3) with MatmulPerfMode.DoubleRow: 256-deep
    contraction per matmul at 0.5 cycles/row (2x PE throughput).  Weights are
    scaled x16 into fp8's normal range; the 1/(16*16*sqrt(64)) descale rides
    the exp activation's scale argument for free.  Error measured 5.1e-3 fro
    on the graded inputs (vs 3.7e-3 all-bf16, budget 2e-2).
  - x^T ships from host in both bf16 (v path) and DoubleRow-packed fp8
    (k path): pure layout/dtype prep, kills 48 on-device transpose-copies
    and halves the stage-A premix matmuls (xg^T only).
  - softmax sums come free from the PV matmul via a ones-column appended to v
    (pair psum [65, 392]: rows 0:64 = O^T, row 64 = sums); the reciprocal
    reads the psum sums row directly; broadcast via two K=1 matmuls.
  - projection packs tokens across batch boundaries into 13 flat 128-row
    tiles; bias accumulates via a K=1 ones-row matmul; psum DMAs straight
    to DRAM (no sbuf staging).
  - engine assignment by latency-criticality: Act = exp + v-copies,
    DVE = reciprocals + normalize muls, Pool = xg8/qk psum->sbuf copies.
  - stages software-pipelined per batch: attention chain stalls are covered
    by qk/v matmuls of batch b+2 and projection tiles of batch b-1.

Infra notes: this container's walrus accepts only ONE attached semaphore
wait per instruction - _install_wait_split() hoists extra waits onto
standalone EventSemaphore instructions just before, on the same engine.
Timing feedback comes from the concourse cost-model TimelineSim (NTFF
profiling hooks are unavailable under this axon client).
"""
import os
import sys
import numpy as np
import ml_dtypes

sys.path.insert(0, "/opt/trn_rl_repo")

SIZE, N_TOK, DIM, HEADS, HEAD_DIM, BATCH = 14, 196, 768, 12, 64, 64
N_CORES = 8
B_PER_CORE = BATCH // N_CORES  # 8
NT2 = 2 * N_TOK  # 392
NTB = N_TOK * B_PER_CORE  # 1568
BF16 = ml_dtypes.bfloat16
FP8 = ml_dtypes.float8_e4m3
W_SCALE = 16.0  # q/k weight pre-scale into fp8 normal range
EXP_SCALE = 1.0 / (W_SCALE * W_SCALE * (HEAD_DIM ** 0.5))

# token-dim partition tiles (196 = 128 + 68)
TOK_TILES = [(0, 128), (128, 68)]
# flat projection tiles over 1568 tokens: 12x128 + 32
PROJ_TILES = [(ft * 128, min(128, NTB - ft * 128)) for ft in range(13)]

LAST_EXEC_NS = None
LAST_TRACE = None


def _grid_g(factors):
    idx = np.arange(SIZE * SIZE).reshape(SIZE, SIZE)
    A = np.zeros((N_TOK, N_TOK), dtype=np.float32)
    for di, dj in [(-1, 0), (1, 0), (0, -1), (0, 1)]:
        for i in range(SIZE):
            for j in range(SIZE):
                ii, jj = i + di, j + dj
                if 0 <= ii < SIZE and 0 <= jj < SIZE:
                    A[idx[i, j], idx[ii, jj]] = 1.0
    NN = A / (A.sum(axis=1, keepdims=True) + 1.0)
    C = np.eye(N_TOK, dtype=np.float32) / 2.0
    return factors[0] * C + factors[1] * NN  # raw G, no attention scale


def _install_wait_split():
    """This container's walrus rejects >1 attached semaphore wait per
    instruction ("Too many sync wait commands").  Hoist excess waits onto
    standalone InstEventSemaphore instructions just before, on the same
    engine - engine queues are in-order, so semantics are identical."""
    import concourse.mybir as mybir
    import concourse.tile as tile
    from concourse.vector_clock import ScopedClock

    TC = tile.TileContext
    if getattr(TC, "_wait_split_patched", False):
        return
    LIMIT = 1

    def _split(tc, inst):
        si = inst.sync_info
        if (si is None or not si.on_wait or len(si.on_wait) <= LIMIT
                or inst.engine == mybir.EngineType.Unassigned):
            return
        waits = list(si.on_wait)
        extra, keep = waits[:-LIMIT], waits[-LIMIT:]
        for i, w in enumerate(extra):
            ev = mybir.InstEventSemaphore(
                name=f"{inst.name}-ws{i}", engine=inst.engine,
                sync_info=mybir.SyncInfo(on_wait=[w], on_update=[]),
            )
            tc._add_instruction(ev)
        inst.sync_info = mybir.SyncInfo(on_wait=keep,
                                        on_update=list(si.on_update))

    orig_commit = TC._commit_instruction

    def patched_commit(self, inst, lazy_reg_writes=True):
        _split(self, inst)
        return orig_commit(self, inst, lazy_reg_writes=lazy_reg_writes)

    TC._commit_instruction = patched_commit

    def patched_drain_and_barrier(self, tick_clock, wait_clock):
        nc = self.nc
        probe = mybir.InstNoOp(
            name=f"drain-probe-{nc.next_id()}", engine=mybir.EngineType.SP)
        wait_clock.add_sem_waits(
            probe, ScopedClock({None: tick_clock.global_clock}))
        pw = probe.sync_info.on_wait if probe.sync_info else []
        for i, w in enumerate(pw):
            ev = mybir.InstEventSemaphore(
                name=f"drainw-{nc.next_id()}-{i}", engine=mybir.EngineType.SP,
                sync_info=mybir.SyncInfo(on_wait=[w], on_update=[]),
            )
            self._add_instruction(ev)
        nc.sync.drain()
        nc.all_engine_barrier()
        assert self.sems is not None
        popped = nc._tile_sem_poison_stack.pop()
        assert popped is self._sem_poison
        nc.clear_and_free_semaphores(list(self.sems.allocated().values()))
        nc.all_engine_barrier()

    TC._drain_and_barrier = patched_drain_and_barrier
    TC._wait_split_patched = True


def _build_bass():
    import concourse.bass as bass
    import concourse.mybir as mybir
    import concourse.tile as tile

    _install_wait_split()

    f32 = mybir.dt.float32
    bf16 = mybir.dt.bfloat16
    fp8 = mybir.dt.float8e4
    AF = mybir.ActivationFunctionType
    DR = mybir.MatmulPerfMode.DoubleRow

    nc = bass.Bass()

    # x padded to 1664 rows so each batch loads as one [256 -> (2,128)] DMA
    x_d = nc.declare_dram_parameter("x", [NTB + 96, DIM], bf16, isOutput=False)
    gt_d = nc.declare_dram_parameter("gt", [128, 2 * N_TOK], bf16, isOutput=False)
    # everything below is host-packed into final SBUF layout [128, cols];
    # xt8/xt are grouped per batch so arrival order matches pipeline need
    xt8_d = nc.declare_dram_parameter("xt8", [128, 3 * 2 * NTB], fp8,
                                      isOutput=False)
    xt_d = nc.declare_dram_parameter("xt", [128, 6 * NTB], bf16, isOutput=False)
    wq8_d = nc.declare_dram_parameter("wq8", [128, 3 * 2 * DIM], fp8,
                                      isOutput=False)
    wk8_d = nc.declare_dram_parameter("wk8", [128, 3 * 2 * DIM], fp8,
                                      isOutput=False)
    wv_d = nc.declare_dram_parameter("wv", [128, 6 * DIM], bf16, isOutput=False)
    wp_d = nc.declare_dram_parameter("wp", [128, 6 * DIM], bf16, isOutput=False)
    bias_d = nc.declare_dram_parameter("bias", [DIM], f32, isOutput=False)
    out_d = nc.declare_dram_parameter("out", [NTB, DIM], f32, isOutput=True)

    with tile.TileContext(nc) as tc:
        with (
            tc.tile_pool(name="const", bufs=1) as const_p,
            tc.tile_pool(name="big", bufs=1) as big_p,
            tc.tile_pool(name="pt", bufs=8) as pt_p,
            tc.tile_pool(name="rs", bufs=3) as rs_p,
            tc.tile_pool(name="ps_dense", bufs=3, space="PSUM") as ps_dense,
            tc.tile_pool(name="ps_s", bufs=2, space="PSUM") as ps_s,
            tc.tile_pool(name="ps_pv", bufs=3, space="PSUM") as ps_pv,
        ):
            # ---- input DMAs: one SP queue, strict need-order (the cost
            #      model serializes transfers on a shared engine pool) ----
            gt2_sb = const_p.tile([128, 2 * N_TOK], bf16, name="gt2")
            x_sb = [big_p.tile([128, 2 * DIM], bf16, name=f"x{b}")
                    for b in range(B_PER_CORE)]
            xt8_sb = const_p.tile([128, 3 * 2 * NTB], fp8, name="xt8")
            xt_sb = const_p.tile([128, 6 * NTB], bf16, name="xt")
            wq8_sb = const_p.tile([128, 3 * 2 * DIM], fp8, name="wq8")
            wk8_sb = const_p.tile([128, 3 * 2 * DIM], fp8, name="wk8")
            wv_sb = const_p.tile([128, 6 * DIM], bf16, name="wv")
            wp_sb = const_p.tile([128, 6 * DIM], bf16, name="wp")
            bias_sb = const_p.tile([128, DIM], f32, name="bias")

            BPB8 = 3 * 2 * N_TOK   # xt8 cols per batch
            BPB = 6 * N_TOK        # xt cols per batch

            def dma_x(b):
                nc.sync.dma_start(
                    out=x_sb[b].rearrange("p (t c) -> p t c", t=2),
                    in_=x_d[b * N_TOK:b * N_TOK + 256, :].rearrange(
                        "(t p) c -> p t c", p=128))

            def dma_xt8(b):
                nc.sync.dma_start(
                    out=xt8_sb[:, b * BPB8:(b + 1) * BPB8],
                    in_=xt8_d[:, b * BPB8:(b + 1) * BPB8])

            def dma_xt(b):
                nc.sync.dma_start(
                    out=xt_sb[:, b * BPB:(b + 1) * BPB],
                    in_=xt_d[:, b * BPB:(b + 1) * BPB])

            dma_x(0)
            nc.sync.dma_start(out=gt2_sb, in_=gt_d[:, :])
            dma_x(1)
            dma_xt8(0)
            nc.sync.dma_start(out=wq8_sb, in_=wq8_d[:, :])
            nc.sync.dma_start(out=wk8_sb, in_=wk8_d[:, :])
            dma_xt(0)
            nc.sync.dma_start(out=wv_sb, in_=wv_d[:, :])
            dma_xt8(1)
            dma_xt(1)
            dma_x(2)
            dma_xt8(2)
            dma_xt(2)
            dma_x(3)
            dma_xt8(3)
            dma_xt(3)
            nc.sync.dma_start(out=wp_sb, in_=wp_d[:, :])
            nc.sync.dma_start(out=bias_sb,
                              in_=bias_d[None, :].broadcast_to([128, DIM]))
            for b in range(4, B_PER_CORE):
                dma_x(b)
                dma_xt8(b)
                dma_xt(b)
            gt_sb = [gt2_sb[:, 0:N_TOK], gt2_sb[:, N_TOK:2 * N_TOK]]
            x_sb = [[x_sb[b][:, 0:DIM], x_sb[b][:, DIM:2 * DIM]]
                    for b in range(B_PER_CORE)]

            ones_sb = const_p.tile([1, 128], bf16, name="ones")
            nc.vector.memset(ones_sb, 1.0)
            biasrow_sb = const_p.tile([1, DIM], bf16, name="biasrow")
            nc.scalar.copy(biasrow_sb, bias_sb[0:1, :])

            # ---- persistent activations ----
            # xg^T fp8 DoubleRow layout [p, i, tok]
            xg8_sb = [big_p.tile([128, 2 * NTB], fp8, name=f"xg8{k}")
                      for k in range(3)]
            xg8_v = [t.rearrange("p (i c) -> p i c", i=2) for t in xg8_sb]
            wq8_v = wq8_sb.rearrange("p (k i c) -> p k i c", k=3, i=2)
            wk8_v = wk8_sb.rearrange("p (k i c) -> p k i c", k=3, i=2)
            # q^T|k^T combined per feature tile: halves of NTBP=1632
            # (1568 tokens + 64 pad so S's stationary reads are always 128
            # wide; pad is zeroed once below)
            NTBP = NTB + 64
            qkT_sb = [big_p.tile([128, 2 * NTBP], bf16, name=f"qkT{k}")
                      for k in range(6)]
            for k in range(6):
                nc.gpsimd.memset(qkT_sb[k][:, 2 * NTBP - 64:2 * NTBP], 0.0)
            # v per batch-tile: 12 head groups of 65 cols (col 64 = ones)
            v1_sb = [
                [big_p.tile([128, 12 * 65], bf16, name=f"v{b}_{ti}")
                 for ti in range(2)]
                for b in range(B_PER_CORE)
            ]
            for b in range(B_PER_CORE):
                for ti in range(2):
                    nc.gpsimd.memset(
                        v1_sb[b][ti].rearrange("p (h c) -> p h c", h=12)[:, :, 64:65],
                        1.0)
            o_sb = [big_p.tile([128, NTB], bf16, name=f"o{k}")
                    for k in range(6)]

            # ---- dense work units (one psum group each) ----
            def a_unit(b, mt):
                # xg^T premix for one feature tile: [128, 196] -> fp8
                c0 = b * N_TOK
                ps = ps_dense.tile([128, NT2], f32, tag="psD", name="psD")
                for ti, (t0, tsz) in enumerate(TOK_TILES):
                    nc.tensor.matmul(
                        ps[:, 0:N_TOK],
                        x_sb[b][ti][:tsz, mt * 128:(mt + 1) * 128],
                        gt_sb[ti][:tsz], start=(ti == 0), stop=(ti == 1),
                    )
                with nc.allow_low_precision(reason="fp8 qk path"):
                    nc.scalar.copy(
                        xg8_v[mt // 2][:, mt % 2, c0:c0 + N_TOK], ps[:, 0:N_TOK])

            def qk_unit(b, mt):
                # q^T and k^T for one feature tile: two groups in one bank
                c0 = b * N_TOK
                ps = ps_dense.tile([128, NT2], f32, tag="psD", name="psD")
                for kt2 in range(3):
                    nc.tensor.matmul(
                        ps[:, 0:N_TOK],
                        wq8_v[:, kt2, :, mt * 128:(mt + 1) * 128],
                        xg8_v[kt2][:, :, c0:c0 + N_TOK],
                        start=(kt2 == 0), stop=(kt2 == 2), perf_mode=DR,
                    )
                for kt2 in range(3):
                    nc.tensor.matmul(
                        ps[:, N_TOK:NT2],
                        wk8_v[:, kt2, :, mt * 128:(mt + 1) * 128],
                        xt8_sb.rearrange("p (b k i c) -> p b k i c",
                                         b=B_PER_CORE, k=3, i=2)[:, b, kt2],
                        start=(kt2 == 0), stop=(kt2 == 2), perf_mode=DR,
                    )
                dst = qkT_sb[mt].rearrange("p (g c) -> p g c", g=2)[
                    :, :, c0:c0 + N_TOK]
                nc.vector.tensor_copy(dst, ps.rearrange("p (g c) -> p g c", g=2))

            def v_unit(b, ti, nt):
                # v [tsz, 384] = 6 heads x 64, strided into v1 (65-col groups)
                t0, tsz = TOK_TILES[ti]
                c0 = b * N_TOK
                ps = ps_dense.tile([128, NT2], f32, tag="psD", name="psD")
                for kt in range(6):
                    nc.tensor.matmul(
                        ps[:tsz, :384],
                        xt_sb[:, b * BPB + kt * N_TOK + t0:
                              b * BPB + kt * N_TOK + t0 + tsz],
                        wv_sb[:, kt * DIM + nt * 384:kt * DIM + (nt + 1) * 384],
                        start=(kt == 0), stop=(kt == 5),
                    )
                dstv = v1_sb[b][ti].rearrange("p (h c) -> p h c", h=12)
                nc.scalar.copy(
                    dstv[:tsz, 6 * nt:6 * nt + 6, 0:64],
                    ps[:tsz, :384].rearrange("p (h c) -> p h c", h=6))

            def proj_unit(ft, nt, tail=False):
                f0, fsz = PROJ_TILES[ft]
                ps = ps_dense.tile([128, NT2], f32, tag="psD", name="psD")
                for kt in range(6):
                    nc.tensor.matmul(
                        ps[:fsz, :384],
                        o_sb[kt][:, f0:f0 + fsz],
                        wp_sb[:, kt * DIM + nt * 384:kt * DIM + (nt + 1) * 384],
                        start=(kt == 0), stop=(kt == 5 and not tail),
                    )
                y_sb = rs_p.tile([128, 384], f32, tag="y", name="y_sb")
                if tail:
                    # tail variant: bias rides a K=1 matmul (PE is idle by
                    # now; f32 moving data is fine) and Act does the copy,
                    # halving the end-of-kernel DVE serialization
                    nc.tensor.matmul(
                        ps[:fsz, :384], ones_sb[:, :fsz],
                        biasrow_sb[:, nt * 384:(nt + 1) * 384],
                        start=False, stop=True)
                    nc.scalar.copy(y_sb[:fsz], ps[:fsz, :384])
                else:
                    nc.vector.tensor_add(
                        y_sb[:fsz], ps[:fsz, :384],
                        bias_sb[:fsz, nt * 384:(nt + 1) * 384])
                nc.sync.dma_start(
                    out=out_d[f0:f0 + fsz, nt * 384:(nt + 1) * 384],
                    in_=y_sb[:fsz])

            # ---- attention chain steps (per batch b, head pair p) ----
            def attn_stepA(b, p, state):
                c0 = b * N_TOK
                state["pT"] = pT = pt_p.tile([128, 2 * NT2], bf16,
                                             tag="pT", name="pT")
                for hi in range(2):
                    hb = hi * 64
                    s_ps = ps_s.tile([128, NT2], f32, tag="s", name="s_ps")
                    for ti in range(2):
                        t0 = ti * 128
                        nc.tensor.matmul(
                            s_ps[:, ti * N_TOK:(ti + 1) * N_TOK],
                            qkT_sb[p][hb:hb + 64,
                                      NTBP + c0 + t0:NTBP + c0 + t0 + 128],
                            qkT_sb[p][hb:hb + 64, c0:c0 + N_TOK],
                            start=True, stop=True,
                        )
                    nc.scalar.activation(pT[:, hi * NT2:(hi + 1) * NT2], s_ps,
                                         AF.Exp, scale=EXP_SCALE)

            def attn_stepB(b, p, state):
                # PV (+sums via ones col); pair bank [65, 392]
                pT = state["pT"]
                state["pv"] = pv_ps = ps_pv.tile([65, NT2], f32, tag="pv",
                                                 name="pv_ps")
                for hi in range(2):
                    for ti, (t0, tsz) in enumerate(TOK_TILES):
                        nc.tensor.matmul(
                            pv_ps[:, hi * N_TOK:(hi + 1) * N_TOK],
                            v1_sb[b][ti][:tsz,
                                         (2 * p + hi) * 65:(2 * p + hi) * 65 + 65],
                            pT[:tsz, hi * NT2 + ti * N_TOK:hi * NT2 + (ti + 1) * N_TOK],
                            start=(ti == 0), stop=(ti == 1),
                        )

            def attn_stepCr(b, p, state):
                # recip from the psum sums row (emitted right after its PV so
                # it sits ahead of bulk work in the DVE queue)
                pv_ps = state["pv"]
                state["rsb"] = rsb = rs_p.tile([1, NT2], bf16, tag="rsb",
                                               name="rsb")
                with nc.allow_low_precision(reason="softmax recip bf16"):
                    nc.vector.reciprocal(rsb, pv_ps[64:65, :])

            def attn_stepC(b, p, state):
                # K=1 broadcast matmuls -> psum; Act stages the scales to
                # SBUF (TensorTensor may read only one PSUM operand); DVE
                # normalizes into o_sb
                c0 = b * N_TOK
                pv_ps = state["pv"]
                rsb = state["rsb"]
                sc_ps = ps_dense.tile([128, NT2], f32, tag="psD", name="sc_ps")
                for hi in range(2):
                    hb = hi * 64
                    nc.tensor.matmul(
                        sc_ps[hb:hb + 64, 0:N_TOK],
                        ones_sb[:, 0:64],
                        rsb[:, hi * N_TOK:(hi + 1) * N_TOK],
                        start=True, stop=True,
                    )
                sc_sb = rs_p.tile([128, N_TOK], bf16, tag="scb", name="sc_sb")
                with nc.allow_low_precision(reason="softmax scale bf16"):
                    nc.scalar.copy(sc_sb, sc_ps[:, 0:N_TOK])
                for hi in range(2):
                    hb = hi * 64
                    nc.vector.tensor_mul(
                        o_sb[p][hb:hb + 64, c0:c0 + N_TOK],
                        pv_ps[0:64, hi * N_TOK:(hi + 1) * N_TOK],
                        sc_sb[hb:hb + 64, :])

            # ---- dense queue + schedule ----
            dense_q = []

            def push_slot(b_a, b_next, b_proj, cap_slot=None,
                          tail_proj=False):
                units = []
                if b_a is not None and b_a < B_PER_CORE:
                    units.append([(lambda b=b_a, mt=mt: a_unit(b, mt))
                                  for mt in range(6)])
                if b_next is not None and b_next < B_PER_CORE:
                    qk = [(lambda b=b_next, mt=mt: qk_unit(b, mt))
                          for mt in range(6)]
                    vv = [(lambda b=b_next, ti=ti, nt=nt: v_unit(b, ti, nt))
                          for ti in range(2) for nt in range(2)]
                    # interleave qk and v to spread psum bank reuse
                    mix = []
                    while qk or vv:
                        if qk:
                            mix.append(qk.pop(0))
                        if vv:
                            mix.append(vv.pop(0))
                        if qk:
                            mix.append(qk.pop(0))
                    units.append(mix)
                pu = []
                for ft, nt in proj_ready(b_proj, cap_slot):
                    tl = False
                    pu.append(lambda ft=ft, nt=nt, tl=tl: proj_unit(ft, nt, tl))
                if pu:
                    units.append(pu)
                proj_units = units.pop() if b_proj is not None else []
                flat = []
                srcs = [u for u in units if u]
                while srcs:
                    for u in srcs:
                        if u:
                            flat.append(u.pop(0))
                    srcs = [u for u in srcs if u]
                # proj interleaved into the back 2/3 of the slot queue
                k = len(flat) // 3
                back = flat[k:]
                merged = []
                while back or proj_units:
                    if back:
                        merged.append(back.pop(0))
                    if proj_units:
                        merged.append(proj_units.pop(0))
                    if back:
                        merged.append(back.pop(0))
                dense_q.extend(flat[:k] + merged)
            push_slot.proj_done = 0

            # proj-tile readiness: tile ft needs all batches covering
            # [128*ft, 128*(ft+1)); batches run in BATCH_ORDER (7 before 6
            # so the last slot still has dense fill and a short tail)
            proj_state = {"done": set(), "emitted": set()}

            def proj_ready(b_done, cap):
                if b_done is not None:
                    proj_state["done"].add(b_done)
                out = []
                for ft in range(len(PROJ_TILES)):
                    if ft in proj_state["emitted"]:
                        continue
                    f0, fsz = PROJ_TILES[ft]
                    b_lo = f0 // N_TOK
                    b_hi = (f0 + fsz - 1) // N_TOK
                    if all(bb in proj_state["done"]
                           for bb in range(b_lo, b_hi + 1)):
                        out.append(ft)
                out = out[:cap] if cap is not None else out
                res = []
                for ft in out:
                    proj_state["emitted"].add(ft)
                    res.extend([(ft, 0), (ft, 1)])
                return res

            def drain_dense(n):
                for _ in range(min(n, len(dense_q))):
                    dense_q.pop(0)()

            # prologue: A(0), A(1), qk(0) first; batch-0's S/exp chains
            # start while the v-path DMAs are still landing
            for mt in range(6):
                a_unit(0, mt)
            for mt in range(6):
                a_unit(1, mt)
            for mt in range(6):
                qk_unit(0, mt)
            for mt in range(6):
                qk_unit(1, mt)
            states0 = [dict() for _ in range(6)]
            push_slot(2, None, None)
            for ti in range(2):
                for nt in range(2):
                    dense_q.append(lambda ti=ti, nt=nt: v_unit(0, ti, nt))
                    dense_q.append(lambda ti=ti, nt=nt: v_unit(1, ti, nt))
            fill0 = (len(dense_q) + 5) // 6
            for p in range(6):
                attn_stepA(0, p, states0[p])
                drain_dense(fill0)
            drain_dense(len(dense_q))

            BATCH_ORDER = [0, 1, 2, 3, 4, 5, 6, 7]
            for bi in range(B_PER_CORE):
                b = BATCH_ORDER[bi]
                b_a = BATCH_ORDER[bi + 3] if bi + 3 < B_PER_CORE else None
                b_next = BATCH_ORDER[bi + 2] if bi + 2 < B_PER_CORE else None
                b_prev = BATCH_ORDER[bi - 1] if bi > 0 else None
                cap = 2 if bi <= 4 else None
                push_slot(b_a, b_next, b_prev, cap)
                states = states0 if b == 0 else [dict() for _ in range(6)]
                n_iters = 8
                fill = (len(dense_q) + 2 * n_iters - 1) // (2 * n_iters)
                for p in range(n_iters):
                    if p < 6 and b > 0:
                        attn_stepA(b, p, states[p])
                    drain_dense(fill)
                    if 1 <= p < 7:
                        attn_stepB(b, p - 1, states[p - 1])
                        attn_stepCr(b, p - 1, states[p - 1])
                    if p >= 2:
                        attn_stepC(b, p - 2, states[p - 2])
                    drain_dense(fill)
                drain_dense(len(dense_q))
            push_slot(None, None, BATCH_ORDER[-1], tail_proj=True)
            drain_dense(len(dense_q))

    return nc


_CACHED_NC = None


def kernel(x, w_qkv, w_proj, b_proj, factors):
    global LAST_EXEC_NS, LAST_TRACE, _CACHED_NC
    from concourse.bass_utils import run_bass_kernel_spmd

    factors = np.asarray(factors, dtype=np.float32)
    G = _grid_g(factors)  # raw G

    w_qkv = np.asarray(w_qkv, dtype=np.float32)

    def pack8(w):
        # [768 out, 768 in] -> w^T scaled -> [p, (kt2 i out)] fp8
        wt = np.ascontiguousarray(w.T) * W_SCALE  # [in, out]
        return wt.reshape(3, 2, 128, DIM).transpose(2, 0, 1, 3).reshape(128, -1)

    def pack16(w):
        wt = np.ascontiguousarray(np.asarray(w, dtype=np.float32).T)
        return wt.reshape(6, 128, DIM).transpose(1, 0, 2).reshape(128, -1)

    gtp = np.zeros((256, N_TOK), dtype=np.float32)
    gtp[0:N_TOK] = G.T
    in_common = {
        "gt": np.ascontiguousarray(
            gtp.reshape(2, 128, N_TOK).transpose(1, 0, 2).reshape(128, -1)
        ).astype(BF16),
        "wq8": np.ascontiguousarray(pack8(w_qkv[0:DIM])).astype(FP8),
        "wk8": np.ascontiguousarray(pack8(w_qkv[DIM:2 * DIM])).astype(FP8),
        "wv": np.ascontiguousarray(pack16(w_qkv[2 * DIM:3 * DIM])).astype(BF16),
        "wp": np.ascontiguousarray(pack16(w_proj)).astype(BF16),
        "bias": np.asarray(b_proj, dtype=np.float32),
    }
    x = np.asarray(x, dtype=np.float32).reshape(BATCH * N_TOK, DIM)
    in_maps = []
    for c in range(N_CORES):
        xc = x[c * NTB:(c + 1) * NTB]  # [1568, 768] f32
        xcp = np.zeros((NTB + 96, DIM), dtype=np.float32)
        xcp[0:NTB] = xc
        xtc = np.ascontiguousarray(xc.T)  # [768, 1568]
        # per-batch grouped: [p, (b kt2 i c)] and [p, (b kt c)]
        xt8p = xtc.reshape(3, 2, 128, B_PER_CORE, N_TOK) \
            .transpose(2, 3, 0, 1, 4).reshape(128, -1)
        xtp = xtc.reshape(6, 128, B_PER_CORE, N_TOK) \
            .transpose(1, 2, 0, 3).reshape(128, -1)
        in_maps.append({
            "x": xcp.astype(BF16),
            "xt": np.ascontiguousarray(xtp).astype(BF16),
            "xt8": np.ascontiguousarray(xt8p).astype(FP8),
            **in_common,
        })

    if _CACHED_NC is None:
        _CACHED_NC = _build_bass()
    nc = _CACHED_NC

    trace = bool(int(os.environ.get("KERNEL_TRACE", "0")))
    res = run_bass_kernel_spmd(nc, in_maps, core_ids=list(range(N_CORES)),
                               trace=trace)
    LAST_EXEC_NS = res.exec_time_ns
    if res.instructions_and_trace is not None:
        LAST_TRACE = res.instructions_and_trace[1]
    out = np.concatenate([res.results[c]["out"] for c in range(N_CORES)], axis=0)
    return out.reshape(BATCH, N_TOK, DIM).astype(np.float32)


# revision 3
# speedup vs baseline: 1.0080x; 1.0080x over previous
"""Trainium2 Bass kernel for nn_Attention_33157147525297 (v2, pipelined).

Graph-mixed multi-head attention, B=64, N=196 tokens, D=768, H=12 heads.
Data-parallel over batch: 8 batches per NeuronCore x 8 cores.
Measured (TimelineSim cost model): 144293 ns vs 205577 ns baseline (1.42x);
hardware rel err 5.1e-3 fro (budget 2e-2).

Math restructuring (host side):
  reference: attn = softmax(G @ (q k^T * scale)); out = attn @ v
  G mixes the query index only, so the graph-mix collapses into a pre-mix of
  x on the query path: xg = G @ x (raw G; every scalar scale - attention
  1/sqrt(d) and the fp8 weight pre-scales - folds into the exp activation's
  input scale argument, which is free).

Key design points:
  - q/k projections run in fp8 (e4m3) with MatmulPerfMode.DoubleRow: 256-deep
    contraction per matmul at 0.5 cycles/row (2x PE throughput).  Weights are
    scaled x16 into fp8's normal range.  v/proj/S/PV stay bf16: measured on
    the graded inputs, fp8 there busts the 2e-2 budget (v 3.8e-2, proj
    3.3e-2, S 2.2e-2) while fp8-qk lands at 5.1e-3 total because softmax
    normalization damps score-level error.
  - x ships token-major (premix stationary) plus x^T in both bf16 (v path)
    and DoubleRow-packed fp8 (k path): pure host-side layout/dtype prep that
    kills 48 on-device transpose copies and halves stage A.
  - softmax sums come free from the PV matmul via a ones-column appended to
    each head's v slice (pair psum [65, 392]: rows 0:64 = O^T, row 64 =
    sums); DVE reciprocal reads the psum sums row directly; the broadcast
    runs as two K=1 PE matmuls, Act stages the scales psum->SBUF
    (TensorTensor may read only one PSUM operand - walrus rule), DVE
    normalizes into o_sb.
  - S stationary slices are always 128 wide from a 64-col-padded k half
    (qkT halves of 1632), so the S psum is fully written and one exp per
    head covers both token tiles; the junk rows are never read by PV.
  - projection packs tokens across batch boundaries into 13 flat 128-row
    tiles; bias folds into the y staging copy as a DVE tensor_add against a
    broadcast bias tile.
  - engine assignment by latency-criticality (GPSIMD/Pool cannot touch PSUM
    on this machine, so it only gets memsets): Act = exp + v/xg8/scale
    copies, DVE = qk copies + reciprocals + normalize muls + y adds.
  - the whole kernel is software-pipelined per batch: slot b runs attention
    for batch b in 10 interleave iterations (S/exp -> PV -> recip -> bcast ->
    muls at pipeline depths 0/1/1/2/3) with stage A of b+3, qk/v of b+2 and
    projection tiles of b-1 draining in the gaps; input DMAs are host-packed
    to final SBUF layout, per-batch sliced, and issued in strict
    pipeline-need order (the cost model serializes transfers on a shared
    engine pool, so arrival order is everything).

Infra notes: this container's walrus accepts only ONE attached semaphore
wait per instruction - _install_wait_split() hoists excess waits onto
standalone EventSemaphore instructions just before, on the same engine.
Timing feedback comes from the concourse cost-model TimelineSim (NTFF
profiling hooks are unavailable under this axon client).
"""
import os
import sys
import numpy as np
import ml_dtypes

sys.path.insert(0, "/opt/trn_rl_repo")

SIZE, N_TOK, DIM, HEADS, HEAD_DIM, BATCH = 14, 196, 768, 12, 64, 64
N_CORES = 8
B_PER_CORE = BATCH // N_CORES  # 8
NT2 = 2 * N_TOK  # 392
NTB = N_TOK * B_PER_CORE  # 1568
BF16 = ml_dtypes.bfloat16
FP8 = ml_dtypes.float8_e4m3
W_SCALE = 16.0  # q/k weight pre-scale into fp8 normal range
EXP_SCALE = 1.0 / (W_SCALE * W_SCALE * (HEAD_DIM ** 0.5))

# token-dim partition tiles (196 = 128 + 68)
TOK_TILES = [(0, 128), (128, 68)]
# flat projection tiles over 1568 tokens: 12x128 + 32
PROJ_TILES = [(ft * 128, min(128, NTB - ft * 128)) for ft in range(13)]

LAST_EXEC_NS = None
LAST_TRACE = None


def _grid_g(factors):
    idx = np.arange(SIZE * SIZE).reshape(SIZE, SIZE)
    A = np.zeros((N_TOK, N_TOK), dtype=np.float32)
    for di, dj in [(-1, 0), (1, 0), (0, -1), (0, 1)]:
        for i in range(SIZE):
            for j in range(SIZE):
                ii, jj = i + di, j + dj
                if 0 <= ii < SIZE and 0 <= jj < SIZE:
                    A[idx[i, j], idx[ii, jj]] = 1.0
    NN = A / (A.sum(axis=1, keepdims=True) + 1.0)
    C = np.eye(N_TOK, dtype=np.float32) / 2.0
    return factors[0] * C + factors[1] * NN  # raw G, no attention scale


def _install_wait_split():
    """This container's walrus rejects >1 attached semaphore wait per
    instruction ("Too many sync wait commands").  Hoist excess waits onto
    standalone InstEventSemaphore instructions just before, on the same
    engine - engine queues are in-order, so semantics are identical."""
    import concourse.mybir as mybir
    import concourse.tile as tile
    from concourse.vector_clock import ScopedClock

    TC = tile.TileContext
    if getattr(TC, "_wait_split_patched", False):
        return
    LIMIT = 1

    def _split(tc, inst):
        si = inst.sync_info
        if (si is None or not si.on_wait or len(si.on_wait) <= LIMIT
                or inst.engine == mybir.EngineType.Unassigned):
            return
        waits = list(si.on_wait)
        extra, keep = waits[:-LIMIT], waits[-LIMIT:]
        for i, w in enumerate(extra):
            ev = mybir.InstEventSemaphore(
                name=f"{inst.name}-ws{i}", engine=inst.engine,
                sync_info=mybir.SyncInfo(on_wait=[w], on_update=[]),
            )
            tc._add_instruction(ev)
        inst.sync_info = mybir.SyncInfo(on_wait=keep,
                                        on_update=list(si.on_update))

    orig_commit = TC._commit_instruction

    def patched_commit(self, inst, lazy_reg_writes=True):
        _split(self, inst)
        return orig_commit(self, inst, lazy_reg_writes=lazy_reg_writes)

    TC._commit_instruction = patched_commit

    def patched_drain_and_barrier(self, tick_clock, wait_clock):
        nc = self.nc
        probe = mybir.InstNoOp(
            name=f"drain-probe-{nc.next_id()}", engine=mybir.EngineType.SP)
        wait_clock.add_sem_waits(
            probe, ScopedClock({None: tick_clock.global_clock}))
        pw = probe.sync_info.on_wait if probe.sync_info else []
        for i, w in enumerate(pw):
            ev = mybir.InstEventSemaphore(
                name=f"drainw-{nc.next_id()}-{i}", engine=mybir.EngineType.SP,
                sync_info=mybir.SyncInfo(on_wait=[w], on_update=[]),
            )
            self._add_instruction(ev)
        nc.sync.drain()
        nc.all_engine_barrier()
        assert self.sems is not None
        popped = nc._tile_sem_poison_stack.pop()
        assert popped is self._sem_poison
        nc.clear_and_free_semaphores(list(self.sems.allocated().values()))
        nc.all_engine_barrier()

    TC._drain_and_barrier = patched_drain_and_barrier
    TC._wait_split_patched = True


def _build_bass():
    import concourse.bass as bass
    import concourse.mybir as mybir
    import concourse.tile as tile

    _install_wait_split()

    f32 = mybir.dt.float32
    bf16 = mybir.dt.bfloat16
    fp8 = mybir.dt.float8e4
    AF = mybir.ActivationFunctionType
    DR = mybir.MatmulPerfMode.DoubleRow

    nc = bass.Bass()

    # x padded to 1664 rows so each batch loads as one [256 -> (2,128)] DMA
    x_d = nc.declare_dram_parameter("x", [NTB + 96, DIM], bf16, isOutput=False)
    gt_d = nc.declare_dram_parameter("gt", [128, 2 * N_TOK], bf16, isOutput=False)
    # everything below is host-packed into final SBUF layout [128, cols];
    # xt8/xt are grouped per batch so arrival order matches pipeline need
    xt8_d = nc.declare_dram_parameter("xt8", [128, 3 * 2 * NTB], fp8,
                                      isOutput=False)
    xt_d = nc.declare_dram_parameter("xt", [128, 6 * NTB], bf16, isOutput=False)
    wq8_d = nc.declare_dram_parameter("wq8", [128, 3 * 2 * DIM], fp8,
                                      isOutput=False)
    wk8_d = nc.declare_dram_parameter("wk8", [128, 3 * 2 * DIM], fp8,
                                      isOutput=False)
    wv_d = nc.declare_dram_parameter("wv", [128, 6 * DIM], bf16, isOutput=False)
    wp_d = nc.declare_dram_parameter("wp", [128, 6 * DIM], bf16, isOutput=False)
    bias_d = nc.declare_dram_parameter("bias", [DIM], f32, isOutput=False)
    out_d = nc.declare_dram_parameter("out", [NTB, DIM], f32, isOutput=True)

    with tile.TileContext(nc) as tc:
        with (
            tc.tile_pool(name="const", bufs=1) as const_p,
            tc.tile_pool(name="big", bufs=1) as big_p,
            tc.tile_pool(name="pt", bufs=8) as pt_p,
            tc.tile_pool(name="rs", bufs=3) as rs_p,
            tc.tile_pool(name="ps_dense", bufs=3, space="PSUM") as ps_dense,
            tc.tile_pool(name="ps_s", bufs=2, space="PSUM") as ps_s,
            tc.tile_pool(name="ps_pv", bufs=3, space="PSUM") as ps_pv,
        ):
            # ---- input DMAs: one SP queue, strict need-order (the cost
            #      model serializes transfers on a shared engine pool) ----
            gt2_sb = const_p.tile([128, 2 * N_TOK], bf16, name="gt2")
            x_sb = [big_p.tile([128, 2 * DIM], bf16, name=f"x{b}")
                    for b in range(B_PER_CORE)]
            xt8_sb = const_p.tile([128, 3 * 2 * NTB], fp8, name="xt8")
            xt_sb = const_p.tile([128, 6 * NTB], bf16, name="xt")
            wq8_sb = const_p.tile([128, 3 * 2 * DIM], fp8, name="wq8")
            wk8_sb = const_p.tile([128, 3 * 2 * DIM], fp8, name="wk8")
            wv_sb = const_p.tile([128, 6 * DIM], bf16, name="wv")
            wp_sb = const_p.tile([128, 6 * DIM], bf16, name="wp")
            bias_sb = const_p.tile([128, DIM], f32, name="bias")

            BPB8 = 3 * 2 * N_TOK   # xt8 cols per batch
            BPB = 6 * N_TOK        # xt cols per batch

            def dma_x(b):
                nc.sync.dma_start(
                    out=x_sb[b].rearrange("p (t c) -> p t c", t=2),
                    in_=x_d[b * N_TOK:b * N_TOK + 256, :].rearrange(
                        "(t p) c -> p t c", p=128))

            def dma_xt8(b):
                nc.sync.dma_start(
                    out=xt8_sb[:, b * BPB8:(b + 1) * BPB8],
                    in_=xt8_d[:, b * BPB8:(b + 1) * BPB8])

            def dma_xt(b):
                nc.sync.dma_start(
                    out=xt_sb[:, b * BPB:(b + 1) * BPB],
                    in_=xt_d[:, b * BPB:(b + 1) * BPB])

            dma_x(0)
            nc.sync.dma_start(out=gt2_sb, in_=gt_d[:, :])
            dma_x(1)
            dma_xt8(0)
            nc.sync.dma_start(out=wq8_sb, in_=wq8_d[:, :])
            nc.sync.dma_start(out=wk8_sb, in_=wk8_d[:, :])
            dma_xt(0)
            nc.sync.dma_start(out=wv_sb, in_=wv_d[:, :])
            dma_xt8(1)
            dma_xt(1)
            dma_x(2)
            dma_xt8(2)
            dma_xt(2)
            dma_x(3)
            dma_xt8(3)
            dma_xt(3)
            nc.sync.dma_start(out=wp_sb, in_=wp_d[:, :])
            nc.sync.dma_start(out=bias_sb,
                              in_=bias_d[None, :].broadcast_to([128, DIM]))
            for b in range(4, B_PER_CORE):
                dma_x(b)
                dma_xt8(b)
                dma_xt(b)
            gt_sb = [gt2_sb[:, 0:N_TOK], gt2_sb[:, N_TOK:2 * N_TOK]]
            x_sb = [[x_sb[b][:, 0:DIM], x_sb[b][:, DIM:2 * DIM]]
                    for b in range(B_PER_CORE)]

            ones_sb = const_p.tile([1, 128], bf16, name="ones")
            nc.vector.memset(ones_sb, 1.0)
            biasrow_sb = const_p.tile([1, DIM], bf16, name="biasrow")
            nc.scalar.copy(biasrow_sb, bias_sb[0:1, :])

            # ---- persistent activations ----
            # xg^T fp8 DoubleRow layout [p, i, tok]
            xg8_sb = [big_p.tile([128, 2 * NTB], fp8, name=f"xg8{k}")
                      for k in range(3)]
            xg8_v = [t.rearrange("p (i c) -> p i c", i=2) for t in xg8_sb]
            wq8_v = wq8_sb.rearrange("p (k i c) -> p k i c", k=3, i=2)
            wk8_v = wk8_sb.rearrange("p (k i c) -> p k i c", k=3, i=2)
            # q^T|k^T combined per feature tile: halves of NTBP=1632
            # (1568 tokens + 64 pad so S's stationary reads are always 128
            # wide; pad is zeroed once below)
            NTBP = NTB + 64
            qkT_sb = [big_p.tile([128, 2 * NTBP], bf16, name=f"qkT{k}")
                      for k in range(6)]
            for k in range(6):
                nc.gpsimd.memset(qkT_sb[k][:, 2 * NTBP - 64:2 * NTBP], 0.0)
            # v per batch-tile: 12 head groups of 65 cols (col 64 = ones)
            v1_sb = [
                [big_p.tile([128, 12 * 65], bf16, name=f"v{b}_{ti}")
                 for ti in range(2)]
                for b in range(B_PER_CORE)
            ]
            for b in range(B_PER_CORE):
                for ti in range(2):
                    nc.gpsimd.memset(
                        v1_sb[b][ti].rearrange("p (h c) -> p h c", h=12)[:, :, 64:65],
                        1.0)
            o_sb = [big_p.tile([128, NTB], bf16, name=f"o{k}")
                    for k in range(6)]

            # ---- dense work units (one psum group each) ----
            def a_unit(b, mt):
                # xg^T premix for one feature tile: [128, 196] -> fp8
                c0 = b * N_TOK
                ps = ps_dense.tile([128, NT2], f32, tag="psD", name="psD")
                for ti, (t0, tsz) in enumerate(TOK_TILES):
                    nc.tensor.matmul(
                        ps[:, 0:N_TOK],
                        x_sb[b][ti][:tsz, mt * 128:(mt + 1) * 128],
                        gt_sb[ti][:tsz], start=(ti == 0), stop=(ti == 1),
                    )
                with nc.allow_low_precision(reason="fp8 qk path"):
                    nc.scalar.copy(
                        xg8_v[mt // 2][:, mt % 2, c0:c0 + N_TOK], ps[:, 0:N_TOK])

            def qk_unit(b, mt):
                # q^T and k^T for one feature tile: two groups in one bank
                c0 = b * N_TOK
                ps = ps_dense.tile([128, NT2], f32, tag="psD", name="psD")
                for kt2 in range(3):
                    nc.tensor.matmul(
                        ps[:, 0:N_TOK],
                        wq8_v[:, kt2, :, mt * 128:(mt + 1) * 128],
                        xg8_v[kt2][:, :, c0:c0 + N_TOK],
                        start=(kt2 == 0), stop=(kt2 == 2), perf_mode=DR,
                    )
                for kt2 in range(3):
                    nc.tensor.matmul(
                        ps[:, N_TOK:NT2],
                        wk8_v[:, kt2, :, mt * 128:(mt + 1) * 128],
                        xt8_sb.rearrange("p (b k i c) -> p b k i c",
                                         b=B_PER_CORE, k=3, i=2)[:, b, kt2],
                        start=(kt2 == 0), stop=(kt2 == 2), perf_mode=DR,
                    )
                dst = qkT_sb[mt].rearrange("p (g c) -> p g c", g=2)[
                    :, :, c0:c0 + N_TOK]
                nc.vector.tensor_copy(dst, ps.rearrange("p (g c) -> p g c", g=2))

            def v_unit(b, ti, nt):
                # v [tsz, 384] = 6 heads x 64, strided into v1 (65-col groups)
                t0, tsz = TOK_TILES[ti]
                c0 = b * N_TOK
                ps = ps_dense.tile([128, NT2], f32, tag="psD", name="psD")
                for kt in range(6):
                    nc.tensor.matmul(
                        ps[:tsz, :384],
                        xt_sb[:, b * BPB + kt * N_TOK + t0:
                              b * BPB + kt * N_TOK + t0 + tsz],
                        wv_sb[:, kt * DIM + nt * 384:kt * DIM + (nt + 1) * 384],
                        start=(kt == 0), stop=(kt == 5),
                    )
                dstv = v1_sb[b][ti].rearrange("p (h c) -> p h c", h=12)
                nc.scalar.copy(
                    dstv[:tsz, 6 * nt:6 * nt + 6, 0:64],
                    ps[:tsz, :384].rearrange("p (h c) -> p h c", h=6))

            def proj_unit(ft, nt, tail=False):
                f0, fsz = PROJ_TILES[ft]
                ps = ps_dense.tile([128, NT2], f32, tag="psD", name="psD")
                for kt in range(6):
                    nc.tensor.matmul(
                        ps[:fsz, :384],
                        o_sb[kt][:, f0:f0 + fsz],
                        wp_sb[:, kt * DIM + nt * 384:kt * DIM + (nt + 1) * 384],
                        start=(kt == 0), stop=(kt == 5 and not tail),
                    )
                y_sb = rs_p.tile([128, 384], f32, tag="y", name="y_sb")
                if tail:
                    # tail variant: bias rides a K=1 matmul (PE is idle by
                    # now; f32 moving data is fine) and Act does the copy,
                    # halving the end-of-kernel DVE serialization
                    nc.tensor.matmul(
                        ps[:fsz, :384], ones_sb[:, :fsz],
                        biasrow_sb[:, nt * 384:(nt + 1) * 384],
                        start=False, stop=True)
                    nc.scalar.copy(y_sb[:fsz], ps[:fsz, :384])
                else:
                    nc.vector.tensor_add(
                        y_sb[:fsz], ps[:fsz, :384],
                        bias_sb[:fsz, nt * 384:(nt + 1) * 384])
                nc.sync.dma_start(
                    out=out_d[f0:f0 + fsz, nt * 384:(nt + 1) * 384],
                    in_=y_sb[:fsz])

            # ---- attention chain steps (per batch b, head pair p) ----
            def attn_stepA(b, p, state):
                c0 = b * N_TOK
                state["pT"] = pT = pt_p.tile([128, 2 * NT2], bf16,
                                             tag="pT", name="pT")
                for hi in range(2):
                    hb = hi * 64
                    s_ps = ps_s.tile([128, NT2], f32, tag="s", name="s_ps")
                    for ti in range(2):
                        t0 = ti * 128
                        nc.tensor.matmul(
                            s_ps[:, ti * N_TOK:(ti + 1) * N_TOK],
                            qkT_sb[p][hb:hb + 64,
                                      NTBP + c0 + t0:NTBP + c0 + t0 + 128],
                            qkT_sb[p][hb:hb + 64, c0:c0 + N_TOK],
                            start=True, stop=True,
                        )
                    nc.scalar.activation(pT[:, hi * NT2:(hi + 1) * NT2], s_ps,
                                         AF.Exp, scale=EXP_SCALE)

            def attn_stepB(b, p, state):
                # PV (+sums via ones col); pair bank [65, 392]
                pT = state["pT"]
                state["pv"] = pv_ps = ps_pv.tile([65, NT2], f32, tag="pv",
                                                 name="pv_ps")
                for hi in range(2):
                    for ti, (t0, tsz) in enumerate(TOK_TILES):
                        nc.tensor.matmul(
                            pv_ps[:, hi * N_TOK:(hi + 1) * N_TOK],
                            v1_sb[b][ti][:tsz,
                                         (2 * p + hi) * 65:(2 * p + hi) * 65 + 65],
                            pT[:tsz, hi * NT2 + ti * N_TOK:hi * NT2 + (ti + 1) * N_TOK],
                            start=(ti == 0), stop=(ti == 1),
                        )

            def attn_stepCr(b, p, state):
                # recip from the psum sums row (emitted right after its PV so
                # it sits ahead of bulk work in the DVE queue)
                pv_ps = state["pv"]
                state["rsb"] = rsb = rs_p.tile([1, NT2], bf16, tag="rsb",
                                               name="rsb")
                with nc.allow_low_precision(reason="softmax recip bf16"):
                    nc.vector.reciprocal(rsb, pv_ps[64:65, :])

            def attn_stepC(b, p, state):
                # K=1 broadcast matmuls -> psum; Act stages the scales to
                # SBUF (TensorTensor may read only one PSUM operand)
                pv_ps = state["pv"]
                rsb = state["rsb"]
                sc_ps = ps_dense.tile([128, NT2], f32, tag="psD", name="sc_ps")
                for hi in range(2):
                    hb = hi * 64
                    nc.tensor.matmul(
                        sc_ps[hb:hb + 64, 0:N_TOK],
                        ones_sb[:, 0:64],
                        rsb[:, hi * N_TOK:(hi + 1) * N_TOK],
                        start=True, stop=True,
                    )
                state["sc"] = sc_sb = rs_p.tile([128, N_TOK], bf16, tag="scb",
                                                name="sc_sb")
                with nc.allow_low_precision(reason="softmax scale bf16"):
                    nc.scalar.copy(sc_sb, sc_ps[:, 0:N_TOK])

            def attn_stepM(b, p, state):
                # DVE normalizes into o_sb one iteration later, keeping the
                # muls out of the reciprocals' way in the DVE queue
                c0 = b * N_TOK
                pv_ps = state["pv"]
                sc_sb = state["sc"]
                for hi in range(2):
                    hb = hi * 64
                    nc.vector.tensor_mul(
                        o_sb[p][hb:hb + 64, c0:c0 + N_TOK],
                        pv_ps[0:64, hi * N_TOK:(hi + 1) * N_TOK],
                        sc_sb[hb:hb + 64, :])

            # ---- dense queue + schedule ----
            dense_q = []

            def push_slot(b_a, b_next, b_proj, cap_slot=None,
                          tail_proj=False):
                units = []
                if b_a is not None and b_a < B_PER_CORE:
                    units.append([(lambda b=b_a, mt=mt: a_unit(b, mt))
                                  for mt in range(6)])
                if b_next is not None and b_next < B_PER_CORE:
                    qk = [(lambda b=b_next, mt=mt: qk_unit(b, mt))
                          for mt in range(6)]
                    vv = [(lambda b=b_next, ti=ti, nt=nt: v_unit(b, ti, nt))
                          for ti in range(2) for nt in range(2)]
                    # interleave qk and v to spread psum bank reuse
                    mix = []
                    while qk or vv:
                        if qk:
                            mix.append(qk.pop(0))
                        if vv:
                            mix.append(vv.pop(0))
                        if qk:
                            mix.append(qk.pop(0))
                    units.append(mix)
                pu = []
                for ft, nt in proj_ready(b_proj, cap_slot):
                    tl = False
                    pu.append(lambda ft=ft, nt=nt, tl=tl: proj_unit(ft, nt, tl))
                if pu:
                    units.append(pu)
                proj_units = units.pop() if b_proj is not None else []
                flat = []
                srcs = [u for u in units if u]
                while srcs:
                    for u in srcs:
                        if u:
                            flat.append(u.pop(0))
                    srcs = [u for u in srcs if u]
                # proj interleaved into the back 2/3 of the slot queue
                k = len(flat) // 3
                back = flat[k:]
                merged = []
                while back or proj_units:
                    if back:
                        merged.append(back.pop(0))
                    if proj_units:
                        merged.append(proj_units.pop(0))
                    if back:
                        merged.append(back.pop(0))
                dense_q.extend(flat[:k] + merged)
            push_slot.proj_done = 0

            # proj-tile readiness: tile ft needs all batches covering
            # [128*ft, 128*(ft+1)); batches run in BATCH_ORDER (7 before 6
            # so the last slot still has dense fill and a short tail)
            proj_state = {"done": set(), "emitted": set()}

            def proj_ready(b_done, cap):
                if b_done is not None:
                    proj_state["done"].add(b_done)
                out = []
                for ft in range(len(PROJ_TILES)):
                    if ft in proj_state["emitted"]:
                        continue
                    f0, fsz = PROJ_TILES[ft]
                    b_lo = f0 // N_TOK
                    b_hi = (f0 + fsz - 1) // N_TOK
                    if all(bb in proj_state["done"]
                           for bb in range(b_lo, b_hi + 1)):
                        out.append(ft)
                out = out[:cap] if cap is not None else out
                res = []
                for ft in out:
                    proj_state["emitted"].add(ft)
                    res.extend([(ft, 0), (ft, 1)])
                return res

            def drain_dense(n):
                for _ in range(min(n, len(dense_q))):
                    dense_q.pop(0)()

            # prologue: A(0), A(1), qk(0) first; batch-0's S/exp chains
            # start while the v-path DMAs are still landing
            for mt in range(6):
                a_unit(0, mt)
            for mt in range(6):
                a_unit(1, mt)
            for mt in range(6):
                qk_unit(0, mt)
            for mt in range(6):
                qk_unit(1, mt)
            states0 = [dict() for _ in range(6)]
            push_slot(2, None, None)
            for ti in range(2):
                for nt in range(2):
                    dense_q.append(lambda ti=ti, nt=nt: v_unit(0, ti, nt))
                    dense_q.append(lambda ti=ti, nt=nt: v_unit(1, ti, nt))
            fill0 = (len(dense_q) + 5) // 6
            for p in range(6):
                attn_stepA(0, p, states0[p])
                drain_dense(fill0)
            drain_dense(len(dense_q))

            BATCH_ORDER = [0, 1, 2, 3, 4, 5, 6, 7]
            for bi in range(B_PER_CORE):
                b = BATCH_ORDER[bi]
                b_a = BATCH_ORDER[bi + 3] if bi + 3 < B_PER_CORE else None
                b_next = BATCH_ORDER[bi + 2] if bi + 2 < B_PER_CORE else None
                b_prev = BATCH_ORDER[bi - 1] if bi > 0 else None
                cap = 2 if bi <= 2 else None
                push_slot(b_a, b_next, b_prev, cap)
                states = states0 if b == 0 else [dict() for _ in range(6)]
                n_iters = 14
                fill = (len(dense_q) + 2 * n_iters - 1) // (2 * n_iters)
                for p in range(n_iters):
                    if p < 6 and b > 0:
                        attn_stepA(b, p, states[p])
                    if 1 <= p < 7:
                        attn_stepB(b, p - 1, states[p - 1])
                        attn_stepCr(b, p - 1, states[p - 1])
                    if 2 <= p < 8:
                        attn_stepC(b, p - 2, states[p - 2])
                    if 3 <= p < 9:
                        attn_stepM(b, p - 3, states[p - 3])
                    drain_dense(2 * fill)
                drain_dense(len(dense_q))
            push_slot(None, None, BATCH_ORDER[-1], tail_proj=True)
            drain_dense(len(dense_q))

    return nc


_CACHED_NC = None


def kernel(x, w_qkv, w_proj, b_proj, factors):
    global LAST_EXEC_NS, LAST_TRACE, _CACHED_NC
    from concourse.bass_utils import run_bass_kernel_spmd

    factors = np.asarray(factors, dtype=np.float32)
    G = _grid_g(factors)  # raw G

    w_qkv = np.asarray(w_qkv, dtype=np.float32)

    def pack8(w):
        # [768 out, 768 in] -> w^T scaled -> [p, (kt2 i out)] fp8
        wt = np.ascontiguousarray(w.T) * W_SCALE  # [in, out]
        return wt.reshape(3, 2, 128, DIM).transpose(2, 0, 1, 3).reshape(128, -1)

    def pack16(w):
        wt = np.ascontiguousarray(np.asarray(w, dtype=np.float32).T)
        return wt.reshape(6, 128, DIM).transpose(1, 0, 2).reshape(128, -1)

    gtp = np.zeros((256, N_TOK), dtype=np.float32)
    gtp[0:N_TOK] = G.T
    in_common = {
        "gt": np.ascontiguousarray(
            gtp.reshape(2, 128, N_TOK).transpose(1, 0, 2).reshape(128, -1)
        ).astype(BF16),
        "wq8": np.ascontiguousarray(pack8(w_qkv[0:DIM])).astype(FP8),
        "wk8": np.ascontiguousarray(pack8(w_qkv[DIM:2 * DIM])).astype(FP8),
        "wv": np.ascontiguousarray(pack16(w_qkv[2 * DIM:3 * DIM])).astype(BF16),
        "wp": np.ascontiguousarray(pack16(w_proj)).astype(BF16),
        "bias": np.asarray(b_proj, dtype=np.float32),
    }
    x = np.asarray(x, dtype=np.float32).reshape(BATCH * N_TOK, DIM)
    in_maps = []
    for c in range(N_CORES):
        xc = x[c * NTB:(c + 1) * NTB]  # [1568, 768] f32
        xcp = np.zeros((NTB + 96, DIM), dtype=np.float32)
        xcp[0:NTB] = xc
        xtc = np.ascontiguousarray(xc.T)  # [768, 1568]
        # per-batch grouped: [p, (b kt2 i c)] and [p, (b kt c)]
        xt8p = xtc.reshape(3, 2, 128, B_PER_CORE, N_TOK) \
            .transpose(2, 3, 0, 1, 4).reshape(128, -1)
        xtp = xtc.reshape(6, 128, B_PER_CORE, N_TOK) \
            .transpose(1, 2, 0, 3).reshape(128, -1)
        in_maps.append({
            "x": xcp.astype(BF16),
            "xt": np.ascontiguousarray(xtp).astype(BF16),
            "xt8": np.ascontiguousarray(xt8p).astype(FP8),
            **in_common,
        })

    if _CACHED_NC is None:
        _CACHED_NC = _build_bass()
    nc = _CACHED_NC

    trace = bool(int(os.environ.get("KERNEL_TRACE", "0")))
    res = run_bass_kernel_spmd(nc, in_maps, core_ids=list(range(N_CORES)),
                               trace=trace)
    LAST_EXEC_NS = res.exec_time_ns
    if res.instructions_and_trace is not None:
        LAST_TRACE = res.instructions_and_trace[1]
    out = np.concatenate([res.results[c]["out"] for c in range(N_CORES)], axis=0)
    return out.reshape(BATCH, N_TOK, DIM).astype(np.float32)


# revision 4
# speedup vs baseline: 1.0126x; 1.0046x over previous
"""Trainium2 Bass kernel for nn_Attention_33157147525297 (v2, pipelined).

Graph-mixed multi-head attention, B=64, N=196 tokens, D=768, H=12 heads.
Data-parallel over batch: 8 batches per NeuronCore x 8 cores.
Measured (TimelineSim cost model): 143151 ns vs 205577 ns baseline (1.44x);
hardware rel err 5.1e-3 fro (budget 2e-2).

Math restructuring (host side):
  reference: attn = softmax(G @ (q k^T * scale)); out = attn @ v
  G mixes the query index only, so the graph-mix collapses into a pre-mix of
  x on the query path: xg = G @ x (raw G; every scalar scale - attention
  1/sqrt(d) and the fp8 weight pre-scales - folds into the exp activation's
  input scale argument, which is free).

Key design points:
  - q/k projections run in fp8 (e4m3) with MatmulPerfMode.DoubleRow: 256-deep
    contraction per matmul at 0.5 cycles/row (2x PE throughput).  Weights are
    scaled x16 into fp8's normal range.  v/proj/S/PV stay bf16: measured on
    the graded inputs, fp8 there busts the 2e-2 budget (v 3.8e-2, proj
    3.3e-2, S 2.2e-2) while fp8-qk lands at 5.1e-3 total because softmax
    normalization damps score-level error.
  - x ships token-major (premix stationary) plus x^T in both bf16 (v path)
    and DoubleRow-packed fp8 (k path): pure host-side layout/dtype prep that
    kills 48 on-device transpose copies and halves stage A.
  - softmax sums come free from the PV matmul via a ones-column appended to
    each head's v slice (pair psum [65, 392]: rows 0:64 = O^T, row 64 =
    sums); DVE reciprocal reads the psum sums row directly; the broadcast
    runs as two K=1 PE matmuls, Act stages the scales psum->SBUF
    (TensorTensor may read only one PSUM operand - walrus rule), DVE
    normalizes into o_sb.
  - S stationary slices are always 128 wide from a 64-col-padded k half
    (qkT halves of 1632), so the S psum is fully written and one exp per
    head covers both token tiles; the junk rows are never read by PV.
  - projection packs tokens across batch boundaries into 13 flat 128-row
    tiles; bias folds into the y staging copy as a DVE tensor_add against a
    broadcast bias tile.
  - engine assignment by latency-criticality (GPSIMD/Pool cannot touch PSUM
    on this machine, so it only gets memsets): Act = exp + v/xg8/scale
    copies, DVE = qk copies + reciprocals + normalize muls + y adds.
  - the whole kernel is software-pipelined per batch: slot b runs attention
    for batch b in 10 interleave iterations (S/exp -> PV -> recip -> bcast ->
    muls at pipeline depths 0/1/1/2/3) with stage A of b+3, qk/v of b+2 and
    projection tiles of b-1 draining in the gaps; input DMAs are host-packed
    to final SBUF layout, per-batch sliced, and issued in strict
    pipeline-need order (the cost model serializes transfers on a shared
    engine pool, so arrival order is everything).

Infra notes: this container's walrus accepts only ONE attached semaphore
wait per instruction - _install_wait_split() hoists excess waits onto
standalone EventSemaphore instructions just before, on the same engine.
Timing feedback comes from the concourse cost-model TimelineSim (NTFF
profiling hooks are unavailable under this axon client).
"""
import os
import sys
import numpy as np
import ml_dtypes

sys.path.insert(0, "/opt/trn_rl_repo")

SIZE, N_TOK, DIM, HEADS, HEAD_DIM, BATCH = 14, 196, 768, 12, 64, 64
N_CORES = 8
B_PER_CORE = BATCH // N_CORES  # 8
NT2 = 2 * N_TOK  # 392
NTB = N_TOK * B_PER_CORE  # 1568
BF16 = ml_dtypes.bfloat16
FP8 = ml_dtypes.float8_e4m3
W_SCALE = 16.0  # q/k weight pre-scale into fp8 normal range
EXP_SCALE = 1.0 / (W_SCALE * W_SCALE * (HEAD_DIM ** 0.5))

# token-dim partition tiles (196 = 128 + 68)
TOK_TILES = [(0, 128), (128, 68)]
# flat projection tiles over 1568 tokens: 12x128 + 32
PROJ_TILES = [(ft * 128, min(128, NTB - ft * 128)) for ft in range(13)]

LAST_EXEC_NS = None
LAST_TRACE = None


def _grid_g(factors):
    idx = np.arange(SIZE * SIZE).reshape(SIZE, SIZE)
    A = np.zeros((N_TOK, N_TOK), dtype=np.float32)
    for di, dj in [(-1, 0), (1, 0), (0, -1), (0, 1)]:
        for i in range(SIZE):
            for j in range(SIZE):
                ii, jj = i + di, j + dj
                if 0 <= ii < SIZE and 0 <= jj < SIZE:
                    A[idx[i, j], idx[ii, jj]] = 1.0
    NN = A / (A.sum(axis=1, keepdims=True) + 1.0)
    C = np.eye(N_TOK, dtype=np.float32) / 2.0
    return factors[0] * C + factors[1] * NN  # raw G, no attention scale


def _install_wait_split():
    """This container's walrus rejects >1 attached semaphore wait per
    instruction ("Too many sync wait commands").  Hoist excess waits onto
    standalone InstEventSemaphore instructions just before, on the same
    engine - engine queues are in-order, so semantics are identical."""
    import concourse.mybir as mybir
    import concourse.tile as tile
    from concourse.vector_clock import ScopedClock

    TC = tile.TileContext
    if getattr(TC, "_wait_split_patched", False):
        return
    LIMIT = 1

    def _split(tc, inst):
        si = inst.sync_info
        if (si is None or not si.on_wait or len(si.on_wait) <= LIMIT
                or inst.engine == mybir.EngineType.Unassigned):
            return
        waits = list(si.on_wait)
        extra, keep = waits[:-LIMIT], waits[-LIMIT:]
        for i, w in enumerate(extra):
            ev = mybir.InstEventSemaphore(
                name=f"{inst.name}-ws{i}", engine=inst.engine,
                sync_info=mybir.SyncInfo(on_wait=[w], on_update=[]),
            )
            tc._add_instruction(ev)
        inst.sync_info = mybir.SyncInfo(on_wait=keep,
                                        on_update=list(si.on_update))

    orig_commit = TC._commit_instruction

    def patched_commit(self, inst, lazy_reg_writes=True):
        _split(self, inst)
        return orig_commit(self, inst, lazy_reg_writes=lazy_reg_writes)

    TC._commit_instruction = patched_commit

    def patched_drain_and_barrier(self, tick_clock, wait_clock):
        nc = self.nc
        probe = mybir.InstNoOp(
            name=f"drain-probe-{nc.next_id()}", engine=mybir.EngineType.SP)
        wait_clock.add_sem_waits(
            probe, ScopedClock({None: tick_clock.global_clock}))
        pw = probe.sync_info.on_wait if probe.sync_info else []
        for i, w in enumerate(pw):
            ev = mybir.InstEventSemaphore(
                name=f"drainw-{nc.next_id()}-{i}", engine=mybir.EngineType.SP,
                sync_info=mybir.SyncInfo(on_wait=[w], on_update=[]),
            )
            self._add_instruction(ev)
        nc.sync.drain()
        nc.all_engine_barrier()
        assert self.sems is not None
        popped = nc._tile_sem_poison_stack.pop()
        assert popped is self._sem_poison
        nc.clear_and_free_semaphores(list(self.sems.allocated().values()))
        nc.all_engine_barrier()

    TC._drain_and_barrier = patched_drain_and_barrier
    TC._wait_split_patched = True


def _build_bass():
    import concourse.bass as bass
    import concourse.mybir as mybir
    import concourse.tile as tile

    _install_wait_split()

    f32 = mybir.dt.float32
    bf16 = mybir.dt.bfloat16
    fp8 = mybir.dt.float8e4
    AF = mybir.ActivationFunctionType
    DR = mybir.MatmulPerfMode.DoubleRow

    nc = bass.Bass()

    # x padded to 1664 rows so each batch loads as one [256 -> (2,128)] DMA
    x_d = nc.declare_dram_parameter("x", [NTB + 96, DIM], bf16, isOutput=False)
    gt_d = nc.declare_dram_parameter("gt", [128, 2 * N_TOK], bf16, isOutput=False)
    # everything below is host-packed into final SBUF layout [128, cols];
    # xt8/xt are grouped per batch so arrival order matches pipeline need
    xt8_d = nc.declare_dram_parameter("xt8", [128, 3 * 2 * NTB], fp8,
                                      isOutput=False)
    xt_d = nc.declare_dram_parameter("xt", [128, 6 * NTB], bf16, isOutput=False)
    wq8_d = nc.declare_dram_parameter("wq8", [128, 3 * 2 * DIM], fp8,
                                      isOutput=False)
    wk8_d = nc.declare_dram_parameter("wk8", [128, 3 * 2 * DIM], fp8,
                                      isOutput=False)
    wv_d = nc.declare_dram_parameter("wv", [128, 6 * DIM], bf16, isOutput=False)
    wp_d = nc.declare_dram_parameter("wp", [128, 6 * DIM], bf16, isOutput=False)
    bias_d = nc.declare_dram_parameter("bias", [DIM], f32, isOutput=False)
    out_d = nc.declare_dram_parameter("out", [NTB, DIM], f32, isOutput=True)

    with tile.TileContext(nc) as tc:
        with (
            tc.tile_pool(name="const", bufs=1) as const_p,
            tc.tile_pool(name="big", bufs=1) as big_p,
            tc.tile_pool(name="pt", bufs=8) as pt_p,
            tc.tile_pool(name="rs", bufs=6) as rs_p,
            tc.tile_pool(name="ps_dense", bufs=3, space="PSUM") as ps_dense,
            tc.tile_pool(name="ps_s", bufs=2, space="PSUM") as ps_s,
            tc.tile_pool(name="ps_pv", bufs=3, space="PSUM") as ps_pv,
        ):
            # ---- input DMAs: one SP queue, strict need-order (the cost
            #      model serializes transfers on a shared engine pool) ----
            gt2_sb = const_p.tile([128, 2 * N_TOK], bf16, name="gt2")
            x_sb = [big_p.tile([128, 2 * DIM], bf16, name=f"x{b}")
                    for b in range(B_PER_CORE)]
            xt8_sb = const_p.tile([128, 3 * 2 * NTB], fp8, name="xt8")
            xt_sb = const_p.tile([128, 6 * NTB], bf16, name="xt")
            wq8_sb = const_p.tile([128, 3 * 2 * DIM], fp8, name="wq8")
            wk8_sb = const_p.tile([128, 3 * 2 * DIM], fp8, name="wk8")
            wv_sb = const_p.tile([128, 6 * DIM], bf16, name="wv")
            wp_sb = const_p.tile([128, 6 * DIM], bf16, name="wp")
            bias_sb = const_p.tile([128, DIM], f32, name="bias")

            BPB8 = 3 * 2 * N_TOK   # xt8 cols per batch
            BPB = 6 * N_TOK        # xt cols per batch

            def dma_x(b):
                nc.sync.dma_start(
                    out=x_sb[b].rearrange("p (t c) -> p t c", t=2),
                    in_=x_d[b * N_TOK:b * N_TOK + 256, :].rearrange(
                        "(t p) c -> p t c", p=128))

            def dma_xt8(b):
                nc.sync.dma_start(
                    out=xt8_sb[:, b * BPB8:(b + 1) * BPB8],
                    in_=xt8_d[:, b * BPB8:(b + 1) * BPB8])

            def dma_xt(b):
                nc.sync.dma_start(
                    out=xt_sb[:, b * BPB:(b + 1) * BPB],
                    in_=xt_d[:, b * BPB:(b + 1) * BPB])

            dma_x(0)
            nc.sync.dma_start(out=gt2_sb, in_=gt_d[:, :])
            dma_x(1)
            dma_xt8(0)
            nc.sync.dma_start(out=wq8_sb, in_=wq8_d[:, :])
            nc.sync.dma_start(out=wk8_sb, in_=wk8_d[:, :])
            dma_xt(0)
            nc.sync.dma_start(out=wv_sb, in_=wv_d[:, :])
            dma_xt8(1)
            dma_xt(1)
            dma_x(2)
            dma_xt8(2)
            dma_xt(2)
            dma_x(3)
            dma_xt8(3)
            dma_xt(3)
            nc.sync.dma_start(out=wp_sb, in_=wp_d[:, :])
            nc.sync.dma_start(out=bias_sb,
                              in_=bias_d[None, :].broadcast_to([128, DIM]))
            for b in range(4, B_PER_CORE):
                dma_x(b)
                dma_xt8(b)
                dma_xt(b)
            gt_sb = [gt2_sb[:, 0:N_TOK], gt2_sb[:, N_TOK:2 * N_TOK]]
            x_sb = [[x_sb[b][:, 0:DIM], x_sb[b][:, DIM:2 * DIM]]
                    for b in range(B_PER_CORE)]

            ones_sb = const_p.tile([1, 128], bf16, name="ones")
            nc.vector.memset(ones_sb, 1.0)
            biasrow_sb = const_p.tile([1, DIM], bf16, name="biasrow")
            nc.scalar.copy(biasrow_sb, bias_sb[0:1, :])

            # ---- persistent activations ----
            # xg^T fp8 DoubleRow layout [p, i, tok]
            xg8_sb = [big_p.tile([128, 2 * NTB], fp8, name=f"xg8{k}")
                      for k in range(3)]
            xg8_v = [t.rearrange("p (i c) -> p i c", i=2) for t in xg8_sb]
            wq8_v = wq8_sb.rearrange("p (k i c) -> p k i c", k=3, i=2)
            wk8_v = wk8_sb.rearrange("p (k i c) -> p k i c", k=3, i=2)
            # q^T|k^T combined per feature tile: halves of NTBP=1632
            # (1568 tokens + 64 pad so S's stationary reads are always 128
            # wide; pad is zeroed once below)
            NTBP = NTB + 64
            qkT_sb = [big_p.tile([128, 2 * NTBP], bf16, name=f"qkT{k}")
                      for k in range(6)]
            for k in range(6):
                nc.gpsimd.memset(qkT_sb[k][:, 2 * NTBP - 64:2 * NTBP], 0.0)
            # v per batch-tile: 12 head groups of 65 cols (col 64 = ones)
            v1_sb = [
                [big_p.tile([128, 12 * 65], bf16, name=f"v{b}_{ti}")
                 for ti in range(2)]
                for b in range(B_PER_CORE)
            ]
            for b in range(B_PER_CORE):
                for ti in range(2):
                    nc.gpsimd.memset(
                        v1_sb[b][ti].rearrange("p (h c) -> p h c", h=12)[:, :, 64:65],
                        1.0)
            o_sb = [big_p.tile([128, NTB], bf16, name=f"o{k}")
                    for k in range(6)]

            # ---- dense work units (one psum group each) ----
            def a_unit(b, mt):
                # xg^T premix for one feature tile: [128, 196] -> fp8
                c0 = b * N_TOK
                ps = ps_dense.tile([128, NT2], f32, tag="psD", name="psD")
                for ti, (t0, tsz) in enumerate(TOK_TILES):
                    nc.tensor.matmul(
                        ps[:, 0:N_TOK],
                        x_sb[b][ti][:tsz, mt * 128:(mt + 1) * 128],
                        gt_sb[ti][:tsz], start=(ti == 0), stop=(ti == 1),
                    )
                with nc.allow_low_precision(reason="fp8 qk path"):
                    nc.scalar.copy(
                        xg8_v[mt // 2][:, mt % 2, c0:c0 + N_TOK], ps[:, 0:N_TOK])

            def qk_unit(b, mt):
                # q^T and k^T for one feature tile: two groups in one bank
                c0 = b * N_TOK
                ps = ps_dense.tile([128, NT2], f32, tag="psD", name="psD")
                for kt2 in range(3):
                    nc.tensor.matmul(
                        ps[:, 0:N_TOK],
                        wq8_v[:, kt2, :, mt * 128:(mt + 1) * 128],
                        xg8_v[kt2][:, :, c0:c0 + N_TOK],
                        start=(kt2 == 0), stop=(kt2 == 2), perf_mode=DR,
                    )
                for kt2 in range(3):
                    nc.tensor.matmul(
                        ps[:, N_TOK:NT2],
                        wk8_v[:, kt2, :, mt * 128:(mt + 1) * 128],
                        xt8_sb.rearrange("p (b k i c) -> p b k i c",
                                         b=B_PER_CORE, k=3, i=2)[:, b, kt2],
                        start=(kt2 == 0), stop=(kt2 == 2), perf_mode=DR,
                    )
                dst = qkT_sb[mt].rearrange("p (g c) -> p g c", g=2)[
                    :, :, c0:c0 + N_TOK]
                nc.vector.tensor_copy(dst, ps.rearrange("p (g c) -> p g c", g=2))

            def v_unit(b, ti, nt):
                # v [tsz, 384] = 6 heads x 64, strided into v1 (65-col groups)
                t0, tsz = TOK_TILES[ti]
                c0 = b * N_TOK
                ps = ps_dense.tile([128, NT2], f32, tag="psD", name="psD")
                for kt in range(6):
                    nc.tensor.matmul(
                        ps[:tsz, :384],
                        xt_sb[:, b * BPB + kt * N_TOK + t0:
                              b * BPB + kt * N_TOK + t0 + tsz],
                        wv_sb[:, kt * DIM + nt * 384:kt * DIM + (nt + 1) * 384],
                        start=(kt == 0), stop=(kt == 5),
                    )
                dstv = v1_sb[b][ti].rearrange("p (h c) -> p h c", h=12)
                nc.scalar.copy(
                    dstv[:tsz, 6 * nt:6 * nt + 6, 0:64],
                    ps[:tsz, :384].rearrange("p (h c) -> p h c", h=6))

            def proj_unit(ft, nt, tail=False):
                f0, fsz = PROJ_TILES[ft]
                ps = ps_dense.tile([128, NT2], f32, tag="psD", name="psD")
                for kt in range(6):
                    nc.tensor.matmul(
                        ps[:fsz, :384],
                        o_sb[kt][:, f0:f0 + fsz],
                        wp_sb[:, kt * DIM + nt * 384:kt * DIM + (nt + 1) * 384],
                        start=(kt == 0), stop=(kt == 5 and not tail),
                    )
                y_sb = rs_p.tile([128, 384], f32, tag="y", name="y_sb")
                if tail:
                    # tail variant: bias rides a K=1 matmul (PE is idle by
                    # now; f32 moving data is fine) and Act does the copy,
                    # halving the end-of-kernel DVE serialization
                    nc.tensor.matmul(
                        ps[:fsz, :384], ones_sb[:, :fsz],
                        biasrow_sb[:, nt * 384:(nt + 1) * 384],
                        start=False, stop=True)
                    nc.scalar.copy(y_sb[:fsz], ps[:fsz, :384])
                else:
                    nc.vector.tensor_add(
                        y_sb[:fsz], ps[:fsz, :384],
                        bias_sb[:fsz, nt * 384:(nt + 1) * 384])
                nc.sync.dma_start(
                    out=out_d[f0:f0 + fsz, nt * 384:(nt + 1) * 384],
                    in_=y_sb[:fsz])

            # ---- attention chain steps (per batch b, head pair p) ----
            def attn_stepA(b, p, state):
                c0 = b * N_TOK
                state["pT"] = pT = pt_p.tile([128, 2 * NT2], bf16,
                                             tag="pT", name="pT")
                for hi in range(2):
                    hb = hi * 64
                    s_ps = ps_s.tile([128, NT2], f32, tag="s", name="s_ps")
                    for ti in range(2):
                        t0 = ti * 128
                        nc.tensor.matmul(
                            s_ps[:, ti * N_TOK:(ti + 1) * N_TOK],
                            qkT_sb[p][hb:hb + 64,
                                      NTBP + c0 + t0:NTBP + c0 + t0 + 128],
                            qkT_sb[p][hb:hb + 64, c0:c0 + N_TOK],
                            start=True, stop=True,
                        )
                    nc.scalar.activation(pT[:, hi * NT2:(hi + 1) * NT2], s_ps,
                                         AF.Exp, scale=EXP_SCALE)

            def attn_stepB(b, p, state):
                # PV (+sums via ones col); pair bank [65, 392]
                pT = state["pT"]
                state["pv"] = pv_ps = ps_pv.tile([65, NT2], f32, tag="pv",
                                                 name="pv_ps")
                for hi in range(2):
                    for ti, (t0, tsz) in enumerate(TOK_TILES):
                        nc.tensor.matmul(
                            pv_ps[:, hi * N_TOK:(hi + 1) * N_TOK],
                            v1_sb[b][ti][:tsz,
                                         (2 * p + hi) * 65:(2 * p + hi) * 65 + 65],
                            pT[:tsz, hi * NT2 + ti * N_TOK:hi * NT2 + (ti + 1) * N_TOK],
                            start=(ti == 0), stop=(ti == 1),
                        )

            def attn_stepCr(b, p, state):
                # recip from the psum sums row (emitted right after its PV so
                # it sits ahead of bulk work in the DVE queue)
                pv_ps = state["pv"]
                state["rsb"] = rsb = rs_p.tile([1, NT2], bf16, tag="rsb",
                                               name="rsb")
                with nc.allow_low_precision(reason="softmax recip bf16"):
                    nc.vector.reciprocal(rsb, pv_ps[64:65, :])

            def attn_stepC(b, p, state):
                # K=1 broadcast matmuls -> psum; Act stages the scales to
                # SBUF (TensorTensor may read only one PSUM operand)
                pv_ps = state["pv"]
                rsb = state["rsb"]
                sc_ps = ps_dense.tile([128, NT2], f32, tag="psD", name="sc_ps")
                for hi in range(2):
                    hb = hi * 64
                    nc.tensor.matmul(
                        sc_ps[hb:hb + 64, 0:N_TOK],
                        ones_sb[:, 0:64],
                        rsb[:, hi * N_TOK:(hi + 1) * N_TOK],
                        start=True, stop=True,
                    )
                state["sc"] = sc_sb = rs_p.tile([128, N_TOK], bf16, tag="scb",
                                                name="sc_sb")
                with nc.allow_low_precision(reason="softmax scale bf16"):
                    nc.scalar.copy(sc_sb, sc_ps[:, 0:N_TOK])

            def attn_stepM(b, p, state):
                # DVE normalizes into o_sb one iteration later, keeping the
                # muls out of the reciprocals' way in the DVE queue
                c0 = b * N_TOK
                pv_ps = state["pv"]
                sc_sb = state["sc"]
                for hi in range(2):
                    hb = hi * 64
                    nc.vector.tensor_mul(
                        o_sb[p][hb:hb + 64, c0:c0 + N_TOK],
                        pv_ps[0:64, hi * N_TOK:(hi + 1) * N_TOK],
                        sc_sb[hb:hb + 64, :])

            # ---- dense queue + schedule ----
            dense_q = []

            def push_slot(b_a, b_next, b_proj, cap_slot=None,
                          tail_proj=False):
                units = []
                if b_a is not None and b_a < B_PER_CORE:
                    units.append([(lambda b=b_a, mt=mt: a_unit(b, mt))
                                  for mt in range(6)])
                if b_next is not None and b_next < B_PER_CORE:
                    qk = [(lambda b=b_next, mt=mt: qk_unit(b, mt))
                          for mt in range(6)]
                    vv = [(lambda b=b_next, ti=ti, nt=nt: v_unit(b, ti, nt))
                          for ti in range(2) for nt in range(2)]
                    # interleave qk and v to spread psum bank reuse
                    mix = []
                    while qk or vv:
                        if qk:
                            mix.append(qk.pop(0))
                        if vv:
                            mix.append(vv.pop(0))
                        if qk:
                            mix.append(qk.pop(0))
                    units.append(mix)
                pu = []
                for ft, nt in proj_ready(b_proj, cap_slot):
                    tl = False
                    pu.append(lambda ft=ft, nt=nt, tl=tl: proj_unit(ft, nt, tl))
                if pu:
                    units.append(pu)
                proj_units = units.pop() if b_proj is not None else []
                flat = []
                srcs = [u for u in units if u]
                while srcs:
                    for u in srcs:
                        if u:
                            flat.append(u.pop(0))
                    srcs = [u for u in srcs if u]
                # proj interleaved into the back 2/3 of the slot queue
                k = len(flat) // 3
                back = flat[k:]
                merged = []
                while back or proj_units:
                    if back:
                        merged.append(back.pop(0))
                    if proj_units:
                        merged.append(proj_units.pop(0))
                    if back:
                        merged.append(back.pop(0))
                dense_q.extend(flat[:k] + merged)
            push_slot.proj_done = 0

            # proj-tile readiness: tile ft needs all batches covering
            # [128*ft, 128*(ft+1)); batches run in BATCH_ORDER (7 before 6
            # so the last slot still has dense fill and a short tail)
            proj_state = {"done": set(), "emitted": set()}

            def proj_ready(b_done, cap):
                if b_done is not None:
                    proj_state["done"].add(b_done)
                out = []
                for ft in range(len(PROJ_TILES)):
                    if ft in proj_state["emitted"]:
                        continue
                    f0, fsz = PROJ_TILES[ft]
                    b_lo = f0 // N_TOK
                    b_hi = (f0 + fsz - 1) // N_TOK
                    if all(bb in proj_state["done"]
                           for bb in range(b_lo, b_hi + 1)):
                        out.append(ft)
                out = out[:cap] if cap is not None else out
                res = []
                for ft in out:
                    proj_state["emitted"].add(ft)
                    res.extend([(ft, 0), (ft, 1)])
                return res

            def drain_dense(n):
                for _ in range(min(n, len(dense_q))):
                    dense_q.pop(0)()

            # prologue: A(0), A(1), qk(0) first; batch-0's S/exp chains
            # start while the v-path DMAs are still landing
            for mt in range(6):
                a_unit(0, mt)
            for mt in range(6):
                a_unit(1, mt)
            for mt in range(6):
                qk_unit(0, mt)
            for mt in range(6):
                qk_unit(1, mt)
            states0 = [dict() for _ in range(6)]
            push_slot(2, None, None)
            for ti in range(2):
                for nt in range(2):
                    dense_q.append(lambda ti=ti, nt=nt: v_unit(0, ti, nt))
                    dense_q.append(lambda ti=ti, nt=nt: v_unit(1, ti, nt))
            fill0 = (len(dense_q) + 5) // 6
            for p in range(6):
                attn_stepA(0, p, states0[p])
                drain_dense(fill0)
            drain_dense(len(dense_q))

            BATCH_ORDER = [0, 1, 2, 3, 4, 5, 6, 7]
            for bi in range(B_PER_CORE):
                b = BATCH_ORDER[bi]
                b_a = BATCH_ORDER[bi + 3] if bi + 3 < B_PER_CORE else None
                b_next = BATCH_ORDER[bi + 2] if bi + 2 < B_PER_CORE else None
                b_prev = BATCH_ORDER[bi - 1] if bi > 0 else None
                cap = 2 if bi <= 2 else None
                push_slot(b_a, b_next, b_prev, cap)
                states = states0 if b == 0 else [dict() for _ in range(6)]
                n_iters = 14
                fill = (len(dense_q) + 2 * n_iters - 1) // (2 * n_iters)
                for p in range(n_iters):
                    if p < 6 and b > 0:
                        attn_stepA(b, p, states[p])
                    if 1 <= p < 7:
                        attn_stepB(b, p - 1, states[p - 1])
                        attn_stepCr(b, p - 1, states[p - 1])
                    if 2 <= p < 8:
                        attn_stepC(b, p - 2, states[p - 2])
                    if 3 <= p < 9:
                        attn_stepM(b, p - 3, states[p - 3])
                    drain_dense(2 * fill)
                drain_dense(len(dense_q))
            push_slot(None, None, BATCH_ORDER[-1], tail_proj=True)
            drain_dense(len(dense_q))

    return nc


_CACHED_NC = None


def kernel(x, w_qkv, w_proj, b_proj, factors):
    global LAST_EXEC_NS, LAST_TRACE, _CACHED_NC
    from concourse.bass_utils import run_bass_kernel_spmd

    factors = np.asarray(factors, dtype=np.float32)
    G = _grid_g(factors)  # raw G

    w_qkv = np.asarray(w_qkv, dtype=np.float32)

    def pack8(w):
        # [768 out, 768 in] -> w^T scaled -> [p, (kt2 i out)] fp8
        wt = np.ascontiguousarray(w.T) * W_SCALE  # [in, out]
        return wt.reshape(3, 2, 128, DIM).transpose(2, 0, 1, 3).reshape(128, -1)

    def pack16(w):
        wt = np.ascontiguousarray(np.asarray(w, dtype=np.float32).T)
        return wt.reshape(6, 128, DIM).transpose(1, 0, 2).reshape(128, -1)

    gtp = np.zeros((256, N_TOK), dtype=np.float32)
    gtp[0:N_TOK] = G.T
    in_common = {
        "gt": np.ascontiguousarray(
            gtp.reshape(2, 128, N_TOK).transpose(1, 0, 2).reshape(128, -1)
        ).astype(BF16),
        "wq8": np.ascontiguousarray(pack8(w_qkv[0:DIM])).astype(FP8),
        "wk8": np.ascontiguousarray(pack8(w_qkv[DIM:2 * DIM])).astype(FP8),
        "wv": np.ascontiguousarray(pack16(w_qkv[2 * DIM:3 * DIM])).astype(BF16),
        "wp": np.ascontiguousarray(pack16(w_proj)).astype(BF16),
        "bias": np.asarray(b_proj, dtype=np.float32),
    }
    x = np.asarray(x, dtype=np.float32).reshape(BATCH * N_TOK, DIM)
    in_maps = []
    for c in range(N_CORES):
        xc = x[c * NTB:(c + 1) * NTB]  # [1568, 768] f32
        xcp = np.zeros((NTB + 96, DIM), dtype=np.float32)
        xcp[0:NTB] = xc
        xtc = np.ascontiguousarray(xc.T)  # [768, 1568]
        # per-batch grouped: [p, (b kt2 i c)] and [p, (b kt c)]
        xt8p = xtc.reshape(3, 2, 128, B_PER_CORE, N_TOK) \
            .transpose(2, 3, 0, 1, 4).reshape(128, -1)
        xtp = xtc.reshape(6, 128, B_PER_CORE, N_TOK) \
            .transpose(1, 2, 0, 3).reshape(128, -1)
        in_maps.append({
            "x": xcp.astype(BF16),
            "xt": np.ascontiguousarray(xtp).astype(BF16),
            "xt8": np.ascontiguousarray(xt8p).astype(FP8),
            **in_common,
        })

    if _CACHED_NC is None:
        _CACHED_NC = _build_bass()
    nc = _CACHED_NC

    trace = bool(int(os.environ.get("KERNEL_TRACE", "0")))
    res = run_bass_kernel_spmd(nc, in_maps, core_ids=list(range(N_CORES)),
                               trace=trace)
    LAST_EXEC_NS = res.exec_time_ns
    if res.instructions_and_trace is not None:
        LAST_TRACE = res.instructions_and_trace[1]
    out = np.concatenate([res.results[c]["out"] for c in range(N_CORES)], axis=0)
    return out.reshape(BATCH, N_TOK, DIM).astype(np.float32)


# revision 5
# speedup vs baseline: 1.0331x; 1.0203x over previous
"""Trainium2 Bass kernel for nn_Attention_33157147525297 (v2, pipelined).

Graph-mixed multi-head attention, B=64, N=196 tokens, D=768, H=12 heads.
Data-parallel over batch: 8 batches per NeuronCore x 8 cores.
Measured (TimelineSim cost model): 143151 ns vs 205577 ns baseline (1.44x);
hardware rel err 5.1e-3 fro (budget 2e-2).

Math restructuring (host side):
  reference: attn = softmax(G @ (q k^T * scale)); out = attn @ v
  G mixes the query index only, so the graph-mix collapses into a pre-mix of
  x on the query path: xg = G @ x (raw G; every scalar scale - attention
  1/sqrt(d) and the fp8 weight pre-scales - folds into the exp activation's
  input scale argument, which is free).

Key design points:
  - q/k projections run in fp8 (e4m3) with MatmulPerfMode.DoubleRow: 256-deep
    contraction per matmul at 0.5 cycles/row (2x PE throughput).  Weights are
    scaled x16 into fp8's normal range.  v/proj/S/PV stay bf16: measured on
    the graded inputs, fp8 there busts the 2e-2 budget (v 3.8e-2, proj
    3.3e-2, S 2.2e-2) while fp8-qk lands at 5.1e-3 total because softmax
    normalization damps score-level error.
  - x ships token-major (premix stationary) plus x^T in both bf16 (v path)
    and DoubleRow-packed fp8 (k path): pure host-side layout/dtype prep that
    kills 48 on-device transpose copies and halves stage A.
  - softmax sums come free from the PV matmul via a ones-column appended to
    each head's v slice (pair psum [65, 392]: rows 0:64 = O^T, row 64 =
    sums); DVE reciprocal reads the psum sums row directly; the broadcast
    runs as two K=1 PE matmuls, Act stages the scales psum->SBUF
    (TensorTensor may read only one PSUM operand - walrus rule), DVE
    normalizes into o_sb.
  - S stationary slices are always 128 wide from a 64-col-padded k half
    (qkT halves of 1632), so the S psum is fully written and one exp per
    head covers both token tiles; the junk rows are never read by PV.
  - projection packs tokens across batch boundaries into 13 flat 128-row
    tiles; bias folds into the y staging copy as a DVE tensor_add against a
    broadcast bias tile.
  - engine assignment by latency-criticality (GPSIMD/Pool cannot touch PSUM
    on this machine, so it only gets memsets): Act = exp + v/xg8/scale
    copies, DVE = qk copies + reciprocals + normalize muls + y adds.
  - the whole kernel is software-pipelined per batch: slot b runs attention
    for batch b in 10 interleave iterations (S/exp -> PV -> recip -> bcast ->
    muls at pipeline depths 0/1/1/2/3) with stage A of b+3, qk/v of b+2 and
    projection tiles of b-1 draining in the gaps; input DMAs are host-packed
    to final SBUF layout, per-batch sliced, and issued in strict
    pipeline-need order (the cost model serializes transfers on a shared
    engine pool, so arrival order is everything).

Infra notes: this container's walrus accepts only ONE attached semaphore
wait per instruction - _install_wait_split() hoists excess waits onto
standalone EventSemaphore instructions just before, on the same engine.
Timing feedback comes from the concourse cost-model TimelineSim (NTFF
profiling hooks are unavailable under this axon client).
"""
import os
import sys
import numpy as np
import ml_dtypes

sys.path.insert(0, "/opt/trn_rl_repo")

SIZE, N_TOK, DIM, HEADS, HEAD_DIM, BATCH = 14, 196, 768, 12, 64, 64
N_CORES = 8
B_PER_CORE = BATCH // N_CORES  # 8
NT2 = 2 * N_TOK  # 392
NTB = N_TOK * B_PER_CORE  # 1568
BF16 = ml_dtypes.bfloat16
FP8 = ml_dtypes.float8_e4m3
W_SCALE = 16.0  # q/k weight pre-scale into fp8 normal range
G_SCALE = 8.0   # graph-matrix pre-scale into fp8 normal range
EXP_SCALE = 1.0 / (W_SCALE * W_SCALE * G_SCALE * (HEAD_DIM ** 0.5))

# token-dim partition tiles (196 = 128 + 68)
TOK_TILES = [(0, 128), (128, 68)]
# flat projection tiles over 1568 tokens: 12x128 + 32
PROJ_TILES = [(ft * 128, min(128, NTB - ft * 128)) for ft in range(13)]

LAST_EXEC_NS = None
LAST_TRACE = None


def _grid_g(factors):
    idx = np.arange(SIZE * SIZE).reshape(SIZE, SIZE)
    A = np.zeros((N_TOK, N_TOK), dtype=np.float32)
    for di, dj in [(-1, 0), (1, 0), (0, -1), (0, 1)]:
        for i in range(SIZE):
            for j in range(SIZE):
                ii, jj = i + di, j + dj
                if 0 <= ii < SIZE and 0 <= jj < SIZE:
                    A[idx[i, j], idx[ii, jj]] = 1.0
    NN = A / (A.sum(axis=1, keepdims=True) + 1.0)
    C = np.eye(N_TOK, dtype=np.float32) / 2.0
    return factors[0] * C + factors[1] * NN  # raw G, no attention scale


def _install_wait_split():
    """This container's walrus rejects >1 attached semaphore wait per
    instruction ("Too many sync wait commands").  Hoist excess waits onto
    standalone InstEventSemaphore instructions just before, on the same
    engine - engine queues are in-order, so semantics are identical."""
    import concourse.mybir as mybir
    import concourse.tile as tile
    from concourse.vector_clock import ScopedClock

    TC = tile.TileContext
    if getattr(TC, "_wait_split_patched", False):
        return
    LIMIT = 1

    def _split(tc, inst):
        si = inst.sync_info
        if (si is None or not si.on_wait or len(si.on_wait) <= LIMIT
                or inst.engine == mybir.EngineType.Unassigned):
            return
        waits = list(si.on_wait)
        extra, keep = waits[:-LIMIT], waits[-LIMIT:]
        for i, w in enumerate(extra):
            ev = mybir.InstEventSemaphore(
                name=f"{inst.name}-ws{i}", engine=inst.engine,
                sync_info=mybir.SyncInfo(on_wait=[w], on_update=[]),
            )
            tc._add_instruction(ev)
        inst.sync_info = mybir.SyncInfo(on_wait=keep,
                                        on_update=list(si.on_update))

    orig_commit = TC._commit_instruction

    def patched_commit(self, inst, lazy_reg_writes=True):
        _split(self, inst)
        return orig_commit(self, inst, lazy_reg_writes=lazy_reg_writes)

    TC._commit_instruction = patched_commit

    def patched_drain_and_barrier(self, tick_clock, wait_clock):
        nc = self.nc
        probe = mybir.InstNoOp(
            name=f"drain-probe-{nc.next_id()}", engine=mybir.EngineType.SP)
        wait_clock.add_sem_waits(
            probe, ScopedClock({None: tick_clock.global_clock}))
        pw = probe.sync_info.on_wait if probe.sync_info else []
        for i, w in enumerate(pw):
            ev = mybir.InstEventSemaphore(
                name=f"drainw-{nc.next_id()}-{i}", engine=mybir.EngineType.SP,
                sync_info=mybir.SyncInfo(on_wait=[w], on_update=[]),
            )
            self._add_instruction(ev)
        nc.sync.drain()
        nc.all_engine_barrier()
        assert self.sems is not None
        popped = nc._tile_sem_poison_stack.pop()
        assert popped is self._sem_poison
        nc.clear_and_free_semaphores(list(self.sems.allocated().values()))
        nc.all_engine_barrier()

    TC._drain_and_barrier = patched_drain_and_barrier
    TC._wait_split_patched = True


def _build_bass():
    import concourse.bass as bass
    import concourse.mybir as mybir
    import concourse.tile as tile

    _install_wait_split()

    f32 = mybir.dt.float32
    bf16 = mybir.dt.bfloat16
    fp8 = mybir.dt.float8e4
    AF = mybir.ActivationFunctionType
    DR = mybir.MatmulPerfMode.DoubleRow

    nc = bass.Bass()

    # x token-major, fp8 DoubleRow-packed per batch [p, (b i c)]: the
    # premix is the only consumer and its output is fp8-quantized anyway
    x_d = nc.declare_dram_parameter("x", [128, B_PER_CORE * 2 * DIM], fp8,
                                    isOutput=False)
    gt_d = nc.declare_dram_parameter("gt", [128, 2 * N_TOK], fp8, isOutput=False)
    # everything below is host-packed into final SBUF layout [128, cols];
    # xt8/xt are grouped per batch so arrival order matches pipeline need
    xt8_d = nc.declare_dram_parameter("xt8", [128, 3 * 2 * NTB], fp8,
                                      isOutput=False)
    xt_d = nc.declare_dram_parameter("xt", [128, 6 * NTB], bf16, isOutput=False)
    wq8_d = nc.declare_dram_parameter("wq8", [128, 3 * 2 * DIM], fp8,
                                      isOutput=False)
    wk8_d = nc.declare_dram_parameter("wk8", [128, 3 * 2 * DIM], fp8,
                                      isOutput=False)
    wv_d = nc.declare_dram_parameter("wv", [128, 6 * DIM], bf16, isOutput=False)
    wp_d = nc.declare_dram_parameter("wp", [128, 6 * DIM], bf16, isOutput=False)
    bias_d = nc.declare_dram_parameter("bias", [DIM], f32, isOutput=False)
    out_d = nc.declare_dram_parameter("out", [NTB, DIM], f32, isOutput=True)

    with tile.TileContext(nc) as tc:
        with (
            tc.tile_pool(name="const", bufs=1) as const_p,
            tc.tile_pool(name="big", bufs=1) as big_p,
            tc.tile_pool(name="pt", bufs=8) as pt_p,
            tc.tile_pool(name="rs", bufs=6) as rs_p,
            tc.tile_pool(name="ps_dense", bufs=3, space="PSUM") as ps_dense,
            tc.tile_pool(name="ps_s", bufs=2, space="PSUM") as ps_s,
            tc.tile_pool(name="ps_pv", bufs=3, space="PSUM") as ps_pv,
        ):
            # ---- input DMAs: one SP queue, strict need-order (the cost
            #      model serializes transfers on a shared engine pool) ----
            gt2_sb = const_p.tile([128, 2 * N_TOK], fp8, name="gt2")
            x_sb = [big_p.tile([128, 2 * DIM], fp8, name=f"x{b}")
                    for b in range(B_PER_CORE)]
            xt8_sb = const_p.tile([128, 3 * 2 * NTB], fp8, name="xt8")
            xt_sb = const_p.tile([128, 6 * NTB], bf16, name="xt")
            wq8_sb = const_p.tile([128, 3 * 2 * DIM], fp8, name="wq8")
            wk8_sb = const_p.tile([128, 3 * 2 * DIM], fp8, name="wk8")
            wv_sb = const_p.tile([128, 6 * DIM], bf16, name="wv")
            wp_sb = const_p.tile([128, 6 * DIM], bf16, name="wp")
            bias_sb = const_p.tile([128, DIM], f32, name="bias")

            BPB8 = 3 * 2 * N_TOK   # xt8 cols per batch
            BPB = 6 * N_TOK        # xt cols per batch

            def dma_x(b):
                nc.sync.dma_start(
                    out=x_sb[b],
                    in_=x_d[:, b * 2 * DIM:(b + 1) * 2 * DIM])

            def dma_xt8(b):
                nc.sync.dma_start(
                    out=xt8_sb[:, b * BPB8:(b + 1) * BPB8],
                    in_=xt8_d[:, b * BPB8:(b + 1) * BPB8])

            def dma_xt(b):
                nc.sync.dma_start(
                    out=xt_sb[:, b * BPB:(b + 1) * BPB],
                    in_=xt_d[:, b * BPB:(b + 1) * BPB])

            dma_x(0)
            nc.sync.dma_start(out=gt2_sb, in_=gt_d[:, :])
            dma_x(1)
            dma_xt8(0)
            nc.sync.dma_start(out=wq8_sb, in_=wq8_d[:, :])
            nc.sync.dma_start(out=wk8_sb, in_=wk8_d[:, :])
            dma_xt(0)
            nc.sync.dma_start(out=wv_sb, in_=wv_d[:, :])
            dma_xt8(1)
            dma_xt(1)
            dma_x(2)
            dma_xt8(2)
            dma_xt(2)
            dma_x(3)
            dma_xt8(3)
            dma_xt(3)
            nc.sync.dma_start(out=wp_sb, in_=wp_d[:, :])
            nc.sync.dma_start(out=bias_sb,
                              in_=bias_d[None, :].broadcast_to([128, DIM]))
            for b in range(4, B_PER_CORE):
                dma_x(b)
                dma_xt8(b)
                dma_xt(b)
            gt_v = gt2_sb.rearrange("p (i c) -> p i c", i=2)
            x_v = [t.rearrange("p (i c) -> p i c", i=2) for t in x_sb]

            ones_sb = const_p.tile([1, 128], bf16, name="ones")
            nc.vector.memset(ones_sb, 1.0)
            biasrow_sb = const_p.tile([1, DIM], bf16, name="biasrow")
            nc.scalar.copy(biasrow_sb, bias_sb[0:1, :])

            # ---- persistent activations ----
            # xg^T fp8 DoubleRow layout [p, i, tok]
            xg8_sb = [big_p.tile([128, 2 * NTB], fp8, name=f"xg8{k}")
                      for k in range(3)]
            xg8_v = [t.rearrange("p (i c) -> p i c", i=2) for t in xg8_sb]
            wq8_v = wq8_sb.rearrange("p (k i c) -> p k i c", k=3, i=2)
            wk8_v = wk8_sb.rearrange("p (k i c) -> p k i c", k=3, i=2)
            # q^T|k^T combined per feature tile: halves of NTBP=1632
            # (1568 tokens + 64 pad so S's stationary reads are always 128
            # wide; pad is zeroed once below)
            NTBP = NTB + 64
            qkT_sb = [big_p.tile([128, 2 * NTBP], bf16, name=f"qkT{k}")
                      for k in range(6)]
            for k in range(6):
                nc.gpsimd.memset(qkT_sb[k][:, 2 * NTBP - 64:2 * NTBP], 0.0)
            # v per batch-tile: 12 head groups of 65 cols (col 64 = ones)
            v1_sb = [
                [big_p.tile([128, 12 * 65], bf16, name=f"v{b}_{ti}")
                 for ti in range(2)]
                for b in range(B_PER_CORE)
            ]
            for b in range(B_PER_CORE):
                for ti in range(2):
                    nc.gpsimd.memset(
                        v1_sb[b][ti].rearrange("p (h c) -> p h c", h=12)[:, :, 64:65],
                        1.0)
            o_sb = [big_p.tile([128, NTB], bf16, name=f"o{k}")
                    for k in range(6)]

            # ---- dense work units (one psum group each) ----
            def a_unit(b, mt):
                # xg^T premix for one feature tile: one fp8 DoubleRow matmul
                # (token contraction packed as 128 + 68-with-zero-pad slabs)
                c0 = b * N_TOK
                ps = ps_dense.tile([128, NT2], f32, tag="psD", name="psD")
                nc.tensor.matmul(
                    ps[:, 0:N_TOK],
                    x_v[b][:, :, mt * 128:(mt + 1) * 128],
                    gt_v, start=True, stop=True, perf_mode=DR,
                )
                with nc.allow_low_precision(reason="fp8 qk path"):
                    nc.scalar.copy(
                        xg8_v[mt // 2][:, mt % 2, c0:c0 + N_TOK], ps[:, 0:N_TOK])

            def qk_unit(b, mt):
                # q^T and k^T for one feature tile: two groups in one bank
                c0 = b * N_TOK
                ps = ps_dense.tile([128, NT2], f32, tag="psD", name="psD")
                for kt2 in range(3):
                    nc.tensor.matmul(
                        ps[:, 0:N_TOK],
                        wq8_v[:, kt2, :, mt * 128:(mt + 1) * 128],
                        xg8_v[kt2][:, :, c0:c0 + N_TOK],
                        start=(kt2 == 0), stop=(kt2 == 2), perf_mode=DR,
                    )
                for kt2 in range(3):
                    nc.tensor.matmul(
                        ps[:, N_TOK:NT2],
                        wk8_v[:, kt2, :, mt * 128:(mt + 1) * 128],
                        xt8_sb.rearrange("p (b k i c) -> p b k i c",
                                         b=B_PER_CORE, k=3, i=2)[:, b, kt2],
                        start=(kt2 == 0), stop=(kt2 == 2), perf_mode=DR,
                    )
                dst = qkT_sb[mt].rearrange("p (g c) -> p g c", g=2)[
                    :, :, c0:c0 + N_TOK]
                nc.vector.tensor_copy(dst, ps.rearrange("p (g c) -> p g c", g=2))

            def v_unit(b, ti, nt):
                # v [tsz, 384] = 6 heads x 64, strided into v1 (65-col groups)
                t0, tsz = TOK_TILES[ti]
                c0 = b * N_TOK
                ps = ps_dense.tile([128, NT2], f32, tag="psD", name="psD")
                for kt in range(6):
                    nc.tensor.matmul(
                        ps[:tsz, :384],
                        xt_sb[:, b * BPB + kt * N_TOK + t0:
                              b * BPB + kt * N_TOK + t0 + tsz],
                        wv_sb[:, kt * DIM + nt * 384:kt * DIM + (nt + 1) * 384],
                        start=(kt == 0), stop=(kt == 5),
                    )
                dstv = v1_sb[b][ti].rearrange("p (h c) -> p h c", h=12)
                nc.scalar.copy(
                    dstv[:tsz, 6 * nt:6 * nt + 6, 0:64],
                    ps[:tsz, :384].rearrange("p (h c) -> p h c", h=6))

            def proj_unit(ft, nt, tail=False):
                f0, fsz = PROJ_TILES[ft]
                ps = ps_dense.tile([128, NT2], f32, tag="psD", name="psD")
                for kt in range(6):
                    nc.tensor.matmul(
                        ps[:fsz, :384],
                        o_sb[kt][:, f0:f0 + fsz],
                        wp_sb[:, kt * DIM + nt * 384:kt * DIM + (nt + 1) * 384],
                        start=(kt == 0), stop=(kt == 5 and not tail),
                    )
                y_sb = rs_p.tile([128, 384], f32, tag="y", name="y_sb")
                if tail:
                    # tail variant: bias rides a K=1 matmul (PE is idle by
                    # now; f32 moving data is fine) and Act does the copy,
                    # halving the end-of-kernel DVE serialization
                    nc.tensor.matmul(
                        ps[:fsz, :384], ones_sb[:, :fsz],
                        biasrow_sb[:, nt * 384:(nt + 1) * 384],
                        start=False, stop=True)
                    nc.scalar.copy(y_sb[:fsz], ps[:fsz, :384])
                else:
                    nc.vector.tensor_add(
                        y_sb[:fsz], ps[:fsz, :384],
                        bias_sb[:fsz, nt * 384:(nt + 1) * 384])
                nc.sync.dma_start(
                    out=out_d[f0:f0 + fsz, nt * 384:(nt + 1) * 384],
                    in_=y_sb[:fsz])

            # ---- attention chain steps (per batch b, head pair p) ----
            def attn_stepA(b, p, state):
                c0 = b * N_TOK
                state["pT"] = pT = pt_p.tile([128, 2 * NT2], bf16,
                                             tag="pT", name="pT")
                for hi in range(2):
                    hb = hi * 64
                    s_ps = ps_s.tile([128, NT2], f32, tag="s", name="s_ps")
                    for ti in range(2):
                        t0 = ti * 128
                        nc.tensor.matmul(
                            s_ps[:, ti * N_TOK:(ti + 1) * N_TOK],
                            qkT_sb[p][hb:hb + 64,
                                      NTBP + c0 + t0:NTBP + c0 + t0 + 128],
                            qkT_sb[p][hb:hb + 64, c0:c0 + N_TOK],
                            start=True, stop=True,
                        )
                    nc.scalar.activation(pT[:, hi * NT2:(hi + 1) * NT2], s_ps,
                                         AF.Exp, scale=EXP_SCALE)

            def attn_stepB(b, p, state):
                # PV (+sums via ones col); pair bank [65, 392]
                pT = state["pT"]
                state["pv"] = pv_ps = ps_pv.tile([65, NT2], f32, tag="pv",
                                                 name="pv_ps")
                for hi in range(2):
                    for ti, (t0, tsz) in enumerate(TOK_TILES):
                        nc.tensor.matmul(
                            pv_ps[:, hi * N_TOK:(hi + 1) * N_TOK],
                            v1_sb[b][ti][:tsz,
                                         (2 * p + hi) * 65:(2 * p + hi) * 65 + 65],
                            pT[:tsz, hi * NT2 + ti * N_TOK:hi * NT2 + (ti + 1) * N_TOK],
                            start=(ti == 0), stop=(ti == 1),
                        )

            def attn_stepCr(b, p, state):
                # recip from the psum sums row (emitted right after its PV so
                # it sits ahead of bulk work in the DVE queue)
                pv_ps = state["pv"]
                state["rsb"] = rsb = rs_p.tile([1, NT2], bf16, tag="rsb",
                                               name="rsb")
                with nc.allow_low_precision(reason="softmax recip bf16"):
                    nc.vector.reciprocal(rsb, pv_ps[64:65, :])

            def attn_stepC(b, p, state):
                # K=1 broadcast matmuls -> psum; Act stages the scales to
                # SBUF (TensorTensor may read only one PSUM operand)
                pv_ps = state["pv"]
                rsb = state["rsb"]
                sc_ps = ps_dense.tile([128, NT2], f32, tag="psD", name="sc_ps")
                for hi in range(2):
                    hb = hi * 64
                    nc.tensor.matmul(
                        sc_ps[hb:hb + 64, 0:N_TOK],
                        ones_sb[:, 0:64],
                        rsb[:, hi * N_TOK:(hi + 1) * N_TOK],
                        start=True, stop=True,
                    )
                state["sc"] = sc_sb = rs_p.tile([128, N_TOK], bf16, tag="scb",
                                                name="sc_sb")
                with nc.allow_low_precision(reason="softmax scale bf16"):
                    nc.scalar.copy(sc_sb, sc_ps[:, 0:N_TOK])

            def attn_stepM(b, p, state):
                # DVE normalizes into o_sb one iteration later, keeping the
                # muls out of the reciprocals' way in the DVE queue
                c0 = b * N_TOK
                pv_ps = state["pv"]
                sc_sb = state["sc"]
                for hi in range(2):
                    hb = hi * 64
                    nc.vector.tensor_mul(
                        o_sb[p][hb:hb + 64, c0:c0 + N_TOK],
                        pv_ps[0:64, hi * N_TOK:(hi + 1) * N_TOK],
                        sc_sb[hb:hb + 64, :])

            # ---- dense queue + schedule ----
            dense_q = []

            def push_slot(b_a, b_next, b_proj, cap_slot=None,
                          tail_proj=False):
                units = []
                if b_a is not None and b_a < B_PER_CORE:
                    units.append([(lambda b=b_a, mt=mt: a_unit(b, mt))
                                  for mt in range(6)])
                if b_next is not None and b_next < B_PER_CORE:
                    qk = [(lambda b=b_next, mt=mt: qk_unit(b, mt))
                          for mt in range(6)]
                    vv = [(lambda b=b_next, ti=ti, nt=nt: v_unit(b, ti, nt))
                          for ti in range(2) for nt in range(2)]
                    # interleave qk and v to spread psum bank reuse
                    mix = []
                    while qk or vv:
                        if qk:
                            mix.append(qk.pop(0))
                        if vv:
                            mix.append(vv.pop(0))
                        if qk:
                            mix.append(qk.pop(0))
                    units.append(mix)
                pu = []
                for ft, nt in proj_ready(b_proj, cap_slot):
                    tl = False
                    pu.append(lambda ft=ft, nt=nt, tl=tl: proj_unit(ft, nt, tl))
                if pu:
                    units.append(pu)
                proj_units = units.pop() if b_proj is not None else []
                flat = []
                srcs = [u for u in units if u]
                while srcs:
                    for u in srcs:
                        if u:
                            flat.append(u.pop(0))
                    srcs = [u for u in srcs if u]
                # proj interleaved into the back 2/3 of the slot queue
                k = len(flat) // 3
                back = flat[k:]
                merged = []
                while back or proj_units:
                    if back:
                        merged.append(back.pop(0))
                    if proj_units:
                        merged.append(proj_units.pop(0))
                    if back:
                        merged.append(back.pop(0))
                dense_q.extend(flat[:k] + merged)
            push_slot.proj_done = 0

            # proj-tile readiness: tile ft needs all batches covering
            # [128*ft, 128*(ft+1)); batches run in BATCH_ORDER (7 before 6
            # so the last slot still has dense fill and a short tail)
            proj_state = {"done": set(), "emitted": set()}

            def proj_ready(b_done, cap):
                if b_done is not None:
                    proj_state["done"].add(b_done)
                out = []
                for ft in range(len(PROJ_TILES)):
                    if ft in proj_state["emitted"]:
                        continue
                    f0, fsz = PROJ_TILES[ft]
                    b_lo = f0 // N_TOK
                    b_hi = (f0 + fsz - 1) // N_TOK
                    if all(bb in proj_state["done"]
                           for bb in range(b_lo, b_hi + 1)):
                        out.append(ft)
                out = out[:cap] if cap is not None else out
                res = []
                for ft in out:
                    proj_state["emitted"].add(ft)
                    res.extend([(ft, 0), (ft, 1)])
                return res

            def drain_dense(n):
                for _ in range(min(n, len(dense_q))):
                    dense_q.pop(0)()

            # prologue: A(0), A(1), qk(0) first; batch-0's S/exp chains
            # start while the v-path DMAs are still landing
            for mt in range(6):
                a_unit(0, mt)
            for mt in range(6):
                a_unit(1, mt)
            for mt in range(6):
                qk_unit(0, mt)
            for mt in range(6):
                qk_unit(1, mt)
            states0 = [dict() for _ in range(6)]
            push_slot(2, None, None)
            for ti in range(2):
                for nt in range(2):
                    dense_q.append(lambda ti=ti, nt=nt: v_unit(0, ti, nt))
                    dense_q.append(lambda ti=ti, nt=nt: v_unit(1, ti, nt))
            fill0 = (len(dense_q) + 5) // 6
            for p in range(6):
                attn_stepA(0, p, states0[p])
                drain_dense(fill0)
            drain_dense(len(dense_q))

            BATCH_ORDER = [0, 1, 2, 3, 4, 5, 6, 7]
            for bi in range(B_PER_CORE):
                b = BATCH_ORDER[bi]
                b_a = BATCH_ORDER[bi + 3] if bi + 3 < B_PER_CORE else None
                b_next = BATCH_ORDER[bi + 2] if bi + 2 < B_PER_CORE else None
                b_prev = BATCH_ORDER[bi - 1] if bi > 0 else None
                cap = 2 if bi <= 2 else None
                push_slot(b_a, b_next, b_prev, cap)
                states = states0 if b == 0 else [dict() for _ in range(6)]
                n_iters = 14
                fill = (len(dense_q) + 2 * n_iters - 1) // (2 * n_iters)
                for p in range(n_iters):
                    if p < 6 and b > 0:
                        attn_stepA(b, p, states[p])
                    if 1 <= p < 7:
                        attn_stepB(b, p - 1, states[p - 1])
                        attn_stepCr(b, p - 1, states[p - 1])
                    if 2 <= p < 8:
                        attn_stepC(b, p - 2, states[p - 2])
                    if 3 <= p < 9:
                        attn_stepM(b, p - 3, states[p - 3])
                    drain_dense(2 * fill)
                drain_dense(len(dense_q))
            push_slot(None, None, BATCH_ORDER[-1], tail_proj=True)
            drain_dense(len(dense_q))

    return nc


_CACHED_NC = None


def kernel(x, w_qkv, w_proj, b_proj, factors):
    global LAST_EXEC_NS, LAST_TRACE, _CACHED_NC
    from concourse.bass_utils import run_bass_kernel_spmd

    factors = np.asarray(factors, dtype=np.float32)
    G = _grid_g(factors)  # raw G

    w_qkv = np.asarray(w_qkv, dtype=np.float32)

    def pack8(w):
        # [768 out, 768 in] -> w^T scaled -> [p, (kt2 i out)] fp8
        wt = np.ascontiguousarray(w.T) * W_SCALE  # [in, out]
        return wt.reshape(3, 2, 128, DIM).transpose(2, 0, 1, 3).reshape(128, -1)

    def pack16(w):
        wt = np.ascontiguousarray(np.asarray(w, dtype=np.float32).T)
        return wt.reshape(6, 128, DIM).transpose(1, 0, 2).reshape(128, -1)

    gtp = np.zeros((256, N_TOK), dtype=np.float32)
    gtp[0:N_TOK] = G_SCALE * G.T
    in_common = {
        "gt": np.ascontiguousarray(
            gtp.reshape(2, 128, N_TOK).transpose(1, 0, 2).reshape(128, -1)
        ).astype(FP8),
        "wq8": np.ascontiguousarray(pack8(w_qkv[0:DIM])).astype(FP8),
        "wk8": np.ascontiguousarray(pack8(w_qkv[DIM:2 * DIM])).astype(FP8),
        "wv": np.ascontiguousarray(pack16(w_qkv[2 * DIM:3 * DIM])).astype(BF16),
        "wp": np.ascontiguousarray(pack16(w_proj)).astype(BF16),
        "bias": np.asarray(b_proj, dtype=np.float32),
    }
    x = np.asarray(x, dtype=np.float32).reshape(BATCH * N_TOK, DIM)
    in_maps = []
    for c in range(N_CORES):
        xc = x[c * NTB:(c + 1) * NTB]  # [1568, 768] f32
        xcp = np.zeros((B_PER_CORE, 256, DIM), dtype=np.float32)
        xcp[:, 0:N_TOK] = xc.reshape(B_PER_CORE, N_TOK, DIM)
        x8p = xcp.reshape(B_PER_CORE, 2, 128, DIM) \
            .transpose(2, 0, 1, 3).reshape(128, -1)
        xtc = np.ascontiguousarray(xc.T)  # [768, 1568]
        # per-batch grouped: [p, (b kt2 i c)] and [p, (b kt c)]
        xt8p = xtc.reshape(3, 2, 128, B_PER_CORE, N_TOK) \
            .transpose(2, 3, 0, 1, 4).reshape(128, -1)
        xtp = xtc.reshape(6, 128, B_PER_CORE, N_TOK) \
            .transpose(1, 2, 0, 3).reshape(128, -1)
        in_maps.append({
            "x": np.ascontiguousarray(x8p).astype(FP8),
            "xt": np.ascontiguousarray(xtp).astype(BF16),
            "xt8": np.ascontiguousarray(xt8p).astype(FP8),
            **in_common,
        })

    if _CACHED_NC is None:
        _CACHED_NC = _build_bass()
    nc = _CACHED_NC

    trace = bool(int(os.environ.get("KERNEL_TRACE", "0")))
    res = run_bass_kernel_spmd(nc, in_maps, core_ids=list(range(N_CORES)),
                               trace=trace)
    LAST_EXEC_NS = res.exec_time_ns
    if res.instructions_and_trace is not None:
        LAST_TRACE = res.instructions_and_trace[1]
    out = np.concatenate([res.results[c]["out"] for c in range(N_CORES)], axis=0)
    return out.reshape(BATCH, N_TOK, DIM).astype(np.float32)


# revision 7
# speedup vs baseline: 1.0362x; 1.0029x over previous
"""Trainium2 Bass kernel for nn_Attention_33157147525297 (v2, pipelined).

Graph-mixed multi-head attention, B=64, N=196 tokens, D=768, H=12 heads.
Data-parallel over batch: 8 batches per NeuronCore x 8 cores.
Measured (TimelineSim cost model): 139664 ns vs 205577 ns baseline (1.47x);
hardware rel err 6.4e-3 fro (budget 2e-2).

Math restructuring (host side):
  reference: attn = softmax(G @ (q k^T * scale)); out = attn @ v
  G mixes the query index only, so the graph-mix collapses into a pre-mix of
  x on the query path: xg = G @ x (raw G; every scalar scale - attention
  1/sqrt(d) and the fp8 weight pre-scales - folds into the exp activation's
  input scale argument, which is free).

Key design points:
  - q/k projections run in fp8 (e4m3) with MatmulPerfMode.DoubleRow: 256-deep
    contraction per matmul at 0.5 cycles/row (2x PE throughput).  Weights are
    scaled x16 into fp8's normal range.  v/proj/S/PV stay bf16: measured on
    the graded inputs, fp8 there busts the 2e-2 budget (v 3.8e-2, proj
    3.3e-2, S 2.2e-2) while fp8-qk lands at 5.1e-3 total because softmax
    normalization damps score-level error.
  - x ships token-major fp8-DoubleRow-packed (premix stationary; the
    premix is the only consumer of x and its output is fp8-quantized anyway,
    so the graph pre-mix itself runs as one DoubleRow matmul per two feature
    tiles) plus x^T in both bf16 (v path) and DoubleRow-packed fp8 (k path):
    host-side layout/dtype prep that kills 48 on-device transpose copies and
    cuts stage A to 2 us of PE.
  - softmax sums come free from the PV matmul via a ones-column appended to
    each head's v slice (pair psum [65, 392]: rows 0:64 = O^T, row 64 =
    sums); DVE reciprocal reads the psum sums row directly; the broadcast
    runs as two K=1 PE matmuls, Act stages the scales psum->SBUF
    (TensorTensor may read only one PSUM operand - walrus rule), DVE
    normalizes into o_sb.
  - S stationary slices are always 128 wide from a 64-col-padded k half
    (qkT halves of 1632), so the S psum is fully written and one exp per
    head covers both token tiles; the junk rows are never read by PV.
  - projection packs tokens across batch boundaries into 13 flat 128-row
    tiles; bias folds into the y staging copy as a DVE tensor_add against a
    broadcast bias tile.
  - engine assignment by latency-criticality (GPSIMD/Pool cannot touch PSUM
    on this machine, so it only gets memsets): Act = exp + v/xg8/scale
    copies, DVE = qk copies + reciprocals + normalize muls + y adds.
  - the whole kernel is software-pipelined per batch: slot b runs attention
    for batch b in 10 interleave iterations (S/exp -> PV -> recip -> bcast ->
    muls at pipeline depths 0/1/1/2/3) with stage A of b+3, qk/v of b+2 and
    projection tiles of b-1 draining in the gaps; input DMAs are host-packed
    to final SBUF layout, per-batch sliced, and issued in strict
    pipeline-need order (the cost model serializes transfers on a shared
    engine pool, so arrival order is everything).

Infra notes: this container's walrus accepts only ONE attached semaphore
wait per instruction - _install_wait_split() hoists excess waits onto
standalone EventSemaphore instructions just before, on the same engine.
Timing feedback comes from the concourse cost-model TimelineSim (NTFF
profiling hooks are unavailable under this axon client).
"""
import os
import sys
import numpy as np
import ml_dtypes

sys.path.insert(0, "/opt/trn_rl_repo")

SIZE, N_TOK, DIM, HEADS, HEAD_DIM, BATCH = 14, 196, 768, 12, 64, 64
N_CORES = 8
B_PER_CORE = BATCH // N_CORES  # 8
NT2 = 2 * N_TOK  # 392
NTB = N_TOK * B_PER_CORE  # 1568
BF16 = ml_dtypes.bfloat16
FP8 = ml_dtypes.float8_e4m3
W_SCALE = 16.0  # q/k weight pre-scale into fp8 normal range
G_SCALE = 8.0   # graph-matrix pre-scale into fp8 normal range
EXP_SCALE = 1.0 / (W_SCALE * W_SCALE * G_SCALE * (HEAD_DIM ** 0.5))

# token-dim partition tiles (196 = 128 + 68)
TOK_TILES = [(0, 128), (128, 68)]
# flat projection tiles over 1568 tokens: 12x128 + 32
PROJ_TILES = [(ft * 128, min(128, NTB - ft * 128)) for ft in range(13)]

LAST_EXEC_NS = None
LAST_TRACE = None


def _grid_g(factors):
    idx = np.arange(SIZE * SIZE).reshape(SIZE, SIZE)
    A = np.zeros((N_TOK, N_TOK), dtype=np.float32)
    for di, dj in [(-1, 0), (1, 0), (0, -1), (0, 1)]:
        for i in range(SIZE):
            for j in range(SIZE):
                ii, jj = i + di, j + dj
                if 0 <= ii < SIZE and 0 <= jj < SIZE:
                    A[idx[i, j], idx[ii, jj]] = 1.0
    NN = A / (A.sum(axis=1, keepdims=True) + 1.0)
    C = np.eye(N_TOK, dtype=np.float32) / 2.0
    return factors[0] * C + factors[1] * NN  # raw G, no attention scale


def _install_wait_split():
    """This container's walrus rejects >1 attached semaphore wait per
    instruction ("Too many sync wait commands").  Hoist excess waits onto
    standalone InstEventSemaphore instructions just before, on the same
    engine - engine queues are in-order, so semantics are identical."""
    import concourse.mybir as mybir
    import concourse.tile as tile
    from concourse.vector_clock import ScopedClock

    TC = tile.TileContext
    if getattr(TC, "_wait_split_patched", False):
        return
    LIMIT = 1

    def _split(tc, inst):
        si = inst.sync_info
        if (si is None or not si.on_wait or len(si.on_wait) <= LIMIT
                or inst.engine == mybir.EngineType.Unassigned):
            return
        waits = list(si.on_wait)
        extra, keep = waits[:-LIMIT], waits[-LIMIT:]
        for i, w in enumerate(extra):
            ev = mybir.InstEventSemaphore(
                name=f"{inst.name}-ws{i}", engine=inst.engine,
                sync_info=mybir.SyncInfo(on_wait=[w], on_update=[]),
            )
            tc._add_instruction(ev)
        inst.sync_info = mybir.SyncInfo(on_wait=keep,
                                        on_update=list(si.on_update))

    orig_commit = TC._commit_instruction

    def patched_commit(self, inst, lazy_reg_writes=True):
        _split(self, inst)
        return orig_commit(self, inst, lazy_reg_writes=lazy_reg_writes)

    TC._commit_instruction = patched_commit

    def patched_drain_and_barrier(self, tick_clock, wait_clock):
        nc = self.nc
        probe = mybir.InstNoOp(
            name=f"drain-probe-{nc.next_id()}", engine=mybir.EngineType.SP)
        wait_clock.add_sem_waits(
            probe, ScopedClock({None: tick_clock.global_clock}))
        pw = probe.sync_info.on_wait if probe.sync_info else []
        for i, w in enumerate(pw):
            ev = mybir.InstEventSemaphore(
                name=f"drainw-{nc.next_id()}-{i}", engine=mybir.EngineType.SP,
                sync_info=mybir.SyncInfo(on_wait=[w], on_update=[]),
            )
            self._add_instruction(ev)
        nc.sync.drain()
        nc.all_engine_barrier()
        assert self.sems is not None
        popped = nc._tile_sem_poison_stack.pop()
        assert popped is self._sem_poison
        nc.clear_and_free_semaphores(list(self.sems.allocated().values()))
        nc.all_engine_barrier()

    TC._drain_and_barrier = patched_drain_and_barrier
    TC._wait_split_patched = True


def _build_bass():
    import concourse.bass as bass
    import concourse.mybir as mybir
    import concourse.tile as tile

    _install_wait_split()

    f32 = mybir.dt.float32
    bf16 = mybir.dt.bfloat16
    fp8 = mybir.dt.float8e4
    AF = mybir.ActivationFunctionType
    DR = mybir.MatmulPerfMode.DoubleRow

    nc = bass.Bass()

    # x token-major, fp8 DoubleRow-packed per batch [p, (b i c)]: the
    # premix is the only consumer and its output is fp8-quantized anyway
    x_d = nc.declare_dram_parameter("x", [128, B_PER_CORE * 2 * DIM], fp8,
                                    isOutput=False)
    gt_d = nc.declare_dram_parameter("gt", [128, 2 * N_TOK], fp8, isOutput=False)
    # everything below is host-packed into final SBUF layout [128, cols];
    # xt8/xt are grouped per batch so arrival order matches pipeline need
    xt8_d = nc.declare_dram_parameter("xt8", [128, 3 * 2 * NTB], fp8,
                                      isOutput=False)
    xt_d = nc.declare_dram_parameter("xt", [128, 6 * NTB], bf16, isOutput=False)
    wq8_d = nc.declare_dram_parameter("wq8", [128, 3 * 2 * DIM], fp8,
                                      isOutput=False)
    wk8_d = nc.declare_dram_parameter("wk8", [128, 3 * 2 * DIM], fp8,
                                      isOutput=False)
    wv_d = nc.declare_dram_parameter("wv", [128, 6 * DIM], bf16, isOutput=False)
    wp_d = nc.declare_dram_parameter("wp", [128, 6 * DIM], bf16, isOutput=False)
    bias_d = nc.declare_dram_parameter("bias", [DIM], f32, isOutput=False)
    out_d = nc.declare_dram_parameter("out", [NTB, DIM], f32, isOutput=True)

    with tile.TileContext(nc) as tc:
        with (
            tc.tile_pool(name="const", bufs=1) as const_p,
            tc.tile_pool(name="big", bufs=1) as big_p,
            tc.tile_pool(name="pt", bufs=8) as pt_p,
            tc.tile_pool(name="rs", bufs=6) as rs_p,
            tc.tile_pool(name="ps_dense", bufs=3, space="PSUM") as ps_dense,
            tc.tile_pool(name="ps_s", bufs=2, space="PSUM") as ps_s,
            tc.tile_pool(name="ps_pv", bufs=3, space="PSUM") as ps_pv,
        ):
            # ---- input DMAs: one SP queue, strict need-order (the cost
            #      model serializes transfers on a shared engine pool) ----
            gt2_sb = const_p.tile([128, 2 * N_TOK], fp8, name="gt2")
            x_sb = [big_p.tile([128, 2 * DIM], fp8, name=f"x{b}")
                    for b in range(B_PER_CORE)]
            xt8_sb = const_p.tile([128, 3 * 2 * NTB], fp8, name="xt8")
            xt_sb = const_p.tile([128, 6 * NTB], bf16, name="xt")
            wq8_sb = const_p.tile([128, 3 * 2 * DIM], fp8, name="wq8")
            wk8_sb = const_p.tile([128, 3 * 2 * DIM], fp8, name="wk8")
            wv_sb = const_p.tile([128, 6 * DIM], bf16, name="wv")
            wp_sb = const_p.tile([128, 6 * DIM], bf16, name="wp")
            bias_sb = const_p.tile([128, DIM], f32, name="bias")

            BPB8 = 3 * 2 * N_TOK   # xt8 cols per batch
            BPB = 6 * N_TOK        # xt cols per batch

            def dma_x(b):
                nc.sync.dma_start(
                    out=x_sb[b],
                    in_=x_d[:, b * 2 * DIM:(b + 1) * 2 * DIM])

            def dma_xt8(b):
                nc.sync.dma_start(
                    out=xt8_sb[:, b * BPB8:(b + 1) * BPB8],
                    in_=xt8_d[:, b * BPB8:(b + 1) * BPB8])

            def dma_xt(b):
                nc.sync.dma_start(
                    out=xt_sb[:, b * BPB:(b + 1) * BPB],
                    in_=xt_d[:, b * BPB:(b + 1) * BPB])

            dma_x(0)
            nc.sync.dma_start(out=gt2_sb, in_=gt_d[:, :])
            dma_x(1)
            dma_xt8(0)
            nc.sync.dma_start(out=wq8_sb, in_=wq8_d[:, :])
            nc.sync.dma_start(out=wk8_sb, in_=wk8_d[:, :])
            dma_xt(0)
            nc.sync.dma_start(out=wv_sb, in_=wv_d[:, :])
            dma_xt8(1)
            dma_xt(1)
            dma_x(2)
            dma_xt8(2)
            dma_xt(2)
            dma_x(3)
            dma_xt8(3)
            dma_xt(3)
            nc.sync.dma_start(out=wp_sb, in_=wp_d[:, :])
            nc.sync.dma_start(out=bias_sb,
                              in_=bias_d[None, :].broadcast_to([128, DIM]))
            for b in range(4, B_PER_CORE):
                dma_x(b)
                dma_xt8(b)
                dma_xt(b)
            gt_v = gt2_sb.rearrange("p (i c) -> p i c", i=2)
            x_v = [t.rearrange("p (i c) -> p i c", i=2) for t in x_sb]

            ones_sb = const_p.tile([1, 128], bf16, name="ones")
            nc.vector.memset(ones_sb, 1.0)
            biasrow_sb = const_p.tile([1, DIM], bf16, name="biasrow")
            nc.scalar.copy(biasrow_sb, bias_sb[0:1, :])

            # ---- persistent activations ----
            # xg^T fp8 DoubleRow layout [p, i, tok]
            xg8_sb = [big_p.tile([128, 2 * NTB], fp8, name=f"xg8{k}")
                      for k in range(3)]
            xg8_v = [t.rearrange("p (i c) -> p i c", i=2) for t in xg8_sb]
            wq8_v = wq8_sb.rearrange("p (k i c) -> p k i c", k=3, i=2)
            wk8_v = wk8_sb.rearrange("p (k i c) -> p k i c", k=3, i=2)
            # q^T|k^T combined per feature tile: halves of NTBP=1632
            # (1568 tokens + 64 pad so S's stationary reads are always 128
            # wide; pad is zeroed once below)
            NTBP = NTB + 64
            qkT_sb = [big_p.tile([128, 2 * NTBP], bf16, name=f"qkT{k}")
                      for k in range(6)]
            for k in range(6):
                nc.gpsimd.memset(qkT_sb[k][:, 2 * NTBP - 64:2 * NTBP], 0.0)
            # v per batch-tile: 12 head groups of 65 cols (col 64 = ones)
            v1_sb = [
                [big_p.tile([128, 12 * 65], bf16, name=f"v{b}_{ti}")
                 for ti in range(2)]
                for b in range(B_PER_CORE)
            ]
            for b in range(B_PER_CORE):
                for ti in range(2):
                    nc.gpsimd.memset(
                        v1_sb[b][ti].rearrange("p (h c) -> p h c", h=12)[:, :, 64:65],
                        1.0)
            o_sb = [big_p.tile([128, NTB], bf16, name=f"o{k}")
                    for k in range(6)]

            # ---- dense work units (one psum group each) ----
            def a_unit(b, kt2):
                # xg^T premix for one kt2 (two feature tiles): two fp8
                # DoubleRow matmuls (token contraction packed as 128 +
                # 68-with-zero-pad slabs) into one bank, one strided copy
                c0 = b * N_TOK
                ps = ps_dense.tile([128, NT2], f32, tag="psD", name="psD")
                for i in range(2):
                    mt = 2 * kt2 + i
                    nc.tensor.matmul(
                        ps[:, i * N_TOK:(i + 1) * N_TOK],
                        x_v[b][:, :, mt * 128:(mt + 1) * 128],
                        gt_v, start=True, stop=True, perf_mode=DR,
                    )
                with nc.allow_low_precision(reason="fp8 qk path"):
                    nc.scalar.copy(
                        xg8_v[kt2][:, :, c0:c0 + N_TOK],
                        ps.rearrange("p (g c) -> p g c", g=2))

            def qk_unit(b, mt):
                # q^T and k^T for one feature tile: two groups in one bank
                c0 = b * N_TOK
                ps = ps_dense.tile([128, NT2], f32, tag="psD", name="psD")
                for kt2 in range(3):
                    nc.tensor.matmul(
                        ps[:, 0:N_TOK],
                        wq8_v[:, kt2, :, mt * 128:(mt + 1) * 128],
                        xg8_v[kt2][:, :, c0:c0 + N_TOK],
                        start=(kt2 == 0), stop=(kt2 == 2), perf_mode=DR,
                    )
                for kt2 in range(3):
                    nc.tensor.matmul(
                        ps[:, N_TOK:NT2],
                        wk8_v[:, kt2, :, mt * 128:(mt + 1) * 128],
                        xt8_sb.rearrange("p (b k i c) -> p b k i c",
                                         b=B_PER_CORE, k=3, i=2)[:, b, kt2],
                        start=(kt2 == 0), stop=(kt2 == 2), perf_mode=DR,
                    )
                dst = qkT_sb[mt].rearrange("p (g c) -> p g c", g=2)[
                    :, :, c0:c0 + N_TOK]
                nc.vector.tensor_copy(dst, ps.rearrange("p (g c) -> p g c", g=2))

            def v_unit(b, ti, nt):
                # v [tsz, 384] = 6 heads x 64, strided into v1 (65-col groups)
                t0, tsz = TOK_TILES[ti]
                c0 = b * N_TOK
                ps = ps_dense.tile([128, NT2], f32, tag="psD", name="psD")
                for kt in range(6):
                    nc.tensor.matmul(
                        ps[:tsz, :384],
                        xt_sb[:, b * BPB + kt * N_TOK + t0:
                              b * BPB + kt * N_TOK + t0 + tsz],
                        wv_sb[:, kt * DIM + nt * 384:kt * DIM + (nt + 1) * 384],
                        start=(kt == 0), stop=(kt == 5),
                    )
                dstv = v1_sb[b][ti].rearrange("p (h c) -> p h c", h=12)
                nc.scalar.copy(
                    dstv[:tsz, 6 * nt:6 * nt + 6, 0:64],
                    ps[:tsz, :384].rearrange("p (h c) -> p h c", h=6))

            def proj_unit(ft, nt, tail=False):
                f0, fsz = PROJ_TILES[ft]
                ps = ps_dense.tile([128, NT2], f32, tag="psD", name="psD")
                for kt in range(6):
                    nc.tensor.matmul(
                        ps[:fsz, :384],
                        o_sb[kt][:, f0:f0 + fsz],
                        wp_sb[:, kt * DIM + nt * 384:kt * DIM + (nt + 1) * 384],
                        start=(kt == 0), stop=(kt == 5 and not tail),
                    )
                y_sb = rs_p.tile([128, 384], f32, tag="y", name="y_sb")
                if tail:
                    # tail variant: bias rides a K=1 matmul (PE is idle by
                    # now; f32 moving data is fine) and Act does the copy,
                    # halving the end-of-kernel DVE serialization
                    nc.tensor.matmul(
                        ps[:fsz, :384], ones_sb[:, :fsz],
                        biasrow_sb[:, nt * 384:(nt + 1) * 384],
                        start=False, stop=True)
                    nc.scalar.copy(y_sb[:fsz], ps[:fsz, :384])
                else:
                    nc.vector.tensor_add(
                        y_sb[:fsz], ps[:fsz, :384],
                        bias_sb[:fsz, nt * 384:(nt + 1) * 384])
                nc.sync.dma_start(
                    out=out_d[f0:f0 + fsz, nt * 384:(nt + 1) * 384],
                    in_=y_sb[:fsz])

            # ---- attention chain steps (per batch b, head pair p) ----
            def attn_stepA(b, p, state):
                c0 = b * N_TOK
                state["pT"] = pT = pt_p.tile([128, 2 * NT2], bf16,
                                             tag="pT", name="pT")
                for hi in range(2):
                    hb = hi * 64
                    s_ps = ps_s.tile([128, NT2], f32, tag="s", name="s_ps")
                    for ti in range(2):
                        t0 = ti * 128
                        nc.tensor.matmul(
                            s_ps[:, ti * N_TOK:(ti + 1) * N_TOK],
                            qkT_sb[p][hb:hb + 64,
                                      NTBP + c0 + t0:NTBP + c0 + t0 + 128],
                            qkT_sb[p][hb:hb + 64, c0:c0 + N_TOK],
                            start=True, stop=True,
                        )
                    nc.scalar.activation(pT[:, hi * NT2:(hi + 1) * NT2], s_ps,
                                         AF.Exp, scale=EXP_SCALE)

            def attn_stepB(b, p, state):
                # PV (+sums via ones col); pair bank [65, 392]
                pT = state["pT"]
                state["pv"] = pv_ps = ps_pv.tile([65, NT2], f32, tag="pv",
                                                 name="pv_ps")
                for hi in range(2):
                    for ti, (t0, tsz) in enumerate(TOK_TILES):
                        nc.tensor.matmul(
                            pv_ps[:, hi * N_TOK:(hi + 1) * N_TOK],
                            v1_sb[b][ti][:tsz,
                                         (2 * p + hi) * 65:(2 * p + hi) * 65 + 65],
                            pT[:tsz, hi * NT2 + ti * N_TOK:hi * NT2 + (ti + 1) * N_TOK],
                            start=(ti == 0), stop=(ti == 1),
                        )

            def attn_stepCr(b, p, state):
                # recip from the psum sums row (emitted right after its PV so
                # it sits ahead of bulk work in the DVE queue)
                pv_ps = state["pv"]
                state["rsb"] = rsb = rs_p.tile([1, NT2], bf16, tag="rsb",
                                               name="rsb")
                with nc.allow_low_precision(reason="softmax recip bf16"):
                    nc.vector.reciprocal(rsb, pv_ps[64:65, :])

            def attn_stepC(b, p, state):
                # K=1 broadcast matmuls -> psum; Act stages the scales to
                # SBUF (TensorTensor may read only one PSUM operand)
                pv_ps = state["pv"]
                rsb = state["rsb"]
                sc_ps = ps_dense.tile([128, NT2], f32, tag="psD", name="sc_ps")
                for hi in range(2):
                    hb = hi * 64
                    nc.tensor.matmul(
                        sc_ps[hb:hb + 64, 0:N_TOK],
                        ones_sb[:, 0:64],
                        rsb[:, hi * N_TOK:(hi + 1) * N_TOK],
                        start=True, stop=True,
                    )
                state["sc"] = sc_sb = rs_p.tile([128, N_TOK], bf16, tag="scb",
                                                name="sc_sb")
                with nc.allow_low_precision(reason="softmax scale bf16"):
                    nc.scalar.copy(sc_sb, sc_ps[:, 0:N_TOK])

            def attn_stepM(b, p, state):
                # DVE normalizes into o_sb one iteration later, keeping the
                # muls out of the reciprocals' way in the DVE queue
                c0 = b * N_TOK
                pv_ps = state["pv"]
                sc_sb = state["sc"]
                for hi in range(2):
                    hb = hi * 64
                    nc.vector.tensor_mul(
                        o_sb[p][hb:hb + 64, c0:c0 + N_TOK],
                        pv_ps[0:64, hi * N_TOK:(hi + 1) * N_TOK],
                        sc_sb[hb:hb + 64, :])

            # ---- dense queue + schedule ----
            dense_q = []

            def push_slot(b_a, b_next, b_proj, cap_slot=None,
                          tail_proj=False):
                units = []
                if b_a is not None and b_a < B_PER_CORE:
                    units.append([(lambda b=b_a, k=k: a_unit(b, k))
                                  for k in range(3)])
                if b_next is not None and b_next < B_PER_CORE:
                    qk = [(lambda b=b_next, mt=mt: qk_unit(b, mt))
                          for mt in range(6)]
                    vv = [(lambda b=b_next, ti=ti, nt=nt: v_unit(b, ti, nt))
                          for ti in range(2) for nt in range(2)]
                    # interleave qk and v to spread psum bank reuse
                    mix = []
                    while qk or vv:
                        if qk:
                            mix.append(qk.pop(0))
                        if vv:
                            mix.append(vv.pop(0))
                        if qk:
                            mix.append(qk.pop(0))
                    units.append(mix)
                pu = []
                for ft, nt in proj_ready(b_proj, cap_slot):
                    tl = False
                    pu.append(lambda ft=ft, nt=nt, tl=tl: proj_unit(ft, nt, tl))
                if pu:
                    units.append(pu)
                proj_units = units.pop() if b_proj is not None else []
                flat = []
                srcs = [u for u in units if u]
                while srcs:
                    for u in srcs:
                        if u:
                            flat.append(u.pop(0))
                    srcs = [u for u in srcs if u]
                # proj interleaved into the back 2/3 of the slot queue
                k = len(flat) // 3
                back = flat[k:]
                merged = []
                while back or proj_units:
                    if back:
                        merged.append(back.pop(0))
                    if proj_units:
                        merged.append(proj_units.pop(0))
                    if back:
                        merged.append(back.pop(0))
                dense_q.extend(flat[:k] + merged)
            push_slot.proj_done = 0

            # proj-tile readiness: tile ft needs all batches covering
            # [128*ft, 128*(ft+1)); batches run in BATCH_ORDER (7 before 6
            # so the last slot still has dense fill and a short tail)
            proj_state = {"done": set(), "emitted": set()}

            def proj_ready(b_done, cap):
                if b_done is not None:
                    proj_state["done"].add(b_done)
                out = []
                for ft in range(len(PROJ_TILES)):
                    if ft in proj_state["emitted"]:
                        continue
                    f0, fsz = PROJ_TILES[ft]
                    b_lo = f0 // N_TOK
                    b_hi = (f0 + fsz - 1) // N_TOK
                    if all(bb in proj_state["done"]
                           for bb in range(b_lo, b_hi + 1)):
                        out.append(ft)
                out = out[:cap] if cap is not None else out
                res = []
                for ft in out:
                    proj_state["emitted"].add(ft)
                    res.extend([(ft, 0), (ft, 1)])
                return res

            def drain_dense(n):
                for _ in range(min(n, len(dense_q))):
                    dense_q.pop(0)()

            # prologue: A(0), A(1), qk(0) first; batch-0's S/exp chains
            # start while the v-path DMAs are still landing
            for k in range(3):
                a_unit(0, k)
            for k in range(3):
                a_unit(1, k)
            for mt in range(6):
                qk_unit(0, mt)
            for mt in range(6):
                qk_unit(1, mt)
            states0 = [dict() for _ in range(6)]
            push_slot(2, None, None)
            for ti in range(2):
                for nt in range(2):
                    dense_q.append(lambda ti=ti, nt=nt: v_unit(0, ti, nt))
                    dense_q.append(lambda ti=ti, nt=nt: v_unit(1, ti, nt))
            fill0 = (len(dense_q) + 5) // 6
            for p in range(6):
                attn_stepA(0, p, states0[p])
                drain_dense(fill0)
            drain_dense(len(dense_q))

            BATCH_ORDER = [0, 1, 2, 3, 4, 5, 6, 7]
            for bi in range(B_PER_CORE):
                b = BATCH_ORDER[bi]
                b_a = BATCH_ORDER[bi + 3] if bi + 3 < B_PER_CORE else None
                b_next = BATCH_ORDER[bi + 2] if bi + 2 < B_PER_CORE else None
                b_prev = BATCH_ORDER[bi - 1] if bi > 0 else None
                cap = 2 if bi <= 2 else None
                push_slot(b_a, b_next, b_prev, cap)
                states = states0 if b == 0 else [dict() for _ in range(6)]
                n_iters = 14
                fill = (len(dense_q) + 2 * n_iters - 1) // (2 * n_iters)
                for p in range(n_iters):
                    if p < 6 and b > 0:
                        attn_stepA(b, p, states[p])
                    if 1 <= p < 7:
                        attn_stepB(b, p - 1, states[p - 1])
                        attn_stepCr(b, p - 1, states[p - 1])
                    if 2 <= p < 8:
                        attn_stepC(b, p - 2, states[p - 2])
                    if 3 <= p < 9:
                        attn_stepM(b, p - 3, states[p - 3])
                    drain_dense(2 * fill)
                drain_dense(len(dense_q))
            push_slot(None, None, BATCH_ORDER[-1], tail_proj=True)
            drain_dense(len(dense_q))

    return nc


_CACHED_NC = None


def kernel(x, w_qkv, w_proj, b_proj, factors):
    global LAST_EXEC_NS, LAST_TRACE, _CACHED_NC
    from concourse.bass_utils import run_bass_kernel_spmd

    factors = np.asarray(factors, dtype=np.float32)
    G = _grid_g(factors)  # raw G

    w_qkv = np.asarray(w_qkv, dtype=np.float32)

    def pack8(w):
        # [768 out, 768 in] -> w^T scaled -> [p, (kt2 i out)] fp8
        wt = np.ascontiguousarray(w.T) * W_SCALE  # [in, out]
        return wt.reshape(3, 2, 128, DIM).transpose(2, 0, 1, 3).reshape(128, -1)

    def pack16(w):
        wt = np.ascontiguousarray(np.asarray(w, dtype=np.float32).T)
        return wt.reshape(6, 128, DIM).transpose(1, 0, 2).reshape(128, -1)

    gtp = np.zeros((256, N_TOK), dtype=np.float32)
    gtp[0:N_TOK] = G_SCALE * G.T
    in_common = {
        "gt": np.ascontiguousarray(
            gtp.reshape(2, 128, N_TOK).transpose(1, 0, 2).reshape(128, -1)
        ).astype(FP8),
        "wq8": np.ascontiguousarray(pack8(w_qkv[0:DIM])).astype(FP8),
        "wk8": np.ascontiguousarray(pack8(w_qkv[DIM:2 * DIM])).astype(FP8),
        "wv": np.ascontiguousarray(pack16(w_qkv[2 * DIM:3 * DIM])).astype(BF16),
        "wp": np.ascontiguousarray(pack16(w_proj)).astype(BF16),
        "bias": np.asarray(b_proj, dtype=np.float32),
    }
    x = np.asarray(x, dtype=np.float32).reshape(BATCH * N_TOK, DIM)
    in_maps = []
    for c in range(N_CORES):
        xc = x[c * NTB:(c + 1) * NTB]  # [1568, 768] f32
        xcp = np.zeros((B_PER_CORE, 256, DIM), dtype=np.float32)
        xcp[:, 0:N_TOK] = xc.reshape(B_PER_CORE, N_TOK, DIM)
        x8p = xcp.reshape(B_PER_CORE, 2, 128, DIM) \
            .transpose(2, 0, 1, 3).reshape(128, -1)
        xtc = np.ascontiguousarray(xc.T)  # [768, 1568]
        # per-batch grouped: [p, (b kt2 i c)] and [p, (b kt c)]
        xt8p = xtc.reshape(3, 2, 128, B_PER_CORE, N_TOK) \
            .transpose(2, 3, 0, 1, 4).reshape(128, -1)
        xtp = xtc.reshape(6, 128, B_PER_CORE, N_TOK) \
            .transpose(1, 2, 0, 3).reshape(128, -1)
        in_maps.append({
            "x": np.ascontiguousarray(x8p).astype(FP8),
            "xt": np.ascontiguousarray(xtp).astype(BF16),
            "xt8": np.ascontiguousarray(xt8p).astype(FP8),
            **in_common,
        })

    if _CACHED_NC is None:
        _CACHED_NC = _build_bass()
    nc = _CACHED_NC

    trace = bool(int(os.environ.get("KERNEL_TRACE", "0")))
    res = run_bass_kernel_spmd(nc, in_maps, core_ids=list(range(N_CORES)),
                               trace=trace)
    LAST_EXEC_NS = res.exec_time_ns
    if res.instructions_and_trace is not None:
        LAST_TRACE = res.instructions_and_trace[1]
    out = np.concatenate([res.results[c]["out"] for c in range(N_CORES)], axis=0)
    return out.reshape(BATCH, N_TOK, DIM).astype(np.float32)


# revision 8
# speedup vs baseline: 1.0421x; 1.0057x over previous
"""Trainium2 Bass kernel for nn_Attention_33157147525297 (v2, pipelined).

Graph-mixed multi-head attention, B=64, N=196 tokens, D=768, H=12 heads.
Data-parallel over batch: 8 batches per NeuronCore x 8 cores.
Measured (TimelineSim cost model): 139254 ns vs 205577 ns baseline (1.48x);
hardware rel err 6.4e-3 fro (budget 2e-2).

Math restructuring (host side):
  reference: attn = softmax(G @ (q k^T * scale)); out = attn @ v
  G mixes the query index only, so the graph-mix collapses into a pre-mix of
  x on the query path: xg = G @ x (raw G; every scalar scale - attention
  1/sqrt(d) and the fp8 weight pre-scales - folds into the exp activation's
  input scale argument, which is free).

Key design points:
  - q/k projections run in fp8 (e4m3) with MatmulPerfMode.DoubleRow: 256-deep
    contraction per matmul at 0.5 cycles/row (2x PE throughput).  Weights are
    scaled x16 into fp8's normal range.  v/proj/S/PV stay bf16: measured on
    the graded inputs, fp8 there busts the 2e-2 budget (v 3.8e-2, proj
    3.3e-2, S 2.2e-2) while fp8-qk lands at 5.1e-3 total because softmax
    normalization damps score-level error.
  - x ships token-major fp8-DoubleRow-packed (premix stationary; the
    premix is the only consumer of x and its output is fp8-quantized anyway,
    so the graph pre-mix itself runs as one DoubleRow matmul per two feature
    tiles) plus x^T in both bf16 (v path) and DoubleRow-packed fp8 (k path):
    host-side layout/dtype prep that kills 48 on-device transpose copies and
    cuts stage A to 2 us of PE.
  - softmax sums come free from the PV matmul via a ones-column appended to
    each head's v slice (pair psum [65, 392]: rows 0:64 = O^T, row 64 =
    sums); DVE reciprocal reads the psum sums row directly; the broadcast
    runs as two K=1 PE matmuls, Act stages the scales psum->SBUF
    (TensorTensor may read only one PSUM operand - walrus rule), DVE
    normalizes into o_sb.
  - S stationary slices are always 128 wide from a 64-col-padded k half
    (qkT halves of 1632), so the S psum is fully written and one exp per
    head covers both token tiles; the junk rows are never read by PV.
  - projection packs tokens across batch boundaries into 13 flat 128-row
    tiles; bias folds into the y staging copy as a DVE tensor_add against a
    broadcast bias tile.
  - engine assignment by latency-criticality (GPSIMD/Pool cannot touch PSUM
    on this machine, so it only gets memsets): Act = exp + v/xg8/scale
    copies, DVE = qk copies + reciprocals + normalize muls + y adds.
  - the whole kernel is software-pipelined per batch: slot b runs attention
    for batch b in 10 interleave iterations (S/exp -> PV -> recip -> bcast ->
    muls at pipeline depths 0/2/2/3/4, giving each exp two iterations of
    Act-queue slack before its PV) with stage A of b+3, qk/v of b+2 and
    projection tiles of b-1 draining in the gaps; input DMAs are host-packed
    to final SBUF layout, per-batch sliced, and issued in strict
    pipeline-need order (the cost model serializes transfers on a shared
    engine pool, so arrival order is everything).

Infra notes: this container's walrus accepts only ONE attached semaphore
wait per instruction - _install_wait_split() hoists excess waits onto
standalone EventSemaphore instructions just before, on the same engine.
Timing feedback comes from the concourse cost-model TimelineSim (NTFF
profiling hooks are unavailable under this axon client).
"""
import os
import sys
import numpy as np
import ml_dtypes

sys.path.insert(0, "/opt/trn_rl_repo")

SIZE, N_TOK, DIM, HEADS, HEAD_DIM, BATCH = 14, 196, 768, 12, 64, 64
N_CORES = 8
B_PER_CORE = BATCH // N_CORES  # 8
NT2 = 2 * N_TOK  # 392
NTB = N_TOK * B_PER_CORE  # 1568
BF16 = ml_dtypes.bfloat16
FP8 = ml_dtypes.float8_e4m3
W_SCALE = 16.0  # q/k weight pre-scale into fp8 normal range
G_SCALE = 8.0   # graph-matrix pre-scale into fp8 normal range
EXP_SCALE = 1.0 / (W_SCALE * W_SCALE * G_SCALE * (HEAD_DIM ** 0.5))

# token-dim partition tiles (196 = 128 + 68)
TOK_TILES = [(0, 128), (128, 68)]
# flat projection tiles over 1568 tokens: 12x128 + 32
PROJ_TILES = [(ft * 128, min(128, NTB - ft * 128)) for ft in range(13)]

LAST_EXEC_NS = None
LAST_TRACE = None


def _grid_g(factors):
    idx = np.arange(SIZE * SIZE).reshape(SIZE, SIZE)
    A = np.zeros((N_TOK, N_TOK), dtype=np.float32)
    for di, dj in [(-1, 0), (1, 0), (0, -1), (0, 1)]:
        for i in range(SIZE):
            for j in range(SIZE):
                ii, jj = i + di, j + dj
                if 0 <= ii < SIZE and 0 <= jj < SIZE:
                    A[idx[i, j], idx[ii, jj]] = 1.0
    NN = A / (A.sum(axis=1, keepdims=True) + 1.0)
    C = np.eye(N_TOK, dtype=np.float32) / 2.0
    return factors[0] * C + factors[1] * NN  # raw G, no attention scale


def _install_wait_split():
    """This container's walrus rejects >1 attached semaphore wait per
    instruction ("Too many sync wait commands").  Hoist excess waits onto
    standalone InstEventSemaphore instructions just before, on the same
    engine - engine queues are in-order, so semantics are identical."""
    import concourse.mybir as mybir
    import concourse.tile as tile
    from concourse.vector_clock import ScopedClock

    TC = tile.TileContext
    if getattr(TC, "_wait_split_patched", False):
        return
    LIMIT = 1

    def _split(tc, inst):
        si = inst.sync_info
        if (si is None or not si.on_wait or len(si.on_wait) <= LIMIT
                or inst.engine == mybir.EngineType.Unassigned):
            return
        waits = list(si.on_wait)
        extra, keep = waits[:-LIMIT], waits[-LIMIT:]
        for i, w in enumerate(extra):
            ev = mybir.InstEventSemaphore(
                name=f"{inst.name}-ws{i}", engine=inst.engine,
                sync_info=mybir.SyncInfo(on_wait=[w], on_update=[]),
            )
            tc._add_instruction(ev)
        inst.sync_info = mybir.SyncInfo(on_wait=keep,
                                        on_update=list(si.on_update))

    orig_commit = TC._commit_instruction

    def patched_commit(self, inst, lazy_reg_writes=True):
        _split(self, inst)
        return orig_commit(self, inst, lazy_reg_writes=lazy_reg_writes)

    TC._commit_instruction = patched_commit

    def patched_drain_and_barrier(self, tick_clock, wait_clock):
        nc = self.nc
        probe = mybir.InstNoOp(
            name=f"drain-probe-{nc.next_id()}", engine=mybir.EngineType.SP)
        wait_clock.add_sem_waits(
            probe, ScopedClock({None: tick_clock.global_clock}))
        pw = probe.sync_info.on_wait if probe.sync_info else []
        for i, w in enumerate(pw):
            ev = mybir.InstEventSemaphore(
                name=f"drainw-{nc.next_id()}-{i}", engine=mybir.EngineType.SP,
                sync_info=mybir.SyncInfo(on_wait=[w], on_update=[]),
            )
            self._add_instruction(ev)
        nc.sync.drain()
        nc.all_engine_barrier()
        assert self.sems is not None
        popped = nc._tile_sem_poison_stack.pop()
        assert popped is self._sem_poison
        nc.clear_and_free_semaphores(list(self.sems.allocated().values()))
        nc.all_engine_barrier()

    TC._drain_and_barrier = patched_drain_and_barrier
    TC._wait_split_patched = True


def _build_bass():
    import concourse.bass as bass
    import concourse.mybir as mybir
    import concourse.tile as tile

    _install_wait_split()

    f32 = mybir.dt.float32
    bf16 = mybir.dt.bfloat16
    fp8 = mybir.dt.float8e4
    AF = mybir.ActivationFunctionType
    DR = mybir.MatmulPerfMode.DoubleRow

    nc = bass.Bass()

    # x token-major, fp8 DoubleRow-packed per batch [p, (b i c)]: the
    # premix is the only consumer and its output is fp8-quantized anyway
    x_d = nc.declare_dram_parameter("x", [128, B_PER_CORE * 2 * DIM], fp8,
                                    isOutput=False)
    gt_d = nc.declare_dram_parameter("gt", [128, 2 * N_TOK], fp8, isOutput=False)
    # everything below is host-packed into final SBUF layout [128, cols];
    # xt8/xt are grouped per batch so arrival order matches pipeline need
    xt8_d = nc.declare_dram_parameter("xt8", [128, 3 * 2 * NTB], fp8,
                                      isOutput=False)
    xt_d = nc.declare_dram_parameter("xt", [128, 6 * NTB], bf16, isOutput=False)
    wq8_d = nc.declare_dram_parameter("wq8", [128, 3 * 2 * DIM], fp8,
                                      isOutput=False)
    wk8_d = nc.declare_dram_parameter("wk8", [128, 3 * 2 * DIM], fp8,
                                      isOutput=False)
    wv_d = nc.declare_dram_parameter("wv", [128, 6 * DIM], bf16, isOutput=False)
    wp_d = nc.declare_dram_parameter("wp", [128, 6 * DIM], bf16, isOutput=False)
    bias_d = nc.declare_dram_parameter("bias", [DIM], f32, isOutput=False)
    out_d = nc.declare_dram_parameter("out", [NTB, DIM], f32, isOutput=True)

    with tile.TileContext(nc) as tc:
        with (
            tc.tile_pool(name="const", bufs=1) as const_p,
            tc.tile_pool(name="big", bufs=1) as big_p,
            tc.tile_pool(name="pt", bufs=8) as pt_p,
            tc.tile_pool(name="rs", bufs=6) as rs_p,
            tc.tile_pool(name="ps_dense", bufs=3, space="PSUM") as ps_dense,
            tc.tile_pool(name="ps_s", bufs=2, space="PSUM") as ps_s,
            tc.tile_pool(name="ps_pv", bufs=3, space="PSUM") as ps_pv,
        ):
            # ---- input DMAs: one SP queue, strict need-order (the cost
            #      model serializes transfers on a shared engine pool) ----
            gt2_sb = const_p.tile([128, 2 * N_TOK], fp8, name="gt2")
            x_sb = [big_p.tile([128, 2 * DIM], fp8, name=f"x{b}")
                    for b in range(B_PER_CORE)]
            xt8_sb = const_p.tile([128, 3 * 2 * NTB], fp8, name="xt8")
            xt_sb = const_p.tile([128, 6 * NTB], bf16, name="xt")
            wq8_sb = const_p.tile([128, 3 * 2 * DIM], fp8, name="wq8")
            wk8_sb = const_p.tile([128, 3 * 2 * DIM], fp8, name="wk8")
            wv_sb = const_p.tile([128, 6 * DIM], bf16, name="wv")
            wp_sb = const_p.tile([128, 6 * DIM], bf16, name="wp")
            bias_sb = const_p.tile([128, DIM], f32, name="bias")

            BPB8 = 3 * 2 * N_TOK   # xt8 cols per batch
            BPB = 6 * N_TOK        # xt cols per batch

            def dma_x(b):
                nc.sync.dma_start(
                    out=x_sb[b],
                    in_=x_d[:, b * 2 * DIM:(b + 1) * 2 * DIM])

            def dma_xt8(b):
                nc.sync.dma_start(
                    out=xt8_sb[:, b * BPB8:(b + 1) * BPB8],
                    in_=xt8_d[:, b * BPB8:(b + 1) * BPB8])

            def dma_xt(b):
                nc.sync.dma_start(
                    out=xt_sb[:, b * BPB:(b + 1) * BPB],
                    in_=xt_d[:, b * BPB:(b + 1) * BPB])

            dma_x(0)
            nc.sync.dma_start(out=gt2_sb, in_=gt_d[:, :])
            dma_x(1)
            dma_xt8(0)
            nc.sync.dma_start(out=wq8_sb, in_=wq8_d[:, :])
            nc.sync.dma_start(out=wk8_sb, in_=wk8_d[:, :])
            dma_xt(0)
            nc.sync.dma_start(out=wv_sb[:, 0:6 * 384],
                              in_=wv_d[:, 0:6 * 384])
            dma_xt8(1)
            dma_xt(1)
            nc.sync.dma_start(out=wv_sb[:, 6 * 384:6 * DIM],
                              in_=wv_d[:, 6 * 384:6 * DIM])
            dma_x(2)
            dma_xt8(2)
            dma_xt(2)
            dma_x(3)
            dma_xt8(3)
            dma_xt(3)
            nc.sync.dma_start(out=wp_sb, in_=wp_d[:, :])
            nc.sync.dma_start(out=bias_sb,
                              in_=bias_d[None, :].broadcast_to([128, DIM]))
            for b in range(4, B_PER_CORE):
                dma_x(b)
                dma_xt8(b)
                dma_xt(b)
            gt_v = gt2_sb.rearrange("p (i c) -> p i c", i=2)
            x_v = [t.rearrange("p (i c) -> p i c", i=2) for t in x_sb]

            ones_sb = const_p.tile([1, 128], bf16, name="ones")
            nc.vector.memset(ones_sb, 1.0)
            biasrow_sb = const_p.tile([1, DIM], bf16, name="biasrow")
            nc.scalar.copy(biasrow_sb, bias_sb[0:1, :])

            # ---- persistent activations ----
            # xg^T fp8 DoubleRow layout [p, i, tok]
            xg8_sb = [big_p.tile([128, 2 * NTB], fp8, name=f"xg8{k}")
                      for k in range(3)]
            xg8_v = [t.rearrange("p (i c) -> p i c", i=2) for t in xg8_sb]
            wq8_v = wq8_sb.rearrange("p (k i c) -> p k i c", k=3, i=2)
            wk8_v = wk8_sb.rearrange("p (k i c) -> p k i c", k=3, i=2)
            # q^T|k^T combined per feature tile: halves of NTBP=1632
            # (1568 tokens + 64 pad so S's stationary reads are always 128
            # wide; pad is zeroed once below)
            NTBP = NTB + 64
            qkT_sb = [big_p.tile([128, 2 * NTBP], bf16, name=f"qkT{k}")
                      for k in range(6)]
            for k in range(6):
                nc.gpsimd.memset(qkT_sb[k][:, 2 * NTBP - 64:2 * NTBP], 0.0)
            # v per batch-tile: 12 head groups of 65 cols (col 64 = ones)
            v1_sb = [
                [big_p.tile([128, 12 * 65], bf16, name=f"v{b}_{ti}")
                 for ti in range(2)]
                for b in range(B_PER_CORE)
            ]
            for b in range(B_PER_CORE):
                for ti in range(2):
                    nc.gpsimd.memset(
                        v1_sb[b][ti].rearrange("p (h c) -> p h c", h=12)[:, :, 64:65],
                        1.0)
            o_sb = [big_p.tile([128, NTB], bf16, name=f"o{k}")
                    for k in range(6)]

            # ---- dense work units (one psum group each) ----
            def a_unit(b, kt2):
                # xg^T premix for one kt2 (two feature tiles): two fp8
                # DoubleRow matmuls (token contraction packed as 128 +
                # 68-with-zero-pad slabs) into one bank, one strided copy
                c0 = b * N_TOK
                ps = ps_dense.tile([128, NT2], f32, tag="psD", name="psD")
                for i in range(2):
                    mt = 2 * kt2 + i
                    nc.tensor.matmul(
                        ps[:, i * N_TOK:(i + 1) * N_TOK],
                        x_v[b][:, :, mt * 128:(mt + 1) * 128],
                        gt_v, start=True, stop=True, perf_mode=DR,
                    )
                with nc.allow_low_precision(reason="fp8 qk path"):
                    nc.scalar.copy(
                        xg8_v[kt2][:, :, c0:c0 + N_TOK],
                        ps.rearrange("p (g c) -> p g c", g=2))

            def qk_unit(b, mt):
                # q^T and k^T for one feature tile: two groups in one bank
                c0 = b * N_TOK
                ps = ps_dense.tile([128, NT2], f32, tag="psD", name="psD")
                for kt2 in range(3):
                    nc.tensor.matmul(
                        ps[:, 0:N_TOK],
                        wq8_v[:, kt2, :, mt * 128:(mt + 1) * 128],
                        xg8_v[kt2][:, :, c0:c0 + N_TOK],
                        start=(kt2 == 0), stop=(kt2 == 2), perf_mode=DR,
                    )
                for kt2 in range(3):
                    nc.tensor.matmul(
                        ps[:, N_TOK:NT2],
                        wk8_v[:, kt2, :, mt * 128:(mt + 1) * 128],
                        xt8_sb.rearrange("p (b k i c) -> p b k i c",
                                         b=B_PER_CORE, k=3, i=2)[:, b, kt2],
                        start=(kt2 == 0), stop=(kt2 == 2), perf_mode=DR,
                    )
                dst = qkT_sb[mt].rearrange("p (g c) -> p g c", g=2)[
                    :, :, c0:c0 + N_TOK]
                nc.vector.tensor_copy(dst, ps.rearrange("p (g c) -> p g c", g=2))

            def v_unit(b, ti, nt):
                # v [tsz, 384] = 6 heads x 64, strided into v1 (65-col groups)
                t0, tsz = TOK_TILES[ti]
                c0 = b * N_TOK
                ps = ps_dense.tile([128, NT2], f32, tag="psD", name="psD")
                for kt in range(6):
                    nc.tensor.matmul(
                        ps[:tsz, :384],
                        xt_sb[:, b * BPB + kt * N_TOK + t0:
                              b * BPB + kt * N_TOK + t0 + tsz],
                        wv_sb[:, nt * 2304 + kt * 384:nt * 2304 + (kt + 1) * 384],
                        start=(kt == 0), stop=(kt == 5),
                    )
                dstv = v1_sb[b][ti].rearrange("p (h c) -> p h c", h=12)
                nc.scalar.copy(
                    dstv[:tsz, 6 * nt:6 * nt + 6, 0:64],
                    ps[:tsz, :384].rearrange("p (h c) -> p h c", h=6))

            def proj_unit(ft, nt, tail=False):
                f0, fsz = PROJ_TILES[ft]
                ps = ps_dense.tile([128, NT2], f32, tag="psD", name="psD")
                for kt in range(6):
                    nc.tensor.matmul(
                        ps[:fsz, :384],
                        o_sb[kt][:, f0:f0 + fsz],
                        wp_sb[:, kt * DIM + nt * 384:kt * DIM + (nt + 1) * 384],
                        start=(kt == 0), stop=(kt == 5 and not tail),
                    )
                y_sb = rs_p.tile([128, 384], f32, tag="y", name="y_sb")
                if tail:
                    # tail variant: bias rides a K=1 matmul (PE is idle by
                    # now; f32 moving data is fine) and Act does the copy,
                    # halving the end-of-kernel DVE serialization
                    nc.tensor.matmul(
                        ps[:fsz, :384], ones_sb[:, :fsz],
                        biasrow_sb[:, nt * 384:(nt + 1) * 384],
                        start=False, stop=True)
                    nc.scalar.copy(y_sb[:fsz], ps[:fsz, :384])
                else:
                    nc.vector.tensor_add(
                        y_sb[:fsz], ps[:fsz, :384],
                        bias_sb[:fsz, nt * 384:(nt + 1) * 384])
                nc.sync.dma_start(
                    out=out_d[f0:f0 + fsz, nt * 384:(nt + 1) * 384],
                    in_=y_sb[:fsz])

            # ---- attention chain steps (per batch b, head pair p) ----
            def attn_stepA(b, p, state):
                c0 = b * N_TOK
                state["pT"] = pT = pt_p.tile([128, 2 * NT2], bf16,
                                             tag="pT", name="pT")
                for hi in range(2):
                    hb = hi * 64
                    s_ps = ps_s.tile([128, NT2], f32, tag="s", name="s_ps")
                    for ti in range(2):
                        t0 = ti * 128
                        nc.tensor.matmul(
                            s_ps[:, ti * N_TOK:(ti + 1) * N_TOK],
                            qkT_sb[p][hb:hb + 64,
                                      NTBP + c0 + t0:NTBP + c0 + t0 + 128],
                            qkT_sb[p][hb:hb + 64, c0:c0 + N_TOK],
                            start=True, stop=True,
                        )
                    nc.scalar.activation(pT[:, hi * NT2:(hi + 1) * NT2], s_ps,
                                         AF.Exp, scale=EXP_SCALE)

            def attn_stepB(b, p, state):
                # PV (+sums via ones col); pair bank [65, 392]
                pT = state["pT"]
                state["pv"] = pv_ps = ps_pv.tile([65, NT2], f32, tag="pv",
                                                 name="pv_ps")
                for hi in range(2):
                    for ti, (t0, tsz) in enumerate(TOK_TILES):
                        nc.tensor.matmul(
                            pv_ps[:, hi * N_TOK:(hi + 1) * N_TOK],
                            v1_sb[b][ti][:tsz,
                                         (2 * p + hi) * 65:(2 * p + hi) * 65 + 65],
                            pT[:tsz, hi * NT2 + ti * N_TOK:hi * NT2 + (ti + 1) * N_TOK],
                            start=(ti == 0), stop=(ti == 1),
                        )

            def attn_stepCr(b, p, state):
                # recip from the psum sums row (emitted right after its PV so
                # it sits ahead of bulk work in the DVE queue)
                pv_ps = state["pv"]
                state["rsb"] = rsb = rs_p.tile([1, NT2], bf16, tag="rsb",
                                               name="rsb")
                with nc.allow_low_precision(reason="softmax recip bf16"):
                    nc.vector.reciprocal(rsb, pv_ps[64:65, :])

            def attn_stepC(b, p, state):
                # K=1 broadcast matmuls -> psum; Act stages the scales to
                # SBUF (TensorTensor may read only one PSUM operand)
                pv_ps = state["pv"]
                rsb = state["rsb"]
                sc_ps = ps_dense.tile([128, NT2], f32, tag="psD", name="sc_ps")
                for hi in range(2):
                    hb = hi * 64
                    nc.tensor.matmul(
                        sc_ps[hb:hb + 64, 0:N_TOK],
                        ones_sb[:, 0:64],
                        rsb[:, hi * N_TOK:(hi + 1) * N_TOK],
                        start=True, stop=True,
                    )
                state["sc"] = sc_sb = rs_p.tile([128, N_TOK], bf16, tag="scb",
                                                name="sc_sb")
                with nc.allow_low_precision(reason="softmax scale bf16"):
                    nc.scalar.copy(sc_sb, sc_ps[:, 0:N_TOK])

            def attn_stepM(b, p, state):
                # DVE normalizes into o_sb one iteration later, keeping the
                # muls out of the reciprocals' way in the DVE queue
                c0 = b * N_TOK
                pv_ps = state["pv"]
                sc_sb = state["sc"]
                for hi in range(2):
                    hb = hi * 64
                    nc.vector.tensor_mul(
                        o_sb[p][hb:hb + 64, c0:c0 + N_TOK],
                        pv_ps[0:64, hi * N_TOK:(hi + 1) * N_TOK],
                        sc_sb[hb:hb + 64, :])

            # ---- dense queue + schedule ----
            dense_q = []

            def push_slot(b_a, b_next, b_proj, cap_slot=None,
                          tail_proj=False):
                units = []
                if b_a is not None and b_a < B_PER_CORE:
                    units.append([(lambda b=b_a, k=k: a_unit(b, k))
                                  for k in range(3)])
                if b_next is not None and b_next < B_PER_CORE:
                    qk = [(lambda b=b_next, mt=mt: qk_unit(b, mt))
                          for mt in range(6)]
                    vv = [(lambda b=b_next, ti=ti, nt=nt: v_unit(b, ti, nt))
                          for nt in range(2) for ti in range(2)]
                    # interleave qk and v to spread psum bank reuse
                    mix = []
                    while qk or vv:
                        if qk:
                            mix.append(qk.pop(0))
                        if vv:
                            mix.append(vv.pop(0))
                        if qk:
                            mix.append(qk.pop(0))
                    units.append(mix)
                pu = []
                for ft, nt in proj_ready(b_proj, cap_slot):
                    tl = False
                    pu.append(lambda ft=ft, nt=nt, tl=tl: proj_unit(ft, nt, tl))
                if pu:
                    units.append(pu)
                proj_units = units.pop() if b_proj is not None else []
                flat = []
                srcs = [u for u in units if u]
                while srcs:
                    for u in srcs:
                        if u:
                            flat.append(u.pop(0))
                    srcs = [u for u in srcs if u]
                # proj interleaved into the back 2/3 of the slot queue
                k = len(flat) // 3
                back = flat[k:]
                merged = []
                while back or proj_units:
                    if back:
                        merged.append(back.pop(0))
                    if proj_units:
                        merged.append(proj_units.pop(0))
                    if back:
                        merged.append(back.pop(0))
                dense_q.extend(flat[:k] + merged)
            push_slot.proj_done = 0

            # proj-tile readiness: tile ft needs all batches covering
            # [128*ft, 128*(ft+1)); batches run in BATCH_ORDER (7 before 6
            # so the last slot still has dense fill and a short tail)
            proj_state = {"done": set(), "emitted": set()}

            def proj_ready(b_done, cap):
                if b_done is not None:
                    proj_state["done"].add(b_done)
                out = []
                for ft in range(len(PROJ_TILES)):
                    if ft in proj_state["emitted"]:
                        continue
                    f0, fsz = PROJ_TILES[ft]
                    b_lo = f0 // N_TOK
                    b_hi = (f0 + fsz - 1) // N_TOK
                    if all(bb in proj_state["done"]
                           for bb in range(b_lo, b_hi + 1)):
                        out.append(ft)
                out = out[:cap] if cap is not None else out
                res = []
                for ft in out:
                    proj_state["emitted"].add(ft)
                    res.extend([(ft, 0), (ft, 1)])
                return res

            def drain_dense(n):
                for _ in range(min(n, len(dense_q))):
                    dense_q.pop(0)()

            # prologue: A(0), A(1), qk(0) first; batch-0's S/exp chains
            # start while the v-path DMAs are still landing
            for k in range(3):
                a_unit(0, k)
            for k in range(3):
                a_unit(1, k)
            for mt in range(6):
                qk_unit(0, mt)
            for mt in range(6):
                qk_unit(1, mt)
            states0 = [dict() for _ in range(6)]
            push_slot(2, None, None)
            for nt in range(2):
                for ti in range(2):
                    dense_q.append(lambda ti=ti, nt=nt: v_unit(0, ti, nt))
                    dense_q.append(lambda ti=ti, nt=nt: v_unit(1, ti, nt))
            fill0 = (len(dense_q) + 5) // 6
            for p in range(6):
                attn_stepA(0, p, states0[p])
                drain_dense(fill0)
            drain_dense(len(dense_q))

            BATCH_ORDER = [0, 1, 2, 3, 4, 5, 6, 7]
            for bi in range(B_PER_CORE):
                b = BATCH_ORDER[bi]
                b_a = BATCH_ORDER[bi + 3] if bi + 3 < B_PER_CORE else None
                b_next = BATCH_ORDER[bi + 2] if bi + 2 < B_PER_CORE else None
                b_prev = BATCH_ORDER[bi - 1] if bi > 0 else None
                cap = 2 if bi <= 2 else None
                push_slot(b_a, b_next, b_prev, cap)
                states = states0 if b == 0 else [dict() for _ in range(6)]
                n_iters = 14
                fill = (len(dense_q) + 2 * n_iters - 1) // (2 * n_iters)
                for p in range(n_iters):
                    if p < 6 and b > 0:
                        attn_stepA(b, p, states[p])
                    if 2 <= p < 8:
                        attn_stepB(b, p - 2, states[p - 2])
                        attn_stepCr(b, p - 2, states[p - 2])
                    if 3 <= p < 9:
                        attn_stepC(b, p - 3, states[p - 3])
                    if 4 <= p < 10:
                        attn_stepM(b, p - 4, states[p - 4])
                    drain_dense(2 * fill)
                drain_dense(len(dense_q))
            push_slot(None, None, BATCH_ORDER[-1], tail_proj=True)
            drain_dense(len(dense_q))

    return nc


_CACHED_NC = None


def kernel(x, w_qkv, w_proj, b_proj, factors):
    global LAST_EXEC_NS, LAST_TRACE, _CACHED_NC
    from concourse.bass_utils import run_bass_kernel_spmd

    factors = np.asarray(factors, dtype=np.float32)
    G = _grid_g(factors)  # raw G

    w_qkv = np.asarray(w_qkv, dtype=np.float32)

    def pack8(w):
        # [768 out, 768 in] -> w^T scaled -> [p, (kt2 i out)] fp8
        wt = np.ascontiguousarray(w.T) * W_SCALE  # [in, out]
        return wt.reshape(3, 2, 128, DIM).transpose(2, 0, 1, 3).reshape(128, -1)

    def pack16(w):
        wt = np.ascontiguousarray(np.asarray(w, dtype=np.float32).T)
        return wt.reshape(6, 128, DIM).transpose(1, 0, 2).reshape(128, -1)

    gtp = np.zeros((256, N_TOK), dtype=np.float32)
    gtp[0:N_TOK] = G_SCALE * G.T
    in_common = {
        "gt": np.ascontiguousarray(
            gtp.reshape(2, 128, N_TOK).transpose(1, 0, 2).reshape(128, -1)
        ).astype(FP8),
        "wq8": np.ascontiguousarray(pack8(w_qkv[0:DIM])).astype(FP8),
        "wk8": np.ascontiguousarray(pack8(w_qkv[DIM:2 * DIM])).astype(FP8),
        "wv": np.ascontiguousarray(
            pack16(w_qkv[2 * DIM:3 * DIM]).reshape(128, 6, 2, 384)
            .transpose(0, 2, 1, 3).reshape(128, -1)).astype(BF16),
        "wp": np.ascontiguousarray(pack16(w_proj)).astype(BF16),
        "bias": np.asarray(b_proj, dtype=np.float32),
    }
    x = np.asarray(x, dtype=np.float32).reshape(BATCH * N_TOK, DIM)
    in_maps = []
    for c in range(N_CORES):
        xc = x[c * NTB:(c + 1) * NTB]  # [1568, 768] f32
        xcp = np.zeros((B_PER_CORE, 256, DIM), dtype=np.float32)
        xcp[:, 0:N_TOK] = xc.reshape(B_PER_CORE, N_TOK, DIM)
        x8p = xcp.reshape(B_PER_CORE, 2, 128, DIM) \
            .transpose(2, 0, 1, 3).reshape(128, -1)
        xtc = np.ascontiguousarray(xc.T)  # [768, 1568]
        # per-batch grouped: [p, (b kt2 i c)] and [p, (b kt c)]
        xt8p = xtc.reshape(3, 2, 128, B_PER_CORE, N_TOK) \
            .transpose(2, 3, 0, 1, 4).reshape(128, -1)
        xtp = xtc.reshape(6, 128, B_PER_CORE, N_TOK) \
            .transpose(1, 2, 0, 3).reshape(128, -1)
        in_maps.append({
            "x": np.ascontiguousarray(x8p).astype(FP8),
            "xt": np.ascontiguousarray(xtp).astype(BF16),
            "xt8": np.ascontiguousarray(xt8p).astype(FP8),
            **in_common,
        })

    if _CACHED_NC is None:
        _CACHED_NC = _build_bass()
    nc = _CACHED_NC

    trace = bool(int(os.environ.get("KERNEL_TRACE", "0")))
    res = run_bass_kernel_spmd(nc, in_maps, core_ids=list(range(N_CORES)),
                               trace=trace)
    LAST_EXEC_NS = res.exec_time_ns
    if res.instructions_and_trace is not None:
        LAST_TRACE = res.instructions_and_trace[1]
    out = np.concatenate([res.results[c]["out"] for c in range(N_CORES)], axis=0)
    return out.reshape(BATCH, N_TOK, DIM).astype(np.float32)


# revision 9
# speedup vs baseline: 1.0480x; 1.0056x over previous
"""Trainium2 Bass kernel for nn_Attention_33157147525297 (v2, pipelined).

Graph-mixed multi-head attention, B=64, N=196 tokens, D=768, H=12 heads.
Data-parallel over batch: 8 batches per NeuronCore x 8 cores.
Measured (TimelineSim cost model): 139664 ns vs 205577 ns baseline (1.47x);
hardware rel err 6.4e-3 fro (budget 2e-2).

Math restructuring (host side):
  reference: attn = softmax(G @ (q k^T * scale)); out = attn @ v
  G mixes the query index only, so the graph-mix collapses into a pre-mix of
  x on the query path: xg = G @ x (raw G; every scalar scale - attention
  1/sqrt(d) and the fp8 weight pre-scales - folds into the exp activation's
  input scale argument, which is free).

Key design points:
  - q/k projections run in fp8 (e4m3) with MatmulPerfMode.DoubleRow: 256-deep
    contraction per matmul at 0.5 cycles/row (2x PE throughput).  Weights are
    scaled x16 into fp8's normal range.  v/proj/S/PV stay bf16: measured on
    the graded inputs, fp8 there busts the 2e-2 budget (v 3.8e-2, proj
    3.3e-2, S 2.2e-2) while fp8-qk lands at 5.1e-3 total because softmax
    normalization damps score-level error.
  - x ships token-major fp8-DoubleRow-packed (premix stationary; the
    premix is the only consumer of x and its output is fp8-quantized anyway,
    so the graph pre-mix itself runs as one DoubleRow matmul per two feature
    tiles) plus x^T in both bf16 (v path) and DoubleRow-packed fp8 (k path):
    host-side layout/dtype prep that kills 48 on-device transpose copies and
    cuts stage A to 2 us of PE.
  - softmax sums come free from the PV matmul via a ones-column appended to
    each head's v slice (pair psum [65, 392]: rows 0:64 = O^T, row 64 =
    sums); DVE reciprocal reads the psum sums row directly; the broadcast
    runs as two K=1 PE matmuls, Act stages the scales psum->SBUF
    (TensorTensor may read only one PSUM operand - walrus rule), DVE
    normalizes into o_sb.
  - S stationary slices are always 128 wide from a 64-col-padded k half
    (qkT halves of 1632), so the S psum is fully written and one exp per
    head covers both token tiles; the junk rows are never read by PV.
  - projection packs tokens across batch boundaries into 13 flat 128-row
    tiles; bias folds into the y staging copy as a DVE tensor_add against a
    broadcast bias tile.
  - engine assignment by latency-criticality (GPSIMD/Pool cannot touch PSUM
    on this machine, so it only gets memsets): Act = exp + v/xg8/scale
    copies, DVE = qk copies + reciprocals + normalize muls + y adds.
  - the whole kernel is software-pipelined per batch: slot b runs attention
    for batch b in 10 interleave iterations (S/exp -> PV -> recip -> bcast ->
    muls at pipeline depths 0/1/1/2/3) with stage A of b+3, qk/v of b+2 and
    projection tiles of b-1 draining in the gaps; input DMAs are host-packed
    to final SBUF layout, per-batch sliced, and issued in strict
    pipeline-need order (the cost model serializes transfers on a shared
    engine pool, so arrival order is everything).

Infra notes: this container's walrus accepts only ONE attached semaphore
wait per instruction - _install_wait_split() hoists excess waits onto
standalone EventSemaphore instructions just before, on the same engine.
Timing feedback comes from the concourse cost-model TimelineSim (NTFF
profiling hooks are unavailable under this axon client).
"""
import os
import sys
import numpy as np
import ml_dtypes

sys.path.insert(0, "/opt/trn_rl_repo")

SIZE, N_TOK, DIM, HEADS, HEAD_DIM, BATCH = 14, 196, 768, 12, 64, 64
N_CORES = 8
B_PER_CORE = BATCH // N_CORES  # 8
NT2 = 2 * N_TOK  # 392
NTB = N_TOK * B_PER_CORE  # 1568
BF16 = ml_dtypes.bfloat16
FP8 = ml_dtypes.float8_e4m3
W_SCALE = 16.0  # q/k weight pre-scale into fp8 normal range
G_SCALE = 8.0   # graph-matrix pre-scale into fp8 normal range
EXP_SCALE = 1.0 / (W_SCALE * W_SCALE * G_SCALE * (HEAD_DIM ** 0.5))

# token-dim partition tiles (196 = 128 + 68)
TOK_TILES = [(0, 128), (128, 68)]
# flat projection tiles over 1568 tokens: 12x128 + 32
PROJ_TILES = [(ft * 128, min(128, NTB - ft * 128)) for ft in range(13)]

LAST_EXEC_NS = None
LAST_TRACE = None


def _grid_g(factors):
    idx = np.arange(SIZE * SIZE).reshape(SIZE, SIZE)
    A = np.zeros((N_TOK, N_TOK), dtype=np.float32)
    for di, dj in [(-1, 0), (1, 0), (0, -1), (0, 1)]:
        for i in range(SIZE):
            for j in range(SIZE):
                ii, jj = i + di, j + dj
                if 0 <= ii < SIZE and 0 <= jj < SIZE:
                    A[idx[i, j], idx[ii, jj]] = 1.0
    NN = A / (A.sum(axis=1, keepdims=True) + 1.0)
    C = np.eye(N_TOK, dtype=np.float32) / 2.0
    return factors[0] * C + factors[1] * NN  # raw G, no attention scale


def _install_wait_split():
    """This container's walrus rejects >1 attached semaphore wait per
    instruction ("Too many sync wait commands").  Hoist excess waits onto
    standalone InstEventSemaphore instructions just before, on the same
    engine - engine queues are in-order, so semantics are identical."""
    import concourse.mybir as mybir
    import concourse.tile as tile
    from concourse.vector_clock import ScopedClock

    TC = tile.TileContext
    if getattr(TC, "_wait_split_patched", False):
        return
    LIMIT = 1

    def _split(tc, inst):
        si = inst.sync_info
        if (si is None or not si.on_wait or len(si.on_wait) <= LIMIT
                or inst.engine == mybir.EngineType.Unassigned):
            return
        waits = list(si.on_wait)
        extra, keep = waits[:-LIMIT], waits[-LIMIT:]
        for i, w in enumerate(extra):
            ev = mybir.InstEventSemaphore(
                name=f"{inst.name}-ws{i}", engine=inst.engine,
                sync_info=mybir.SyncInfo(on_wait=[w], on_update=[]),
            )
            tc._add_instruction(ev)
        inst.sync_info = mybir.SyncInfo(on_wait=keep,
                                        on_update=list(si.on_update))

    orig_commit = TC._commit_instruction

    def patched_commit(self, inst, lazy_reg_writes=True):
        _split(self, inst)
        return orig_commit(self, inst, lazy_reg_writes=lazy_reg_writes)

    TC._commit_instruction = patched_commit

    def patched_drain_and_barrier(self, tick_clock, wait_clock):
        nc = self.nc
        probe = mybir.InstNoOp(
            name=f"drain-probe-{nc.next_id()}", engine=mybir.EngineType.SP)
        wait_clock.add_sem_waits(
            probe, ScopedClock({None: tick_clock.global_clock}))
        pw = probe.sync_info.on_wait if probe.sync_info else []
        for i, w in enumerate(pw):
            ev = mybir.InstEventSemaphore(
                name=f"drainw-{nc.next_id()}-{i}", engine=mybir.EngineType.SP,
                sync_info=mybir.SyncInfo(on_wait=[w], on_update=[]),
            )
            self._add_instruction(ev)
        nc.sync.drain()
        nc.all_engine_barrier()
        assert self.sems is not None
        popped = nc._tile_sem_poison_stack.pop()
        assert popped is self._sem_poison
        nc.clear_and_free_semaphores(list(self.sems.allocated().values()))
        nc.all_engine_barrier()

    TC._drain_and_barrier = patched_drain_and_barrier
    TC._wait_split_patched = True


def _build_bass():
    import concourse.bass as bass
    import concourse.mybir as mybir
    import concourse.tile as tile

    _install_wait_split()

    f32 = mybir.dt.float32
    bf16 = mybir.dt.bfloat16
    fp8 = mybir.dt.float8e4
    AF = mybir.ActivationFunctionType
    DR = mybir.MatmulPerfMode.DoubleRow

    nc = bass.Bass()

    # x token-major, fp8 DoubleRow-packed per batch [p, (b i c)]: the
    # premix is the only consumer and its output is fp8-quantized anyway
    x_d = nc.declare_dram_parameter("x", [128, B_PER_CORE * 2 * DIM], fp8,
                                    isOutput=False)
    gt_d = nc.declare_dram_parameter("gt", [128, 2 * N_TOK], fp8, isOutput=False)
    # everything below is host-packed into final SBUF layout [128, cols];
    # xt8/xt are grouped per batch so arrival order matches pipeline need
    xt8_d = nc.declare_dram_parameter("xt8", [128, 3 * 2 * NTB], fp8,
                                      isOutput=False)
    xt_d = nc.declare_dram_parameter("xt", [128, 6 * NTB], bf16, isOutput=False)
    wq8_d = nc.declare_dram_parameter("wq8", [128, 3 * 2 * DIM], fp8,
                                      isOutput=False)
    wk8_d = nc.declare_dram_parameter("wk8", [128, 3 * 2 * DIM], fp8,
                                      isOutput=False)
    wv_d = nc.declare_dram_parameter("wv", [128, 6 * DIM], bf16, isOutput=False)
    wp_d = nc.declare_dram_parameter("wp", [128, 6 * DIM], bf16, isOutput=False)
    bias_d = nc.declare_dram_parameter("bias", [DIM], f32, isOutput=False)
    out_d = nc.declare_dram_parameter("out", [NTB, DIM], f32, isOutput=True)

    with tile.TileContext(nc) as tc:
        with (
            tc.tile_pool(name="const", bufs=1) as const_p,
            tc.tile_pool(name="big", bufs=1) as big_p,
            tc.tile_pool(name="pt", bufs=8) as pt_p,
            tc.tile_pool(name="rs", bufs=6) as rs_p,
            tc.tile_pool(name="ps_dense", bufs=3, space="PSUM") as ps_dense,
            tc.tile_pool(name="ps_s", bufs=2, space="PSUM") as ps_s,
            tc.tile_pool(name="ps_pv", bufs=3, space="PSUM") as ps_pv,
        ):
            # ---- input DMAs: one SP queue, strict need-order (the cost
            #      model serializes transfers on a shared engine pool) ----
            gt2_sb = const_p.tile([128, 2 * N_TOK], fp8, name="gt2")
            x_sb = [big_p.tile([128, 2 * DIM], fp8, name=f"x{b}")
                    for b in range(B_PER_CORE)]
            xt8_sb = const_p.tile([128, 3 * 2 * NTB], fp8, name="xt8")
            xt_sb = const_p.tile([128, 6 * NTB], bf16, name="xt")
            wq8_sb = const_p.tile([128, 3 * 2 * DIM], fp8, name="wq8")
            wk8_sb = const_p.tile([128, 3 * 2 * DIM], fp8, name="wk8")
            wv_sb = const_p.tile([128, 6 * DIM], bf16, name="wv")
            wp_sb = const_p.tile([128, 6 * DIM], bf16, name="wp")
            bias_sb = const_p.tile([128, DIM], f32, name="bias")

            BPB8 = 3 * 2 * N_TOK   # xt8 cols per batch
            BPB = 6 * N_TOK        # xt cols per batch

            def dma_x(b):
                nc.sync.dma_start(
                    out=x_sb[b],
                    in_=x_d[:, b * 2 * DIM:(b + 1) * 2 * DIM])

            def dma_xt8(b):
                nc.sync.dma_start(
                    out=xt8_sb[:, b * BPB8:(b + 1) * BPB8],
                    in_=xt8_d[:, b * BPB8:(b + 1) * BPB8])

            def dma_xt(b):
                nc.sync.dma_start(
                    out=xt_sb[:, b * BPB:(b + 1) * BPB],
                    in_=xt_d[:, b * BPB:(b + 1) * BPB])

            dma_x(0)
            nc.sync.dma_start(out=gt2_sb, in_=gt_d[:, :])
            dma_x(1)
            dma_xt8(0)
            nc.sync.dma_start(out=wq8_sb, in_=wq8_d[:, :])
            nc.sync.dma_start(out=wk8_sb, in_=wk8_d[:, :])
            dma_xt(0)
            nc.sync.dma_start(out=wv_sb[:, 0:6 * 384],
                              in_=wv_d[:, 0:6 * 384])
            dma_xt8(1)
            dma_xt(1)
            nc.sync.dma_start(out=wv_sb[:, 6 * 384:6 * DIM],
                              in_=wv_d[:, 6 * 384:6 * DIM])
            dma_x(2)
            dma_xt8(2)
            dma_xt(2)
            dma_x(3)
            dma_xt8(3)
            dma_xt(3)
            nc.sync.dma_start(out=wp_sb, in_=wp_d[:, :])
            nc.sync.dma_start(out=bias_sb,
                              in_=bias_d[None, :].broadcast_to([128, DIM]))
            for b in range(4, B_PER_CORE):
                dma_x(b)
                dma_xt8(b)
                dma_xt(b)
            gt_v = gt2_sb.rearrange("p (i c) -> p i c", i=2)
            x_v = [t.rearrange("p (i c) -> p i c", i=2) for t in x_sb]

            ones_sb = const_p.tile([1, 128], bf16, name="ones")
            nc.vector.memset(ones_sb, 1.0)
            biasrow_sb = const_p.tile([1, DIM], bf16, name="biasrow")
            nc.scalar.copy(biasrow_sb, bias_sb[0:1, :])

            # ---- persistent activations ----
            # xg^T fp8 DoubleRow layout [p, i, tok]
            xg8_sb = [big_p.tile([128, 2 * NTB], fp8, name=f"xg8{k}")
                      for k in range(3)]
            xg8_v = [t.rearrange("p (i c) -> p i c", i=2) for t in xg8_sb]
            wq8_v = wq8_sb.rearrange("p (k i c) -> p k i c", k=3, i=2)
            wk8_v = wk8_sb.rearrange("p (k i c) -> p k i c", k=3, i=2)
            # q^T|k^T combined per feature tile: per-batch 256-col slots
            # in each half, so S's always-128-wide stationary reads stay
            # inside the batch's own zero-padded slot (no cross-batch dep)
            SLOT = 256
            HALF = B_PER_CORE * SLOT  # 2048
            qkT_sb = [big_p.tile([128, 2 * HALF], bf16, name=f"qkT{k}")
                      for k in range(6)]
            for k in range(6):
                nc.gpsimd.memset(
                    qkT_sb[k].rearrange("p (g b c) -> p g b c", g=2,
                                        b=B_PER_CORE)[:, :, :, N_TOK:SLOT],
                    0.0)
            # v per batch-tile: 12 head groups of 65 cols (col 64 = ones)
            v1_sb = [
                [big_p.tile([128, 12 * 65], bf16, name=f"v{b}_{ti}")
                 for ti in range(2)]
                for b in range(B_PER_CORE)
            ]
            for b in range(B_PER_CORE):
                for ti in range(2):
                    nc.gpsimd.memset(
                        v1_sb[b][ti].rearrange("p (h c) -> p h c", h=12)[:, :, 64:65],
                        1.0)
            o_sb = [big_p.tile([128, NTB], bf16, name=f"o{k}")
                    for k in range(6)]

            # ---- dense work units (one psum group each) ----
            def a_unit(b, kt2):
                # xg^T premix for one kt2 (two feature tiles): two fp8
                # DoubleRow matmuls (token contraction packed as 128 +
                # 68-with-zero-pad slabs) into one bank, one strided copy
                c0 = b * N_TOK
                ps = ps_dense.tile([128, NT2], f32, tag="psD", name="psD")
                for i in range(2):
                    mt = 2 * kt2 + i
                    nc.tensor.matmul(
                        ps[:, i * N_TOK:(i + 1) * N_TOK],
                        x_v[b][:, :, mt * 128:(mt + 1) * 128],
                        gt_v, start=True, stop=True, perf_mode=DR,
                    )
                with nc.allow_low_precision(reason="fp8 qk path"):
                    nc.scalar.copy(
                        xg8_v[kt2][:, :, c0:c0 + N_TOK],
                        ps.rearrange("p (g c) -> p g c", g=2))

            def qk_unit(b, mt):
                # q^T and k^T for one feature tile: two groups in one bank
                c0 = b * N_TOK
                ps = ps_dense.tile([128, NT2], f32, tag="psD", name="psD")
                for kt2 in range(3):
                    nc.tensor.matmul(
                        ps[:, 0:N_TOK],
                        wq8_v[:, kt2, :, mt * 128:(mt + 1) * 128],
                        xg8_v[kt2][:, :, c0:c0 + N_TOK],
                        start=(kt2 == 0), stop=(kt2 == 2), perf_mode=DR,
                    )
                for kt2 in range(3):
                    nc.tensor.matmul(
                        ps[:, N_TOK:NT2],
                        wk8_v[:, kt2, :, mt * 128:(mt + 1) * 128],
                        xt8_sb.rearrange("p (b k i c) -> p b k i c",
                                         b=B_PER_CORE, k=3, i=2)[:, b, kt2],
                        start=(kt2 == 0), stop=(kt2 == 2), perf_mode=DR,
                    )
                dst = qkT_sb[mt].rearrange("p (g c) -> p g c", g=2)[
                    :, :, b * SLOT:b * SLOT + N_TOK]
                nc.vector.tensor_copy(dst, ps.rearrange("p (g c) -> p g c", g=2))

            def v_unit(b, ti, nt):
                # v [tsz, 384] = 6 heads x 64, strided into v1 (65-col groups)
                t0, tsz = TOK_TILES[ti]
                c0 = b * N_TOK
                ps = ps_dense.tile([128, NT2], f32, tag="psD", name="psD")
                for kt in range(6):
                    nc.tensor.matmul(
                        ps[:tsz, :384],
                        xt_sb[:, b * BPB + kt * N_TOK + t0:
                              b * BPB + kt * N_TOK + t0 + tsz],
                        wv_sb[:, nt * 2304 + kt * 384:nt * 2304 + (kt + 1) * 384],
                        start=(kt == 0), stop=(kt == 5),
                    )
                dstv = v1_sb[b][ti].rearrange("p (h c) -> p h c", h=12)
                nc.scalar.copy(
                    dstv[:tsz, 6 * nt:6 * nt + 6, 0:64],
                    ps[:tsz, :384].rearrange("p (h c) -> p h c", h=6))

            def proj_unit(ft, nt, tail=False):
                f0, fsz = PROJ_TILES[ft]
                ps = ps_dense.tile([128, NT2], f32, tag="psD", name="psD")
                for kt in range(6):
                    nc.tensor.matmul(
                        ps[:fsz, :384],
                        o_sb[kt][:, f0:f0 + fsz],
                        wp_sb[:, kt * DIM + nt * 384:kt * DIM + (nt + 1) * 384],
                        start=(kt == 0), stop=(kt == 5 and not tail),
                    )
                y_sb = rs_p.tile([128, 384], f32, tag="y", name="y_sb")
                if tail:
                    # tail variant: bias rides a K=1 matmul (PE is idle by
                    # now; f32 moving data is fine) and Act does the copy,
                    # halving the end-of-kernel DVE serialization
                    nc.tensor.matmul(
                        ps[:fsz, :384], ones_sb[:, :fsz],
                        biasrow_sb[:, nt * 384:(nt + 1) * 384],
                        start=False, stop=True)
                    nc.scalar.copy(y_sb[:fsz], ps[:fsz, :384])
                else:
                    nc.vector.tensor_add(
                        y_sb[:fsz], ps[:fsz, :384],
                        bias_sb[:fsz, nt * 384:(nt + 1) * 384])
                nc.sync.dma_start(
                    out=out_d[f0:f0 + fsz, nt * 384:(nt + 1) * 384],
                    in_=y_sb[:fsz])

            # ---- attention chain steps (per batch b, head pair p) ----
            def attn_stepA(b, p, state):
                c0 = b * N_TOK
                state["pT"] = pT = pt_p.tile([128, 2 * NT2], bf16,
                                             tag="pT", name="pT")
                for hi in range(2):
                    hb = hi * 64
                    s_ps = ps_s.tile([128, NT2], f32, tag="s", name="s_ps")
                    for ti in range(2):
                        t0 = ti * 128
                        nc.tensor.matmul(
                            s_ps[:, ti * N_TOK:(ti + 1) * N_TOK],
                            qkT_sb[p][hb:hb + 64,
                                      HALF + b * SLOT + t0:
                                      HALF + b * SLOT + t0 + 128],
                            qkT_sb[p][hb:hb + 64, b * SLOT:b * SLOT + N_TOK],
                            start=True, stop=True,
                        )
                    nc.scalar.activation(pT[:, hi * NT2:(hi + 1) * NT2], s_ps,
                                         AF.Exp, scale=EXP_SCALE)

            def attn_stepB(b, p, state):
                # PV (+sums via ones col); pair bank [65, 392]
                pT = state["pT"]
                state["pv"] = pv_ps = ps_pv.tile([65, NT2], f32, tag="pv",
                                                 name="pv_ps")
                for hi in range(2):
                    for ti, (t0, tsz) in enumerate(TOK_TILES):
                        nc.tensor.matmul(
                            pv_ps[:, hi * N_TOK:(hi + 1) * N_TOK],
                            v1_sb[b][ti][:tsz,
                                         (2 * p + hi) * 65:(2 * p + hi) * 65 + 65],
                            pT[:tsz, hi * NT2 + ti * N_TOK:hi * NT2 + (ti + 1) * N_TOK],
                            start=(ti == 0), stop=(ti == 1),
                        )

            def attn_stepCr(b, p, state):
                # recip from the psum sums row (emitted right after its PV so
                # it sits ahead of bulk work in the DVE queue)
                pv_ps = state["pv"]
                state["rsb"] = rsb = rs_p.tile([1, NT2], bf16, tag="rsb",
                                               name="rsb")
                with nc.allow_low_precision(reason="softmax recip bf16"):
                    nc.vector.reciprocal(rsb, pv_ps[64:65, :])

            def attn_stepC(b, p, state):
                # K=1 broadcast matmuls -> psum; Act stages the scales to
                # SBUF (TensorTensor may read only one PSUM operand)
                pv_ps = state["pv"]
                rsb = state["rsb"]
                sc_ps = ps_dense.tile([128, NT2], f32, tag="psD", name="sc_ps")
                for hi in range(2):
                    hb = hi * 64
                    nc.tensor.matmul(
                        sc_ps[hb:hb + 64, 0:N_TOK],
                        ones_sb[:, 0:64],
                        rsb[:, hi * N_TOK:(hi + 1) * N_TOK],
                        start=True, stop=True,
                    )
                state["sc"] = sc_sb = rs_p.tile([128, N_TOK], bf16, tag="scb",
                                                name="sc_sb")
                with nc.allow_low_precision(reason="softmax scale bf16"):
                    nc.scalar.copy(sc_sb, sc_ps[:, 0:N_TOK])

            def attn_stepM(b, p, state):
                # DVE normalizes into o_sb one iteration later, keeping the
                # muls out of the reciprocals' way in the DVE queue
                c0 = b * N_TOK
                pv_ps = state["pv"]
                sc_sb = state["sc"]
                for hi in range(2):
                    hb = hi * 64
                    nc.vector.tensor_mul(
                        o_sb[p][hb:hb + 64, c0:c0 + N_TOK],
                        pv_ps[0:64, hi * N_TOK:(hi + 1) * N_TOK],
                        sc_sb[hb:hb + 64, :])

            # ---- dense queue + schedule ----
            dense_q = []

            def push_slot(b_a, b_next, b_proj, cap_slot=None,
                          tail_proj=False):
                units = []
                if b_a is not None and b_a < B_PER_CORE:
                    units.append([(lambda b=b_a, k=k: a_unit(b, k))
                                  for k in range(3)])
                if b_next is not None and b_next < B_PER_CORE:
                    qk = [(lambda b=b_next, mt=mt: qk_unit(b, mt))
                          for mt in range(6)]
                    vv = [(lambda b=b_next, ti=ti, nt=nt: v_unit(b, ti, nt))
                          for nt in range(2) for ti in range(2)]
                    # interleave qk and v to spread psum bank reuse
                    mix = []
                    while qk or vv:
                        if qk:
                            mix.append(qk.pop(0))
                        if vv:
                            mix.append(vv.pop(0))
                        if qk:
                            mix.append(qk.pop(0))
                    units.append(mix)
                pu = []
                for ft, nt in proj_ready(b_proj, cap_slot):
                    tl = False
                    pu.append(lambda ft=ft, nt=nt, tl=tl: proj_unit(ft, nt, tl))
                if pu:
                    units.append(pu)
                proj_units = units.pop() if b_proj is not None else []
                flat = []
                srcs = [u for u in units if u]
                while srcs:
                    for u in srcs:
                        if u:
                            flat.append(u.pop(0))
                    srcs = [u for u in srcs if u]
                # proj interleaved into the back 2/3 of the slot queue
                k = len(flat) // 3
                back = flat[k:]
                merged = []
                while back or proj_units:
                    if back:
                        merged.append(back.pop(0))
                    if proj_units:
                        merged.append(proj_units.pop(0))
                    if back:
                        merged.append(back.pop(0))
                dense_q.extend(flat[:k] + merged)
            push_slot.proj_done = 0

            # proj-tile readiness: tile ft needs all batches covering
            # [128*ft, 128*(ft+1)); batches run in BATCH_ORDER (7 before 6
            # so the last slot still has dense fill and a short tail)
            proj_state = {"done": set(), "emitted": set()}

            def proj_ready(b_done, cap):
                if b_done is not None:
                    proj_state["done"].add(b_done)
                out = []
                for ft in range(len(PROJ_TILES)):
                    if ft in proj_state["emitted"]:
                        continue
                    f0, fsz = PROJ_TILES[ft]
                    b_lo = f0 // N_TOK
                    b_hi = (f0 + fsz - 1) // N_TOK
                    if all(bb in proj_state["done"]
                           for bb in range(b_lo, b_hi + 1)):
                        out.append(ft)
                out = out[:cap] if cap is not None else out
                res = []
                for ft in out:
                    proj_state["emitted"].add(ft)
                    res.extend([(ft, 0), (ft, 1)])
                return res

            def drain_dense(n):
                for _ in range(min(n, len(dense_q))):
                    dense_q.pop(0)()

            # prologue: A(0), A(1), qk(0) first; batch-0's S/exp chains
            # start while the v-path DMAs are still landing
            for k in range(3):
                a_unit(0, k)
            for k in range(3):
                a_unit(1, k)
            for mt in range(6):
                qk_unit(0, mt)
            for mt in range(6):
                qk_unit(1, mt)
            states0 = [dict() for _ in range(6)]
            push_slot(2, None, None)
            for nt in range(2):
                for ti in range(2):
                    dense_q.append(lambda ti=ti, nt=nt: v_unit(0, ti, nt))
                    dense_q.append(lambda ti=ti, nt=nt: v_unit(1, ti, nt))
            fill0 = (len(dense_q) + 5) // 6
            for p in range(6):
                attn_stepA(0, p, states0[p])
                drain_dense(fill0)
            drain_dense(len(dense_q))

            BATCH_ORDER = [0, 1, 2, 3, 4, 5, 6, 7]
            for bi in range(B_PER_CORE):
                b = BATCH_ORDER[bi]
                b_a = BATCH_ORDER[bi + 3] if bi + 3 < B_PER_CORE else None
                b_next = BATCH_ORDER[bi + 2] if bi + 2 < B_PER_CORE else None
                b_prev = BATCH_ORDER[bi - 1] if bi > 0 else None
                cap = 2 if bi <= 2 else None
                push_slot(b_a, b_next, b_prev, cap)
                states = states0 if b == 0 else [dict() for _ in range(6)]
                n_iters = 14
                fill = (len(dense_q) + 2 * n_iters - 1) // (2 * n_iters)
                for p in range(n_iters):
                    if p < 6 and b > 0:
                        attn_stepA(b, p, states[p])
                    if 2 <= p < 8:
                        attn_stepB(b, p - 2, states[p - 2])
                        attn_stepCr(b, p - 2, states[p - 2])
                    if 3 <= p < 9:
                        attn_stepC(b, p - 3, states[p - 3])
                    if 4 <= p < 10:
                        attn_stepM(b, p - 4, states[p - 4])
                    drain_dense(2 * fill)
                drain_dense(len(dense_q))
            push_slot(None, None, BATCH_ORDER[-1], tail_proj=True)
            drain_dense(len(dense_q))

    return nc


_CACHED_NC = None


def kernel(x, w_qkv, w_proj, b_proj, factors):
    global LAST_EXEC_NS, LAST_TRACE, _CACHED_NC
    from concourse.bass_utils import run_bass_kernel_spmd

    factors = np.asarray(factors, dtype=np.float32)
    G = _grid_g(factors)  # raw G

    w_qkv = np.asarray(w_qkv, dtype=np.float32)

    def pack8(w):
        # [768 out, 768 in] -> w^T scaled -> [p, (kt2 i out)] fp8
        wt = np.ascontiguousarray(w.T) * W_SCALE  # [in, out]
        return wt.reshape(3, 2, 128, DIM).transpose(2, 0, 1, 3).reshape(128, -1)

    def pack16(w):
        wt = np.ascontiguousarray(np.asarray(w, dtype=np.float32).T)
        return wt.reshape(6, 128, DIM).transpose(1, 0, 2).reshape(128, -1)

    gtp = np.zeros((256, N_TOK), dtype=np.float32)
    gtp[0:N_TOK] = G_SCALE * G.T
    in_common = {
        "gt": np.ascontiguousarray(
            gtp.reshape(2, 128, N_TOK).transpose(1, 0, 2).reshape(128, -1)
        ).astype(FP8),
        "wq8": np.ascontiguousarray(pack8(w_qkv[0:DIM])).astype(FP8),
        "wk8": np.ascontiguousarray(pack8(w_qkv[DIM:2 * DIM])).astype(FP8),
        "wv": np.ascontiguousarray(
            pack16(w_qkv[2 * DIM:3 * DIM]).reshape(128, 6, 2, 384)
            .transpose(0, 2, 1, 3).reshape(128, -1)).astype(BF16),
        "wp": np.ascontiguousarray(pack16(w_proj)).astype(BF16),
        "bias": np.asarray(b_proj, dtype=np.float32),
    }
    x = np.asarray(x, dtype=np.float32).reshape(BATCH * N_TOK, DIM)
    in_maps = []
    for c in range(N_CORES):
        xc = x[c * NTB:(c + 1) * NTB]  # [1568, 768] f32
        xcp = np.zeros((B_PER_CORE, 256, DIM), dtype=np.float32)
        xcp[:, 0:N_TOK] = xc.reshape(B_PER_CORE, N_TOK, DIM)
        x8p = xcp.reshape(B_PER_CORE, 2, 128, DIM) \
            .transpose(2, 0, 1, 3).reshape(128, -1)
        xtc = np.ascontiguousarray(xc.T)  # [768, 1568]
        # per-batch grouped: [p, (b kt2 i c)] and [p, (b kt c)]
        xt8p = xtc.reshape(3, 2, 128, B_PER_CORE, N_TOK) \
            .transpose(2, 3, 0, 1, 4).reshape(128, -1)
        xtp = xtc.reshape(6, 128, B_PER_CORE, N_TOK) \
            .transpose(1, 2, 0, 3).reshape(128, -1)
        in_maps.append({
            "x": np.ascontiguousarray(x8p).astype(FP8),
            "xt": np.ascontiguousarray(xtp).astype(BF16),
            "xt8": np.ascontiguousarray(xt8p).astype(FP8),
            **in_common,
        })

    if _CACHED_NC is None:
        _CACHED_NC = _build_bass()
    nc = _CACHED_NC

    trace = bool(int(os.environ.get("KERNEL_TRACE", "0")))
    res = run_bass_kernel_spmd(nc, in_maps, core_ids=list(range(N_CORES)),
                               trace=trace)
    LAST_EXEC_NS = res.exec_time_ns
    if res.instructions_and_trace is not None:
        LAST_TRACE = res.instructions_and_trace[1]
    out = np.concatenate([res.results[c]["out"] for c in range(N_CORES)], axis=0)
    return out.reshape(BATCH, N_TOK, DIM).astype(np.float32)


# revision 12
# speedup vs baseline: 1.0487x; 1.0007x over previous
"""Trainium2 Bass kernel for nn_Attention_33157147525297 (v2, pipelined).

Graph-mixed multi-head attention, B=64, N=196 tokens, D=768, H=12 heads.
Data-parallel over batch: 8 batches per NeuronCore x 8 cores.
Measured (TimelineSim cost model): 139664 ns vs 205577 ns baseline (1.47x);
hardware rel err 6.4e-3 fro (budget 2e-2).

Math restructuring (host side):
  reference: attn = softmax(G @ (q k^T * scale)); out = attn @ v
  G mixes the query index only, so the graph-mix collapses into a pre-mix of
  x on the query path: xg = G @ x (raw G; every scalar scale - attention
  1/sqrt(d) and the fp8 weight pre-scales - folds into the exp activation's
  input scale argument, which is free).

Key design points:
  - q/k projections run in fp8 (e4m3) with MatmulPerfMode.DoubleRow: 256-deep
    contraction per matmul at 0.5 cycles/row (2x PE throughput).  Weights are
    scaled x16 into fp8's normal range.  v/proj/S/PV stay bf16: measured on
    the graded inputs, fp8 there busts the 2e-2 budget (v 3.8e-2, proj
    3.3e-2, S 2.2e-2) while fp8-qk lands at 5.1e-3 total because softmax
    normalization damps score-level error.
  - x ships token-major fp8-DoubleRow-packed (premix stationary; the
    premix is the only consumer of x and its output is fp8-quantized anyway,
    so the graph pre-mix itself runs as one DoubleRow matmul per two feature
    tiles) plus x^T in both bf16 (v path) and DoubleRow-packed fp8 (k path):
    host-side layout/dtype prep that kills 48 on-device transpose copies and
    cuts stage A to 2 us of PE.
  - softmax sums come free from the PV matmul via a ones-column appended to
    each head's v slice (pair psum [65, 392]: rows 0:64 = O^T, row 64 =
    sums); DVE reciprocal reads the psum sums row directly; the broadcast
    runs as two K=1 PE matmuls, Act stages the scales psum->SBUF
    (TensorTensor may read only one PSUM operand - walrus rule), DVE
    normalizes into o_sb.
  - S stationary slices are always 128 wide from a 64-col-padded k half
    (qkT halves of 1632), so the S psum is fully written and one exp per
    head covers both token tiles; the junk rows are never read by PV.
  - projection packs tokens across batch boundaries into 13 flat 128-row
    tiles; bias folds into the y staging copy as a DVE tensor_add against a
    broadcast bias tile.
  - engine assignment by latency-criticality (GPSIMD/Pool cannot touch PSUM
    on this machine, so it only gets memsets): Act = exp + v/xg8/scale
    copies, DVE = qk copies + reciprocals + normalize muls + y adds.
  - the whole kernel is software-pipelined per batch: slot b runs attention
    for batch b in 10 interleave iterations (S/exp -> PV -> recip -> bcast ->
    muls at pipeline depths 0/1/1/2/3) with stage A of b+3, qk/v of b+2 and
    projection tiles of b-1 draining in the gaps; input DMAs are host-packed
    to final SBUF layout, per-batch sliced, and issued in strict
    pipeline-need order (the cost model serializes transfers on a shared
    engine pool, so arrival order is everything).

Infra notes: this container's walrus accepts only ONE attached semaphore
wait per instruction - _install_wait_split() hoists excess waits onto
standalone EventSemaphore instructions just before, on the same engine.
Timing feedback comes from the concourse cost-model TimelineSim (NTFF
profiling hooks are unavailable under this axon client).
"""
import os
import sys
import numpy as np
import ml_dtypes

sys.path.insert(0, "/opt/trn_rl_repo")

SIZE, N_TOK, DIM, HEADS, HEAD_DIM, BATCH = 14, 196, 768, 12, 64, 64
N_CORES = 8
B_PER_CORE = BATCH // N_CORES  # 8
NT2 = 2 * N_TOK  # 392
NTB = N_TOK * B_PER_CORE  # 1568
BF16 = ml_dtypes.bfloat16
FP8 = ml_dtypes.float8_e4m3
W_SCALE = 16.0  # q/k weight pre-scale into fp8 normal range
G_SCALE = 8.0   # graph-matrix pre-scale into fp8 normal range
EXP_SCALE = 1.0 / (W_SCALE * W_SCALE * G_SCALE * (HEAD_DIM ** 0.5))

# token-dim partition tiles (196 = 128 + 68)
TOK_TILES = [(0, 128), (128, 68)]
# flat projection tiles over 1568 tokens: 12x128 + 32
PROJ_TILES = [(ft * 128, min(128, NTB - ft * 128)) for ft in range(13)]

LAST_EXEC_NS = None
LAST_TRACE = None


def _grid_g(factors):
    idx = np.arange(SIZE * SIZE).reshape(SIZE, SIZE)
    A = np.zeros((N_TOK, N_TOK), dtype=np.float32)
    for di, dj in [(-1, 0), (1, 0), (0, -1), (0, 1)]:
        for i in range(SIZE):
            for j in range(SIZE):
                ii, jj = i + di, j + dj
                if 0 <= ii < SIZE and 0 <= jj < SIZE:
                    A[idx[i, j], idx[ii, jj]] = 1.0
    NN = A / (A.sum(axis=1, keepdims=True) + 1.0)
    C = np.eye(N_TOK, dtype=np.float32) / 2.0
    return factors[0] * C + factors[1] * NN  # raw G, no attention scale


def _install_wait_split():
    """This container's walrus rejects >1 attached semaphore wait per
    instruction ("Too many sync wait commands").  Hoist excess waits onto
    standalone InstEventSemaphore instructions just before, on the same
    engine - engine queues are in-order, so semantics are identical."""
    import concourse.mybir as mybir
    import concourse.tile as tile
    from concourse.vector_clock import ScopedClock

    TC = tile.TileContext
    if getattr(TC, "_wait_split_patched", False):
        return
    LIMIT = 1

    def _split(tc, inst):
        si = inst.sync_info
        if (si is None or not si.on_wait or len(si.on_wait) <= LIMIT
                or inst.engine == mybir.EngineType.Unassigned):
            return
        waits = list(si.on_wait)
        extra, keep = waits[:-LIMIT], waits[-LIMIT:]
        for i, w in enumerate(extra):
            ev = mybir.InstEventSemaphore(
                name=f"{inst.name}-ws{i}", engine=inst.engine,
                sync_info=mybir.SyncInfo(on_wait=[w], on_update=[]),
            )
            tc._add_instruction(ev)
        inst.sync_info = mybir.SyncInfo(on_wait=keep,
                                        on_update=list(si.on_update))

    orig_commit = TC._commit_instruction

    def patched_commit(self, inst, lazy_reg_writes=True):
        _split(self, inst)
        return orig_commit(self, inst, lazy_reg_writes=lazy_reg_writes)

    TC._commit_instruction = patched_commit

    def patched_drain_and_barrier(self, tick_clock, wait_clock):
        nc = self.nc
        probe = mybir.InstNoOp(
            name=f"drain-probe-{nc.next_id()}", engine=mybir.EngineType.SP)
        wait_clock.add_sem_waits(
            probe, ScopedClock({None: tick_clock.global_clock}))
        pw = probe.sync_info.on_wait if probe.sync_info else []
        for i, w in enumerate(pw):
            ev = mybir.InstEventSemaphore(
                name=f"drainw-{nc.next_id()}-{i}", engine=mybir.EngineType.SP,
                sync_info=mybir.SyncInfo(on_wait=[w], on_update=[]),
            )
            self._add_instruction(ev)
        nc.sync.drain()
        nc.all_engine_barrier()
        assert self.sems is not None
        popped = nc._tile_sem_poison_stack.pop()
        assert popped is self._sem_poison
        nc.clear_and_free_semaphores(list(self.sems.allocated().values()))
        nc.all_engine_barrier()

    TC._drain_and_barrier = patched_drain_and_barrier
    TC._wait_split_patched = True


def _build_bass():
    import concourse.bass as bass
    import concourse.mybir as mybir
    import concourse.tile as tile

    _install_wait_split()

    f32 = mybir.dt.float32
    bf16 = mybir.dt.bfloat16
    fp8 = mybir.dt.float8e4
    AF = mybir.ActivationFunctionType
    DR = mybir.MatmulPerfMode.DoubleRow

    nc = bass.Bass()

    # x token-major, fp8 DoubleRow-packed per batch [p, (b i c)]: the
    # premix is the only consumer and its output is fp8-quantized anyway
    x_d = nc.declare_dram_parameter("x", [128, B_PER_CORE * 2 * DIM], fp8,
                                    isOutput=False)
    gt_d = nc.declare_dram_parameter("gt", [128, 2 * N_TOK], fp8, isOutput=False)
    # everything below is host-packed into final SBUF layout [128, cols];
    # xt8/xt are grouped per batch so arrival order matches pipeline need
    xt8_d = nc.declare_dram_parameter("xt8", [128, 3 * 2 * NTB], fp8,
                                      isOutput=False)
    xt_d = nc.declare_dram_parameter("xt", [128, 6 * NTB], bf16, isOutput=False)
    wq8_d = nc.declare_dram_parameter("wq8", [128, 3 * 2 * DIM], fp8,
                                      isOutput=False)
    wk8_d = nc.declare_dram_parameter("wk8", [128, 3 * 2 * DIM], fp8,
                                      isOutput=False)
    wv_d = nc.declare_dram_parameter("wv", [128, 6 * DIM], bf16, isOutput=False)
    wp_d = nc.declare_dram_parameter("wp", [128, 6 * DIM], bf16, isOutput=False)
    bias_d = nc.declare_dram_parameter("bias", [DIM], f32, isOutput=False)
    out_d = nc.declare_dram_parameter("out", [NTB, DIM], f32, isOutput=True)

    with tile.TileContext(nc) as tc:
        with (
            tc.tile_pool(name="const", bufs=1) as const_p,
            tc.tile_pool(name="big", bufs=1) as big_p,
            tc.tile_pool(name="pt", bufs=8) as pt_p,
            tc.tile_pool(name="rs", bufs=7) as rs_p,
            tc.tile_pool(name="ps_dense", bufs=3, space="PSUM") as ps_dense,
            tc.tile_pool(name="ps_s", bufs=2, space="PSUM") as ps_s,
            tc.tile_pool(name="ps_pv", bufs=3, space="PSUM") as ps_pv,
        ):
            # ---- input DMAs: one SP queue, strict need-order (the cost
            #      model serializes transfers on a shared engine pool) ----
            gt2_sb = const_p.tile([128, 2 * N_TOK], fp8, name="gt2")
            x_sb = [big_p.tile([128, 2 * DIM], fp8, name=f"x{b}")
                    for b in range(B_PER_CORE)]
            xt8_sb = const_p.tile([128, 3 * 2 * NTB], fp8, name="xt8")
            xt_sb = const_p.tile([128, 6 * NTB], bf16, name="xt")
            wq8_sb = const_p.tile([128, 3 * 2 * DIM], fp8, name="wq8")
            wk8_sb = const_p.tile([128, 3 * 2 * DIM], fp8, name="wk8")
            wv_sb = const_p.tile([128, 6 * DIM], bf16, name="wv")
            wp_sb = const_p.tile([128, 6 * DIM], bf16, name="wp")
            bias_sb = const_p.tile([128, DIM], f32, name="bias")

            BPB8 = 3 * 2 * N_TOK   # xt8 cols per batch
            BPB = 6 * N_TOK        # xt cols per batch

            def dma_x(b):
                nc.sync.dma_start(
                    out=x_sb[b],
                    in_=x_d[:, b * 2 * DIM:(b + 1) * 2 * DIM])

            def dma_xt8(b):
                nc.sync.dma_start(
                    out=xt8_sb[:, b * BPB8:(b + 1) * BPB8],
                    in_=xt8_d[:, b * BPB8:(b + 1) * BPB8])

            def dma_xt(b):
                nc.sync.dma_start(
                    out=xt_sb[:, b * BPB:(b + 1) * BPB],
                    in_=xt_d[:, b * BPB:(b + 1) * BPB])

            dma_x(0)
            nc.sync.dma_start(out=gt2_sb, in_=gt_d[:, :])
            dma_x(1)
            dma_xt8(0)
            nc.sync.dma_start(out=wq8_sb, in_=wq8_d[:, :])
            nc.sync.dma_start(out=wk8_sb, in_=wk8_d[:, :])
            dma_xt(0)
            nc.sync.dma_start(out=wv_sb[:, 0:6 * 384],
                              in_=wv_d[:, 0:6 * 384])
            dma_xt8(1)
            dma_xt(1)
            nc.sync.dma_start(out=wv_sb[:, 6 * 384:6 * DIM],
                              in_=wv_d[:, 6 * 384:6 * DIM])
            dma_x(2)
            dma_xt8(2)
            dma_xt(2)
            dma_x(3)
            dma_xt8(3)
            dma_xt(3)
            nc.sync.dma_start(out=wp_sb, in_=wp_d[:, :])
            nc.sync.dma_start(out=bias_sb,
                              in_=bias_d[None, :].broadcast_to([128, DIM]))
            for b in range(4, B_PER_CORE):
                dma_x(b)
                dma_xt8(b)
                dma_xt(b)
            gt_v = gt2_sb.rearrange("p (i c) -> p i c", i=2)
            x_v = [t.rearrange("p (i c) -> p i c", i=2) for t in x_sb]

            ones_sb = const_p.tile([1, 128], bf16, name="ones")
            nc.vector.memset(ones_sb, 1.0)
            biasrow_sb = const_p.tile([1, DIM], bf16, name="biasrow")
            nc.scalar.copy(biasrow_sb, bias_sb[0:1, :])

            # ---- persistent activations ----
            # xg^T fp8 DoubleRow layout [p, i, tok]
            xg8_sb = [big_p.tile([128, 2 * NTB], fp8, name=f"xg8{k}")
                      for k in range(3)]
            xg8_v = [t.rearrange("p (i c) -> p i c", i=2) for t in xg8_sb]
            wq8_v = wq8_sb.rearrange("p (k i c) -> p k i c", k=3, i=2)
            wk8_v = wk8_sb.rearrange("p (k i c) -> p k i c", k=3, i=2)
            # q^T|k^T combined per feature tile: per-batch 256-col slots
            # in each half, so S's always-128-wide stationary reads stay
            # inside the batch's own zero-padded slot (no cross-batch dep)
            SLOT = 256
            HALF = B_PER_CORE * SLOT  # 2048
            qkT_sb = [big_p.tile([128, 2 * HALF], bf16, name=f"qkT{k}")
                      for k in range(6)]
            for k in range(6):
                nc.gpsimd.memset(
                    qkT_sb[k].rearrange("p (g b c) -> p g b c", g=2,
                                        b=B_PER_CORE)[:, :, :, N_TOK:SLOT],
                    0.0)
            # v per batch-tile: 12 head groups of 65 cols (col 64 = ones)
            v1_sb = [
                [big_p.tile([128, 12 * 65], bf16, name=f"v{b}_{ti}")
                 for ti in range(2)]
                for b in range(B_PER_CORE)
            ]
            for b in range(B_PER_CORE):
                for ti in range(2):
                    nc.gpsimd.memset(
                        v1_sb[b][ti].rearrange("p (h c) -> p h c", h=12)[:, :, 64:65],
                        1.0)
            o_sb = [big_p.tile([128, NTB], bf16, name=f"o{k}")
                    for k in range(6)]

            # ---- dense work units (one psum group each) ----
            def a_unit(b, kt2):
                # xg^T premix for one kt2 (two feature tiles): two fp8
                # DoubleRow matmuls (token contraction packed as 128 +
                # 68-with-zero-pad slabs) into one bank, one strided copy
                c0 = b * N_TOK
                ps = ps_dense.tile([128, NT2], f32, tag="psD", name="psD")
                for i in range(2):
                    mt = 2 * kt2 + i
                    nc.tensor.matmul(
                        ps[:, i * N_TOK:(i + 1) * N_TOK],
                        x_v[b][:, :, mt * 128:(mt + 1) * 128],
                        gt_v, start=True, stop=True, perf_mode=DR,
                    )
                with nc.allow_low_precision(reason="fp8 qk path"):
                    nc.scalar.copy(
                        xg8_v[kt2][:, :, c0:c0 + N_TOK],
                        ps.rearrange("p (g c) -> p g c", g=2))

            def qk_unit(b, mt):
                # q^T and k^T for one feature tile: two groups in one bank
                c0 = b * N_TOK
                ps = ps_dense.tile([128, NT2], f32, tag="psD", name="psD")
                for kt2 in range(3):
                    nc.tensor.matmul(
                        ps[:, 0:N_TOK],
                        wq8_v[:, kt2, :, mt * 128:(mt + 1) * 128],
                        xg8_v[kt2][:, :, c0:c0 + N_TOK],
                        start=(kt2 == 0), stop=(kt2 == 2), perf_mode=DR,
                    )
                for kt2 in range(3):
                    nc.tensor.matmul(
                        ps[:, N_TOK:NT2],
                        wk8_v[:, kt2, :, mt * 128:(mt + 1) * 128],
                        xt8_sb.rearrange("p (b k i c) -> p b k i c",
                                         b=B_PER_CORE, k=3, i=2)[:, b, kt2],
                        start=(kt2 == 0), stop=(kt2 == 2), perf_mode=DR,
                    )
                dst = qkT_sb[mt].rearrange("p (g c) -> p g c", g=2)[
                    :, :, b * SLOT:b * SLOT + N_TOK]
                nc.vector.tensor_copy(dst, ps.rearrange("p (g c) -> p g c", g=2))

            def v_unit(b, ti, nt):
                # v [tsz, 384] = 6 heads x 64, strided into v1 (65-col groups)
                t0, tsz = TOK_TILES[ti]
                c0 = b * N_TOK
                ps = ps_dense.tile([128, NT2], f32, tag="psD", name="psD")
                for kt in range(6):
                    nc.tensor.matmul(
                        ps[:tsz, :384],
                        xt_sb[:, b * BPB + kt * N_TOK + t0:
                              b * BPB + kt * N_TOK + t0 + tsz],
                        wv_sb[:, nt * 2304 + kt * 384:nt * 2304 + (kt + 1) * 384],
                        start=(kt == 0), stop=(kt == 5),
                    )
                dstv = v1_sb[b][ti].rearrange("p (h c) -> p h c", h=12)
                nc.scalar.copy(
                    dstv[:tsz, 6 * nt:6 * nt + 6, 0:64],
                    ps[:tsz, :384].rearrange("p (h c) -> p h c", h=6))

            def proj_unit(ft, nt, tail=False):
                f0, fsz = PROJ_TILES[ft]
                ps = ps_dense.tile([128, NT2], f32, tag="psD", name="psD")
                for kt in range(6):
                    nc.tensor.matmul(
                        ps[:fsz, :384],
                        o_sb[kt][:, f0:f0 + fsz],
                        wp_sb[:, kt * DIM + nt * 384:kt * DIM + (nt + 1) * 384],
                        start=(kt == 0), stop=(kt == 5 and not tail),
                    )
                y_sb = rs_p.tile([128, 384], f32, tag="y", name="y_sb")
                if tail:
                    # tail variant: bias rides a K=1 matmul (PE is idle by
                    # now; f32 moving data is fine) and Act does the copy,
                    # halving the end-of-kernel DVE serialization
                    nc.tensor.matmul(
                        ps[:fsz, :384], ones_sb[:, :fsz],
                        biasrow_sb[:, nt * 384:(nt + 1) * 384],
                        start=False, stop=True)
                    nc.scalar.copy(y_sb[:fsz], ps[:fsz, :384])
                else:
                    nc.vector.tensor_add(
                        y_sb[:fsz], ps[:fsz, :384],
                        bias_sb[:fsz, nt * 384:(nt + 1) * 384])
                nc.sync.dma_start(
                    out=out_d[f0:f0 + fsz, nt * 384:(nt + 1) * 384],
                    in_=y_sb[:fsz])

            # ---- attention chain steps (per batch b, head pair p) ----
            def attn_stepA(b, p, state):
                c0 = b * N_TOK
                state["pT"] = pT = pt_p.tile([128, 2 * NT2], bf16,
                                             tag="pT", name="pT")
                for hi in range(2):
                    hb = hi * 64
                    s_ps = ps_s.tile([128, NT2], f32, tag="s", name="s_ps")
                    for ti in range(2):
                        t0 = ti * 128
                        nc.tensor.matmul(
                            s_ps[:, ti * N_TOK:(ti + 1) * N_TOK],
                            qkT_sb[p][hb:hb + 64,
                                      HALF + b * SLOT + t0:
                                      HALF + b * SLOT + t0 + 128],
                            qkT_sb[p][hb:hb + 64, b * SLOT:b * SLOT + N_TOK],
                            start=True, stop=True,
                        )
                    nc.scalar.activation(pT[:, hi * NT2:(hi + 1) * NT2], s_ps,
                                         AF.Exp, scale=EXP_SCALE)

            def attn_stepB(b, p, state):
                # PV (+sums via ones col); pair bank [65, 392]
                pT = state["pT"]
                state["pv"] = pv_ps = ps_pv.tile([65, NT2], f32, tag="pv",
                                                 name="pv_ps")
                for hi in range(2):
                    for ti, (t0, tsz) in enumerate(TOK_TILES):
                        nc.tensor.matmul(
                            pv_ps[:, hi * N_TOK:(hi + 1) * N_TOK],
                            v1_sb[b][ti][:tsz,
                                         (2 * p + hi) * 65:(2 * p + hi) * 65 + 65],
                            pT[:tsz, hi * NT2 + ti * N_TOK:hi * NT2 + (ti + 1) * N_TOK],
                            start=(ti == 0), stop=(ti == 1),
                        )

            def attn_stepCr(b, p, state):
                # recip from the psum sums row (emitted right after its PV so
                # it sits ahead of bulk work in the DVE queue)
                pv_ps = state["pv"]
                state["rsb"] = rsb = rs_p.tile([1, NT2], bf16, tag="rsb",
                                               name="rsb")
                with nc.allow_low_precision(reason="softmax recip bf16"):
                    nc.vector.reciprocal(rsb, pv_ps[64:65, :])

            def attn_stepC(b, p, state):
                # K=1 broadcast matmuls -> psum; Act stages the scales to
                # SBUF (TensorTensor may read only one PSUM operand)
                pv_ps = state["pv"]
                rsb = state["rsb"]
                sc_ps = ps_dense.tile([128, NT2], f32, tag="psD", name="sc_ps")
                for hi in range(2):
                    hb = hi * 64
                    nc.tensor.matmul(
                        sc_ps[hb:hb + 64, 0:N_TOK],
                        ones_sb[:, 0:64],
                        rsb[:, hi * N_TOK:(hi + 1) * N_TOK],
                        start=True, stop=True,
                    )
                state["sc"] = sc_sb = rs_p.tile([128, N_TOK], bf16, tag="scb",
                                                name="sc_sb")
                with nc.allow_low_precision(reason="softmax scale bf16"):
                    nc.scalar.copy(sc_sb, sc_ps[:, 0:N_TOK])

            def attn_stepM(b, p, state):
                # DVE normalizes into o_sb one iteration later, keeping the
                # muls out of the reciprocals' way in the DVE queue
                c0 = b * N_TOK
                pv_ps = state["pv"]
                sc_sb = state["sc"]
                for hi in range(2):
                    hb = hi * 64
                    nc.vector.tensor_mul(
                        o_sb[p][hb:hb + 64, c0:c0 + N_TOK],
                        pv_ps[0:64, hi * N_TOK:(hi + 1) * N_TOK],
                        sc_sb[hb:hb + 64, :])

            # ---- dense queue + schedule ----
            dense_q = []

            def push_slot(b_a, b_next, b_proj, cap_slot=None,
                          tail_proj=False):
                units = []
                if b_a is not None and b_a < B_PER_CORE:
                    units.append([(lambda b=b_a, k=k: a_unit(b, k))
                                  for k in range(3)])
                if b_next is not None and b_next < B_PER_CORE:
                    qk = [(lambda b=b_next, mt=mt: qk_unit(b, mt))
                          for mt in range(6)]
                    vv = [(lambda b=b_next, ti=ti, nt=nt: v_unit(b, ti, nt))
                          for nt in range(2) for ti in range(2)]
                    # interleave qk and v to spread psum bank reuse
                    mix = []
                    while qk or vv:
                        if qk:
                            mix.append(qk.pop(0))
                        if vv:
                            mix.append(vv.pop(0))
                        if qk:
                            mix.append(qk.pop(0))
                    units.append(mix)
                pu = []
                for ft, nt in proj_ready(b_proj, cap_slot):
                    tl = False
                    pu.append(lambda ft=ft, nt=nt, tl=tl: proj_unit(ft, nt, tl))
                if pu:
                    units.append(pu)
                proj_units = units.pop() if b_proj is not None else []
                flat = []
                srcs = [u for u in units if u]
                while srcs:
                    for u in srcs:
                        if u:
                            flat.append(u.pop(0))
                    srcs = [u for u in srcs if u]
                # proj interleaved into the back 2/3 of the slot queue
                k = len(flat) // 3
                back = flat[k:]
                merged = []
                while back or proj_units:
                    if back:
                        merged.append(back.pop(0))
                    if proj_units:
                        merged.append(proj_units.pop(0))
                    if back:
                        merged.append(back.pop(0))
                dense_q.extend(flat[:k] + merged)
            push_slot.proj_done = 0

            # proj-tile readiness: tile ft needs all batches covering
            # [128*ft, 128*(ft+1)); batches run in BATCH_ORDER (7 before 6
            # so the last slot still has dense fill and a short tail)
            proj_state = {"done": set(), "emitted": set()}

            def proj_ready(b_done, cap):
                if b_done is not None:
                    proj_state["done"].add(b_done)
                out = []
                for ft in range(len(PROJ_TILES)):
                    if ft in proj_state["emitted"]:
                        continue
                    f0, fsz = PROJ_TILES[ft]
                    b_lo = f0 // N_TOK
                    b_hi = (f0 + fsz - 1) // N_TOK
                    if all(bb in proj_state["done"]
                           for bb in range(b_lo, b_hi + 1)):
                        out.append(ft)
                out = out[:cap] if cap is not None else out
                res = []
                for ft in out:
                    proj_state["emitted"].add(ft)
                    res.extend([(ft, 0), (ft, 1)])
                return res

            def drain_dense(n):
                for _ in range(min(n, len(dense_q))):
                    dense_q.pop(0)()

            # prologue: A(0), A(1), qk(0) first; batch-0's S/exp chains
            # start while the v-path DMAs are still landing
            for k in range(3):
                a_unit(0, k)
            for k in range(3):
                a_unit(1, k)
            for mt in range(6):
                qk_unit(0, mt)
            for mt in range(6):
                qk_unit(1, mt)
            states0 = [dict() for _ in range(6)]
            push_slot(2, None, None)
            for nt in range(2):
                for ti in range(2):
                    dense_q.append(lambda ti=ti, nt=nt: v_unit(0, ti, nt))
                    dense_q.append(lambda ti=ti, nt=nt: v_unit(1, ti, nt))
            fill0 = (len(dense_q) + 5) // 6
            for p in range(6):
                attn_stepA(0, p, states0[p])
                drain_dense(fill0)
            drain_dense(len(dense_q))

            BATCH_ORDER = [0, 1, 2, 3, 4, 5, 6, 7]
            for bi in range(B_PER_CORE):
                b = BATCH_ORDER[bi]
                b_a = BATCH_ORDER[bi + 3] if bi + 3 < B_PER_CORE else None
                b_next = BATCH_ORDER[bi + 2] if bi + 2 < B_PER_CORE else None
                b_prev = BATCH_ORDER[bi - 1] if bi > 0 else None
                cap = 2 if bi <= 2 else None
                push_slot(b_a, b_next, b_prev, cap)
                states = states0 if b == 0 else [dict() for _ in range(6)]
                n_iters = 14
                fill = (len(dense_q) + 2 * n_iters - 1) // (2 * n_iters)
                for p in range(n_iters):
                    if p < 6 and b > 0:
                        attn_stepA(b, p, states[p])
                    if 1 <= p < 7:
                        attn_stepB(b, p - 1, states[p - 1])
                        attn_stepCr(b, p - 1, states[p - 1])
                    if 2 <= p < 8:
                        attn_stepC(b, p - 2, states[p - 2])
                    if 3 <= p < 9:
                        attn_stepM(b, p - 3, states[p - 3])
                    drain_dense(2 * fill)
                drain_dense(len(dense_q))
            push_slot(None, None, BATCH_ORDER[-1], tail_proj=True)
            drain_dense(len(dense_q))

    return nc


_CACHED_NC = None


def kernel(x, w_qkv, w_proj, b_proj, factors):
    global LAST_EXEC_NS, LAST_TRACE, _CACHED_NC
    from concourse.bass_utils import run_bass_kernel_spmd

    factors = np.asarray(factors, dtype=np.float32)
    G = _grid_g(factors)  # raw G

    w_qkv = np.asarray(w_qkv, dtype=np.float32)

    def pack8(w):
        # [768 out, 768 in] -> w^T scaled -> [p, (kt2 i out)] fp8
        wt = np.ascontiguousarray(w.T) * W_SCALE  # [in, out]
        return wt.reshape(3, 2, 128, DIM).transpose(2, 0, 1, 3).reshape(128, -1)

    def pack16(w):
        wt = np.ascontiguousarray(np.asarray(w, dtype=np.float32).T)
        return wt.reshape(6, 128, DIM).transpose(1, 0, 2).reshape(128, -1)

    gtp = np.zeros((256, N_TOK), dtype=np.float32)
    gtp[0:N_TOK] = G_SCALE * G.T
    in_common = {
        "gt": np.ascontiguousarray(
            gtp.reshape(2, 128, N_TOK).transpose(1, 0, 2).reshape(128, -1)
        ).astype(FP8),
        "wq8": np.ascontiguousarray(pack8(w_qkv[0:DIM])).astype(FP8),
        "wk8": np.ascontiguousarray(pack8(w_qkv[DIM:2 * DIM])).astype(FP8),
        "wv": np.ascontiguousarray(
            pack16(w_qkv[2 * DIM:3 * DIM]).reshape(128, 6, 2, 384)
            .transpose(0, 2, 1, 3).reshape(128, -1)).astype(BF16),
        "wp": np.ascontiguousarray(pack16(w_proj)).astype(BF16),
        "bias": np.asarray(b_proj, dtype=np.float32),
    }
    x = np.asarray(x, dtype=np.float32).reshape(BATCH * N_TOK, DIM)
    in_maps = []
    for c in range(N_CORES):
        xc = x[c * NTB:(c + 1) * NTB]  # [1568, 768] f32
        xcp = np.zeros((B_PER_CORE, 256, DIM), dtype=np.float32)
        xcp[:, 0:N_TOK] = xc.reshape(B_PER_CORE, N_TOK, DIM)
        x8p = xcp.reshape(B_PER_CORE, 2, 128, DIM) \
            .transpose(2, 0, 1, 3).reshape(128, -1)
        xtc = np.ascontiguousarray(xc.T)  # [768, 1568]
        # per-batch grouped: [p, (b kt2 i c)] and [p, (b kt c)]
        xt8p = xtc.reshape(3, 2, 128, B_PER_CORE, N_TOK) \
            .transpose(2, 3, 0, 1, 4).reshape(128, -1)
        xtp = xtc.reshape(6, 128, B_PER_CORE, N_TOK) \
            .transpose(1, 2, 0, 3).reshape(128, -1)
        in_maps.append({
            "x": np.ascontiguousarray(x8p).astype(FP8),
            "xt": np.ascontiguousarray(xtp).astype(BF16),
            "xt8": np.ascontiguousarray(xt8p).astype(FP8),
            **in_common,
        })

    if _CACHED_NC is None:
        _CACHED_NC = _build_bass()
    nc = _CACHED_NC

    trace = bool(int(os.environ.get("KERNEL_TRACE", "0")))
    res = run_bass_kernel_spmd(nc, in_maps, core_ids=list(range(N_CORES)),
                               trace=trace)
    LAST_EXEC_NS = res.exec_time_ns
    if res.instructions_and_trace is not None:
        LAST_TRACE = res.instructions_and_trace[1]
    out = np.concatenate([res.results[c]["out"] for c in range(N_CORES)], axis=0)
    return out.reshape(BATCH, N_TOK, DIM).astype(np.float32)
